# revision 1
# baseline (speedup 1.0000x reference)
"""Trainium2 Bass kernel for a dense transformer block (B=2, T=2048, C=1024,
H=16, DFF=4096), distributed over 8 NeuronCores.

Sharding: 2 batch groups x 4-way query-block sharding. Core c handles batch
g=c//4 and query blocks {j, 7-j} (j=c%4) of 8 blocks of 256 rows. K/V are
computed per-core for the full batch (replicated; no collectives). Causality
is exploited statically: key-chunks 0-7 are needed by both query blocks
(masked only on block-0's columns), chunks 8-15 only by the late block.
The data-dependent causal boundary is applied with per-core 0/1 masks so one
NEFF serves all 8 cores (SPMD).

Activations are kept feature-major ("xT") so every matmul chains without
transposes; layernorm runs row-major with PE transposes between domains.
Softmax denominators ride the AV matmul as an extra ones-column of V.
"""
import numpy as np
import ml_dtypes

import concourse.bass as bass
import concourse.mybir as mybir
import concourse.tile as tile
from concourse.vector_clock import ScopedClock
from concourse.bass_utils import run_bass_kernel_spmd
from concourse.masks import make_identity

bf16 = ml_dtypes.bfloat16
f32 = mybir.dt.float32
bt16 = mybir.dt.bfloat16
AF = mybir.ActivationFunctionType
OP = mybir.AluOpType

B, T, C, H, DH, DFF = 2, 2048, 1024, 16, 64, 4096
P = 128
QB = 256            # rows per query block
R = 512             # own query rows per core
RT = T + R          # ln1 rows per core (full batch + own q rows)
CC = C // P         # 8 feature chunks
MM = DFF // P       # 32 ffn chunks
EPS = 1e-5


# ---------------------------------------------------------------------------
# The walrus build in this container rejects instructions with >1 sync wait.
# Tile's sem assignment can emit several on one instruction; split the excess
# onto same-engine NoOps placed immediately before.
def _patched_drain_and_barrier(self, tick_clock, wait_clock):
    nc = self.nc
    probe = nc.sync.nop(nofuse=True, hint="tail_wait_probe")
    wait_clock.add_sem_waits(probe.ins, ScopedClock({None: tick_clock.global_clock}))
    si = probe.ins.sync_info
    waits = list(si.on_wait) if si is not None else []
    if si is not None:
        si.on_wait = waits[:1]
    for w in waits[1:]:
        n2 = nc.sync.nop(nofuse=True, hint="tail_wait_split")
        n2.ins.sync_info = mybir.SyncInfo(on_wait=[w], on_update=[])
    nc.sync.drain()
    nc.all_engine_barrier()
    assert self.sems is not None
    popped = nc._tile_sem_poison_stack.pop()
    assert popped is self._sem_poison
    nc.clear_and_free_semaphores(list(self.sems.allocated().values()))
    nc.all_engine_barrier()


tile.TileContext._drain_and_barrier = _patched_drain_and_barrier

_MAX_WAITS = 1
_split_counter = [0]


def _split_sync_waits(nc):
    for fn in nc.m.functions:
        for bb in fn.blocks:
            new_insts = []
            for inst in bb.instructions:
                si = getattr(inst, "sync_info", None)
                lim = _MAX_WAITS
                if si is not None and si.on_wait and len(si.on_wait) > lim:
                    waits = list(si.on_wait)
                    keep = waits[-lim:]
                    excess = waits[:-lim]
                    for i in range(0, len(excess), _MAX_WAITS):
                        _split_counter[0] += 1
                        nop = mybir.InstNoOp(
                            name=f"I-wsplit-{_split_counter[0]}", ins=[], outs=[])
                        nop.engine = inst.engine
                        nop.sync_info = mybir.SyncInfo(
                            on_wait=excess[i:i + _MAX_WAITS], on_update=[])
                        new_insts.append(nop)
                    si.on_wait = keep
                new_insts.append(inst)
            bb.instructions = new_insts
# ---------------------------------------------------------------------------


class Ctx:
    pass


def _layernorm_tile(g, xt, out_writes):
    """Row-major LN of xt [128, C] (in place), then transpose chunks and run
    out_writes(c, psum_ap) for each feature chunk c."""
    nc = g.nc
    st = g.stats.tile([P, 2, 6], f32, tag="bnst", name="bnst")
    xv = xt.rearrange("p (s d) -> p s d", s=2)
    for sg in range(2):
        nc.vector.bn_stats(out=st[:, sg, :], in_=xv[:, sg, :])
    mv = g.stats.tile([P, 2], f32, tag="bnmv", name="bnmv")
    nc.vector.bn_aggr(out=mv[:], in_=st[:])
    sq = g.stats.tile([P, 1], f32, tag="bnsq", name="bnsq")
    nc.scalar.activation(out=sq[:], in_=mv[:, 1:2], func=AF.Sqrt,
                         bias=g.eps_sb[:], scale=float(C) / (C - 1))
    rstd = g.stats.tile([P, 1], f32, tag="bnrstd", name="bnrstd")
    nc.vector.reciprocal(rstd[:], sq[:])
    nc.vector.tensor_scalar(out=xt[:], in0=xt[:], scalar1=mv[:, 0:1],
                            scalar2=rstd[:], op0=OP.subtract, op1=OP.mult)
    for c in range(CC):
        pt = g.ps.tile([P, 512], f32, tag="ps", name="ps_t")
        nc.tensor.transpose(pt[:P, :P], xt[:, c * P:(c + 1) * P], g.ident[:])
        out_writes(c, pt[:P, :P])


def _phase_a(g):
    """LN1 + transpose + Q/K/V projections (K/V over the full batch)."""
    nc, tc = g.nc, g.tc
    with tc.tile_pool(name="x1p", bufs=1) as x1p, \
         tc.tile_pool(name="xio", bufs=3) as xio, \
         tc.tile_pool(name="wvp", bufs=1) as wvp:
        # x1T split per 512-row block (rb 0-3 = batch, rb 4 = own q rows)
        x1T = [x1p.tile([P, CC, 512], bt16, tag=f"x1T{rb}", name=f"x1T{rb}")
               for rb in range(5)]
        for kt in range(T // P):
            nc.vector.memset(g.vv[kt][:, :, DH:DH + 1], 1.0)

        for rt in range(RT // P):
            rb, r0 = rt // 4, (rt % 4) * P
            xt = xio.tile([P, C], f32, tag="xin", name="xin")
            nc.sync.dma_start(xt[:], g.xc[rt * P:(rt + 1) * P, :])

            def wr1(c, pt, rb=rb, r0=r0, rt=rt):
                nc.scalar.activation(
                    out=x1T[rb][:, c, r0:r0 + P], in_=pt, func=AF.Identity,
                    bias=g.be1s[:, c:c + 1], scale=g.g1s[:, c:c + 1])
                if rt >= T // P:
                    q0 = (rt - T // P) * P
                    nc.scalar.activation(
                        out=g.x1f[c][:, q0:q0 + P], in_=pt, func=AF.Identity,
                        bias=g.be1s[:, c:c + 1], scale=g.g1s[:, c:c + 1])
            _layernorm_tile(g, xt, wr1)

        # K^T and Q^T projections (feature-major out, weights as lhsT)
        for m in range(CC):
            wkm = g.wstr.tile([P, CC, P], bt16, tag="wstr", name="wkm")
            nc.sync.dma_start(
                wkm[:, :, :],
                g.wk[:, m * P:(m + 1) * P].rearrange("(c p) f -> p c f", p=P))
            for rb in range(4):
                pk = g.ps.tile([P, 512], f32, tag="ps", name="ps_k")
                for c in range(CC):
                    nc.tensor.matmul(pk[:], wkm[:, c, :], x1T[rb][:, c, :],
                                     start=(c == 0), stop=(c == CC - 1))
                nc.vector.tensor_scalar(
                    out=g.kT[m][rb][:, :], in0=pk[:],
                    scalar1=g.sb_vec["bk"][:, m:m + 1], scalar2=None, op0=OP.add)
            wqm = g.wstr.tile([P, CC, P], bt16, tag="wstr", name="wqm")
            nc.sync.dma_start(
                wqm[:, :, :],
                g.wq[:, m * P:(m + 1) * P].rearrange("(c p) f -> p c f", p=P))
            pq = g.ps.tile([P, 512], f32, tag="ps", name="ps_q")
            for c in range(CC):
                nc.tensor.matmul(pq[:], wqm[:, c, :], x1T[4][:, c, :],
                                 start=(c == 0), stop=(c == CC - 1))
            nc.vector.tensor_scalar(
                out=g.qT[m][:, :], in0=pq[:],
                scalar1=g.sb_vec["bq"][:, m:m + 1], scalar2=None, op0=OP.add)

        # V row-major (keys on partitions): lhsT = x1T chunk, rhs = wv
        wvs = wvp.tile([P, CC, C], bt16, tag="wvs", name="wvs")
        nc.sync.dma_start(wvs[:], g.wv.rearrange("(c p) f -> p c f", p=P))
        for kt in range(T // P):
            rb, r0 = kt // 4, (kt % 4) * P
            for half in range(2):
                pv = g.ps.tile([P, 512], f32, tag="ps", name="ps_v")
                for c in range(CC):
                    nc.tensor.matmul(pv[:], x1T[rb][:, c, r0:r0 + P],
                                     wvs[:, c, half * 512:(half + 1) * 512],
                                     start=(c == 0), stop=(c == CC - 1))
                nc.vector.tensor_copy(
                    out=g.vv[kt][:, half * 8:(half + 1) * 8, 0:DH],
                    in_=pv.rearrange("p (h d) -> p h d", h=8))


def _phase_b(g):
    """Attention, both query blocks fused on the free dim (cols 0:256 = early
    block, 256:512 = late block). Key-chunks 0-7 feed both blocks (one N=512
    matmul); chunks 8-15 feed only the late block (N=256)."""
    nc, tc = g.nc, g.tc
    with tc.tile_pool(name="mp", bufs=1) as mp, \
         tc.tile_pool(name="apl", bufs=3) as apl:
        mq = mp.tile([P, 16, QB], bt16, tag="mask", name="mask")
        nc.sync.dma_start(mq[:], g.masks)
        for pair in range(CC):
            for hl in range(2):
                h = 2 * pair + hl
                hs = slice(hl * DH, (hl + 1) * DH)
                aA = apl.tile([P, 8, 512], bt16, tag="aA", name="aA")
                aB = apl.tile([P, 8, QB], bt16, tag="aB", name="aB")
                for kc in range(16):
                    rb, k0 = kc // 4, (kc % 4) * P
                    psc = g.ps.tile([P, 512], f32, tag="ps", name="ps_s")
                    n = 512 if kc < 8 else QB
                    q0 = 0 if kc < 8 else QB
                    nc.tensor.matmul(
                        psc[:, :n], g.kT[pair][rb][hs, k0:k0 + P],
                        g.qT[pair][hs, q0:512],
                        start=True, stop=True, tile_position=(hl * DH, 0))
                    if kc < 8:
                        nc.scalar.activation(out=aA[:, kc, :], in_=psc[:, :512],
                                             func=AF.Exp)
                        nc.vector.tensor_mul(aA[:, kc, 0:QB], aA[:, kc, 0:QB],
                                             mq[:, kc, :])
                    else:
                        nc.scalar.activation(out=aB[:, kc - 8, :],
                                             in_=psc[:, :QB], func=AF.Exp)
                        nc.vector.tensor_mul(aB[:, kc - 8, :], aB[:, kc - 8, :],
                                             mq[:, kc, :])
                pav = g.ps.tile([P, 512], f32, tag="ps", name="ps_av")
                for kc in range(8):
                    nc.tensor.matmul(pav[:DH + 1, :512], g.vv[kc][:, h, :],
                                     aA[:, kc, :], start=(kc == 0), stop=False)
                for kc in range(8, 16):
                    nc.tensor.matmul(pav[:DH + 1, QB:512], g.vv[kc][:, h, :],
                                     aB[:, kc - 8, :], start=False,
                                     stop=(kc == 15))
                den = g.stats.tile([1, 512], f32, tag="den", name="den")
                nc.vector.tensor_copy(den[:], pav[DH:DH + 1, :512])
                rr = g.stats.tile([1, 512], f32, tag="rr", name="rr")
                nc.vector.reciprocal(rr[:], den[:])
                prb = g.ps.tile([P, 512], f32, tag="ps", name="ps_r")
                nc.tensor.matmul(prb[:DH, :512], g.ones64[:], rr[:],
                                 start=True, stop=True)
                rbc = g.stats.tile([DH, 512], f32, tag="rbc", name="rbc")
                nc.vector.tensor_copy(rbc[:], prb[:DH, :512])
                nc.vector.tensor_mul(out=g.hcat[pair][hs, :],
                                     in0=pav[:DH, :512], in1=rbc[:])


def _phase_cd(g):
    """Wo + residual + LN2 + FFN + output."""
    nc, tc = g.nc, g.tc
    with tc.tile_pool(name="cp", bufs=3) as cp, \
         tc.tile_pool(name="wop", bufs=1) as wop, \
         tc.tile_pool(name="x3p", bufs=1) as x3p:
        wos = wop.tile([P, CC, C], bt16, tag="wos", name="wos")
        nc.sync.dma_start(wos[:], g.wo.rearrange("(c p) f -> p c f", p=P))
        x2T = [x3p.tile([P, 512], f32, tag=f"x2T{m}", name=f"x2T{m}")
               for m in range(CC)]
        for m in range(CC):
            pa = g.ps.tile([P, 512], f32, tag="ps", name="ps_o")
            for c in range(CC):
                nc.tensor.matmul(pa[:, :512], wos[:, c, m * P:(m + 1) * P],
                                 g.hcat[c][:, :], start=(c == 0),
                                 stop=(c == CC - 1))
            nc.vector.scalar_tensor_tensor(
                out=x2T[m][:, :], in0=pa[:, :512],
                scalar=g.sb_vec["bo"][:, m:m + 1], in1=g.x1f[m][:, :],
                op0=OP.add, op1=OP.add)

        x3Tb = [x3p.tile([P, 512], bt16, tag=f"x3Tb{c}", name=f"x3Tb{c}")
                for c in range(CC)]
        x3Tf = [x3p.tile([P, 512], f32, tag=f"x3Tf{c}", name=f"x3Tf{c}")
                for c in range(CC)]
        for rt in range(R // P):
            x2r = cp.tile([P, C], f32, tag="x2r", name="x2r")
            for c in range(CC):
                pt = g.ps.tile([P, 512], f32, tag="ps", name="ps_t2")
                nc.tensor.transpose(pt[:P, :P], x2T[c][:, rt * P:(rt + 1) * P],
                                    g.ident[:])
                nc.scalar.copy(out=x2r[:, c * P:(c + 1) * P], in_=pt[:P, :P])

            def wr2(c, pt, rt=rt):
                nc.scalar.activation(
                    out=x3Tb[c][:, rt * P:(rt + 1) * P], in_=pt,
                    func=AF.Identity, bias=g.be2s[:, c:c + 1],
                    scale=g.g2s[:, c:c + 1])
                nc.scalar.activation(
                    out=x3Tf[c][:, rt * P:(rt + 1) * P], in_=pt,
                    func=AF.Identity, bias=g.be2s[:, c:c + 1],
                    scale=g.g2s[:, c:c + 1])
            _layernorm_tile(g, x2r, wr2)

        # FFN
        with tc.tile_pool(name="dp", bufs=1) as dp:
            h1 = [dp.tile([P, 512], bt16, tag=f"h1_{m}", name=f"h1_{m}")
                  for m in range(MM)]
            for m in range(MM):
                w1m = g.wstr.tile([P, CC, P], bt16, tag="wstr", name="w1m")
                nc.sync.dma_start(
                    w1m[:],
                    g.w1[:, m * P:(m + 1) * P].rearrange("(c p) f -> p c f", p=P))
                p1 = g.ps.tile([P, 512], f32, tag="ps", name="ps_f1")
                for c in range(CC):
                    nc.tensor.matmul(p1[:], w1m[:, c, :], x3Tb[c][:, :],
                                     start=(c == 0), stop=(c == CC - 1))
                nc.scalar.activation(out=h1[m][:, :], in_=p1[:], func=AF.Gelu,
                                     bias=g.sb_vec["b1"][:, m:m + 1], scale=1.0)
            for oc in range(CC):
                w2m = g.wstr.tile([P, MM, P], bt16, tag="wstr", name="w2m")
                nc.sync.dma_start(
                    w2m[:],
                    g.w2[:, oc * P:(oc + 1) * P].rearrange("(k p) f -> p k f", p=P))
                p2 = g.ps.tile([P, 512], f32, tag="ps", name="ps_f2")
                for k in range(MM):
                    nc.tensor.matmul(p2[:], w2m[:, k, :], h1[k][:, :],
                                     start=(k == 0), stop=(k == MM - 1))
                ot = cp.tile([P, R], f32, tag="otile", name="otile")
                nc.vector.scalar_tensor_tensor(
                    out=ot[:], in0=p2[:], scalar=g.sb_vec["b2"][:, oc:oc + 1],
                    in1=x3Tf[oc][:, :], op0=OP.add, op1=OP.add)
                nc.sync.dma_start(g.out[oc], ot[:])


def build_kernel():
    nc = bass.Bass("TRN2", target_bir_lowering=False, num_devices=8)
    g = Ctx()
    g.nc = nc

    g.xc = nc.dram_tensor("xc", [RT, C], f32, kind="ExternalInput").ap()
    g.wq = nc.dram_tensor("wq", [C, C], bt16, kind="ExternalInput").ap()
    g.wk = nc.dram_tensor("wk", [C, C], bt16, kind="ExternalInput").ap()
    g.wv = nc.dram_tensor("wv", [C, C], bt16, kind="ExternalInput").ap()
    g.wo = nc.dram_tensor("wo", [C, C], bt16, kind="ExternalInput").ap()
    g.w1 = nc.dram_tensor("w1", [C, DFF], bt16, kind="ExternalInput").ap()
    g.w2 = nc.dram_tensor("w2", [DFF, C], bt16, kind="ExternalInput").ap()
    g.masks = nc.dram_tensor("masks", [P, 16, QB], bt16,
                             kind="ExternalInput").ap()
    vecs = {}
    for nm, n in [("bq", CC), ("bk", CC), ("bo", CC), ("b1", MM),
                  ("b2", CC), ("g1", CC), ("be1", CC), ("g2", CC), ("be2", CC)]:
        vecs[nm] = nc.dram_tensor(nm, [n, P], f32, kind="ExternalInput").ap()
    g.out = nc.dram_tensor("out", [CC, P, R], f32, kind="ExternalOutput").ap()

    with tile.TileContext(nc) as tc:
        g.tc = tc
        with tc.tile_pool(name="setup", bufs=1) as setup, \
             tc.tile_pool(name="stats", bufs=4) as stats, \
             tc.tile_pool(name="ps", bufs=8, space="PSUM") as ps, \
             tc.tile_pool(name="wstr", bufs=2) as wstr, \
             tc.tile_pool(name="x1f_p", bufs=1) as x1f_p, \
             tc.tile_pool(name="hp", bufs=1) as hp:
            g.stats, g.ps, g.wstr = stats, ps, wstr

            ident = setup.tile([P, P], f32, tag="ident", name="ident")
            make_identity(nc, ident[:])
            g.ident = ident
            g.ones64 = setup.tile([1, DH], f32, tag="ones64", name="ones64")
            nc.vector.memset(g.ones64[:], 1.0)
            g.eps_sb = setup.tile([P, 1], f32, tag="eps", name="eps")
            nc.vector.memset(g.eps_sb[:], EPS)
            g.sb_vec = {}
            for nm, ap_ in vecs.items():
                n = ap_.shape[0]
                t = setup.tile([P, n], f32, tag=f"vec_{nm}", name=f"vec_{nm}")
                nc.sync.dma_start(t[:], ap_.rearrange("c p -> p c"))
                g.sb_vec[nm] = t
            g.g1s, g.be1s = g.sb_vec["g1"], g.sb_vec["be1"]
            g.g2s, g.be2s = g.sb_vec["g2"], g.sb_vec["be2"]

            g.x1f = [x1f_p.tile([P, 512], f32, tag=f"x1f{c}", name=f"x1f{c}")
                     for c in range(CC)]
            g.hcat = [hp.tile([P, 512], bt16, tag=f"hcat{c}", name=f"hcat{c}")
                      for c in range(CC)]

            with tc.tile_pool(name="kvp", bufs=1) as kvp:
                g.kT = [[kvp.tile([P, 512], bt16, tag=f"kT{m}_{rb}",
                                  name=f"kT{m}_{rb}") for rb in range(4)]
                        for m in range(CC)]
                g.vv = [kvp.tile([P, H, DH + 1], bt16, tag=f"vv{kt}",
                                 name=f"vv{kt}") for kt in range(T // P)]
                g.qT = [kvp.tile([P, 512], bt16, tag=f"qT{m}", name=f"qT{m}")
                        for m in range(CC)]
                _phase_a(g)
                _phase_b(g)
            _phase_cd(g)
    _split_sync_waits(nc)
    return nc


_NC_CACHE = None


def _get_nc():
    global _NC_CACHE
    if _NC_CACHE is None:
        _NC_CACHE = build_kernel()
    return _NC_CACHE


def _prep_shared(inputs):
    scale = DH ** -0.5
    Wq = np.asarray(inputs["Wq"], np.float32)
    Wk = np.asarray(inputs["Wk"], np.float32)
    Wv = np.asarray(inputs["Wv"], np.float32)
    Wo = np.asarray(inputs["Wo"], np.float32)
    W1 = np.asarray(inputs["W1"], np.float32)
    W2 = np.asarray(inputs["W2"], np.float32)
    bv_c = np.asarray(inputs["bv"], np.float32).reshape(C)
    shared = {
        "wq": np.ascontiguousarray(
            Wq.transpose(1, 0, 2).reshape(C, C) * scale).astype(bf16),
        "wk": np.ascontiguousarray(
            Wk.transpose(1, 0, 2).reshape(C, C)).astype(bf16),
        "wv": np.ascontiguousarray(
            Wv.transpose(1, 0, 2).reshape(C, C)).astype(bf16),
        "wo": Wo.astype(bf16),
        "w1": W1.astype(bf16),
        "w2": W2.astype(bf16),
        "bq": (np.asarray(inputs["bq"], np.float32).reshape(C) * scale
               ).reshape(CC, P).copy(),
        "bk": np.asarray(inputs["bk"], np.float32).reshape(CC, P).copy(),
        "bo": (np.asarray(inputs["bo"], np.float32) + bv_c @ Wo
               ).reshape(CC, P).copy(),
        "b1": np.asarray(inputs["b1"], np.float32).reshape(MM, P).copy(),
        "b2": np.asarray(inputs["b2"], np.float32).reshape(CC, P).copy(),
        "g1": np.asarray(inputs["gamma1"], np.float32).reshape(CC, P).copy(),
        "be1": np.asarray(inputs["beta1"], np.float32).reshape(CC, P).copy(),
        "g2": np.asarray(inputs["gamma2"], np.float32).reshape(CC, P).copy(),
        "be2": np.asarray(inputs["beta2"], np.float32).reshape(CC, P).copy(),
    }
    return shared


def _core_masks(j):
    """[128, 16, 256] bf16 0/1 masks. kc 0-7 mask the early block's columns
    (block j); kc 8-15 mask the late block's columns (block 7-j)."""
    out = np.zeros((P, 16, QB), np.float32)
    for kc in range(16):
        b = j if kc < 8 else 7 - j
        key = kc * P + np.arange(P)[:, None]          # [128, 1]
        qglob = b * QB + np.arange(QB)[None, :]       # [1, 256]
        out[:, kc, :] = (key <= qglob)
    return out.astype(bf16)


def _make_in_maps(inputs):
    x = np.asarray(inputs["x"], np.float32)
    shared = _prep_shared(inputs)
    in_maps = []
    for c in range(8):
        gg, j = c // 4, c % 4
        xb = x[gg]
        xq = np.concatenate([xb[j * QB:(j + 1) * QB],
                             xb[(7 - j) * QB:(8 - j) * QB]], 0)
        m = dict(shared)
        m["xc"] = np.ascontiguousarray(np.concatenate([xb, xq], 0))
        m["masks"] = _core_masks(j)
        in_maps.append(m)
    return in_maps


def _assemble(results):
    out = np.zeros((B, T, C), np.float32)
    for c in range(8):
        gg, j = c // 4, c % 4
        o = results[c]["out"].reshape(C, R).T  # [512, C] rows = 2 blocks
        out[gg, j * QB:(j + 1) * QB] = o[:QB]
        out[gg, (7 - j) * QB:(8 - j) * QB] = o[QB:]
    return out


def kernel(**inputs):
    in_maps = _make_in_maps(inputs)
    nc = _get_nc()
    res = run_bass_kernel_spmd(nc, in_maps, core_ids=list(range(8)))
    return _assemble(res.results)



# revision 12
# speedup vs baseline: 1.0960x; 1.0960x over previous
"""Trainium2 Bass kernel for a dense transformer block (B=2, T=2048, C=1024,
H=16, DFF=4096), distributed over 8 NeuronCores.

Sharding: 2 batch groups x 4-way query-block sharding. Core c handles batch
g=c//4 and query blocks {j, 7-j} (j=c%4) of 8 blocks of 256 rows. K/V are
computed per-core for the full batch (replicated; no collectives). Causality
is exploited statically: key-chunks 0-7 are needed by both query blocks
(masked only on block-0's columns), chunks 8-15 only by the late block.
The data-dependent causal boundary is applied with per-core 0/1 masks so one
NEFF serves all 8 cores (SPMD).

Activations are kept feature-major ("xT") so every matmul chains without
transposes; layernorm runs row-major with PE transposes between domains.
Softmax denominators ride the AV matmul as an extra ones-column of V.
"""
import numpy as np
import ml_dtypes

import concourse.bass as bass
import concourse.mybir as mybir
import concourse.tile as tile
from concourse.vector_clock import ScopedClock
from concourse.bass_utils import run_bass_kernel_spmd
from concourse.masks import make_identity

bf16 = ml_dtypes.bfloat16
f32 = mybir.dt.float32
bt16 = mybir.dt.bfloat16
AF = mybir.ActivationFunctionType
OP = mybir.AluOpType

B, T, C, H, DH, DFF = 2, 2048, 1024, 16, 64, 4096
P = 128
QB = 256            # rows per query block
R = 512             # own query rows per core
RT = T + R          # ln1 rows per core (full batch + own q rows)
CC = C // P         # 8 feature chunks
MM = DFF // P       # 32 ffn chunks
EPS = 1e-5


# ---------------------------------------------------------------------------
# The walrus build in this container rejects instructions with >1 sync wait.
# Tile's sem assignment can emit several on one instruction; split the excess
# onto same-engine NoOps placed immediately before.
def _patched_drain_and_barrier(self, tick_clock, wait_clock):
    nc = self.nc
    probe = nc.sync.nop(nofuse=True, hint="tail_wait_probe")
    wait_clock.add_sem_waits(probe.ins, ScopedClock({None: tick_clock.global_clock}))
    si = probe.ins.sync_info
    waits = list(si.on_wait) if si is not None else []
    if si is not None:
        si.on_wait = waits[:1]
    for w in waits[1:]:
        n2 = nc.sync.nop(nofuse=True, hint="tail_wait_split")
        n2.ins.sync_info = mybir.SyncInfo(on_wait=[w], on_update=[])
    nc.sync.drain()
    nc.all_engine_barrier()
    assert self.sems is not None
    popped = nc._tile_sem_poison_stack.pop()
    assert popped is self._sem_poison
    nc.clear_and_free_semaphores(list(self.sems.allocated().values()))
    nc.all_engine_barrier()


tile.TileContext._drain_and_barrier = _patched_drain_and_barrier

_MAX_WAITS = 1
_split_counter = [0]


def _split_sync_waits(nc):
    for fn in nc.m.functions:
        for bb in fn.blocks:
            new_insts = []
            for inst in bb.instructions:
                si = getattr(inst, "sync_info", None)
                lim = _MAX_WAITS
                if si is not None and si.on_wait and len(si.on_wait) > lim:
                    waits = list(si.on_wait)
                    keep = waits[-lim:]
                    excess = waits[:-lim]
                    for i in range(0, len(excess), _MAX_WAITS):
                        _split_counter[0] += 1
                        nop = mybir.InstNoOp(
                            name=f"I-wsplit-{_split_counter[0]}", ins=[], outs=[])
                        nop.engine = inst.engine
                        nop.sync_info = mybir.SyncInfo(
                            on_wait=excess[i:i + _MAX_WAITS], on_update=[])
                        new_insts.append(nop)
                    si.on_wait = keep
                new_insts.append(inst)
            bb.instructions = new_insts
# ---------------------------------------------------------------------------


class Ctx:
    pass


def _layernorm_tile(g, xt, out_writes):
    """Row-major LN of xt [128, C] (in place), then transpose chunks and run
    out_writes(c, psum_ap) for each feature chunk c."""
    nc = g.nc
    st = g.stats.tile([P, 2, 6], f32, tag="bnst", name="bnst")
    xv = xt.rearrange("p (s d) -> p s d", s=2)
    for sg in range(2):
        nc.vector.bn_stats(out=st[:, sg, :], in_=xv[:, sg, :])
    mv = g.stats.tile([P, 2], f32, tag="bnmv", name="bnmv")
    nc.vector.bn_aggr(out=mv[:], in_=st[:])
    sq = g.stats.tile([P, 1], f32, tag="bnsq", name="bnsq")
    nc.scalar.activation(out=sq[:], in_=mv[:, 1:2], func=AF.Sqrt,
                         bias=g.eps_sb[:], scale=float(C) / (C - 1))
    rstd = g.stats.tile([P, 1], f32, tag="bnrstd", name="bnrstd")
    nc.vector.reciprocal(rstd[:], sq[:])
    nc.vector.tensor_scalar(out=xt[:], in0=xt[:], scalar1=mv[:, 0:1],
                            scalar2=rstd[:], op0=OP.subtract, op1=OP.mult)
    for c in range(CC):
        pt = g.ps.tile([P, 512], f32, tag="ps", name="ps_t")
        nc.tensor.transpose(pt[:P, :P], xt[:, c * P:(c + 1) * P], g.ident[:])
        out_writes(c, pt[:P, :P])


def _phase_a(g):
    """LN1 + transpose + Q/K/V projections (K/V over the full batch)."""
    nc, tc = g.nc, g.tc
    with tc.tile_pool(name="x1p", bufs=1) as x1p, \
         tc.tile_pool(name="xio", bufs=3) as xio, \
         tc.tile_pool(name="wvp", bufs=1) as wvp:
        # x1T split per 512-row block (rb 0-3 = batch, rb 4 = own q rows)
        x1T = [x1p.tile([P, CC, 512], bt16, tag=f"x1T{rb}", name=f"x1T{rb}")
               for rb in range(5)]
        for kt in range(T // P):
            nc.vector.memset(g.vv[kt][:, :, DH:DH + 1], 1.0)

        for rt in range(RT // P):
            rb, r0 = rt // 4, (rt % 4) * P
            xt = xio.tile([P, C], f32, tag="xin", name="xin")
            nc.sync.dma_start(xt[:], g.xc[rt * P:(rt + 1) * P, :])

            def wr1(c, pt, rb=rb, r0=r0, rt=rt):
                nc.scalar.activation(
                    out=x1T[rb][:, c, r0:r0 + P], in_=pt, func=AF.Identity,
                    bias=g.be1s[:, c:c + 1], scale=g.g1s[:, c:c + 1])
                if rt >= T // P:
                    q0 = (rt - T // P) * P
                    nc.scalar.activation(
                        out=g.x1f[c][:, q0:q0 + P], in_=pt, func=AF.Identity,
                        bias=g.be1s[:, c:c + 1], scale=g.g1s[:, c:c + 1])
            _layernorm_tile(g, xt, wr1)

        # K^T and Q^T projections (feature-major out, weights as lhsT)
        for m in range(CC):
            wkm = g.wstr.tile([P, CC, P], bt16, tag="wstr", name="wkm")
            nc.gpsimd.dma_start(wkm[:, :, :], g.wk[m])
            for rb in range(4):
                pk = g.ps.tile([P, 512], f32, tag="ps", name="ps_k")
                for c in range(CC):
                    nc.tensor.matmul(pk[:], wkm[:, c, :], x1T[rb][:, c, :],
                                     start=(c == 0), stop=(c == CC - 1))
                nc.vector.tensor_scalar(
                    out=g.kT[m][rb][:, :], in0=pk[:],
                    scalar1=g.sb_vec["bk"][:, m:m + 1], scalar2=None, op0=OP.add)
            wqm = g.wstr.tile([P, CC, P], bt16, tag="wstr", name="wqm")
            nc.gpsimd.dma_start(wqm[:, :, :], g.wq[m])
            pq = g.ps.tile([P, 512], f32, tag="ps", name="ps_q")
            for c in range(CC):
                nc.tensor.matmul(pq[:], wqm[:, c, :], x1T[4][:, c, :],
                                 start=(c == 0), stop=(c == CC - 1))
            nc.vector.tensor_scalar(
                out=g.qT[m][:, :], in0=pq[:],
                scalar1=g.sb_vec["bq"][:, m:m + 1], scalar2=None, op0=OP.add)

        # V row-major (keys on partitions): lhsT = x1T chunk, rhs = wv
        wvs = wvp.tile([P, CC, C], bt16, tag="wvs", name="wvs")
        nc.gpsimd.dma_start(wvs[:], g.wv)
        for kt in range(T // P):
            rb, r0 = kt // 4, (kt % 4) * P
            for half in range(2):
                pv = g.ps.tile([P, 512], f32, tag="ps", name="ps_v")
                for c in range(CC):
                    nc.tensor.matmul(pv[:], x1T[rb][:, c, r0:r0 + P],
                                     wvs[:, c, half * 512:(half + 1) * 512],
                                     start=(c == 0), stop=(c == CC - 1))
                nc.vector.tensor_copy(
                    out=g.vv[kt][:, half * 8:(half + 1) * 8, 0:DH],
                    in_=pv.rearrange("p (h d) -> p h d", h=8))


def _phase_b(g):
    """Attention, both query blocks fused on the free dim (cols 0:256 = early
    block, 256:512 = late block). Key-chunks 0-7 feed both blocks (one N=512
    matmul); chunks 8-15 feed only the late block (N=256)."""
    nc, tc = g.nc, g.tc
    with tc.tile_pool(name="mp", bufs=1) as mp, \
         tc.tile_pool(name="apl", bufs=3) as apl:
        mq = mp.tile([P, 16, QB], bt16, tag="mask", name="mask")
        nc.sync.dma_start(mq[:], g.masks)
        for pair in range(CC):
            for hl in range(2):
                h = 2 * pair + hl
                hs = slice(hl * DH, (hl + 1) * DH)
                aA = apl.tile([P, 8, 512], bt16, tag="aA", name="aA")
                aB = apl.tile([P, 8, QB], bt16, tag="aB", name="aB")
                for kc in range(16):
                    rb, k0 = kc // 4, (kc % 4) * P
                    psc = g.ps.tile([P, 512], f32, tag="ps", name="ps_s")
                    n = 512 if kc < 8 else QB
                    q0 = 0 if kc < 8 else QB
                    nc.tensor.matmul(
                        psc[:, :n], g.kT[pair][rb][hs, k0:k0 + P],
                        g.qT[pair][hs, q0:512],
                        start=True, stop=True, tile_position=(hl * DH, 0))
                    if kc < 8:
                        nc.scalar.activation(out=aA[:, kc, :], in_=psc[:, :512],
                                             func=AF.Exp)
                        nc.vector.tensor_mul(aA[:, kc, 0:QB], aA[:, kc, 0:QB],
                                             mq[:, kc, :])
                    else:
                        nc.scalar.activation(out=aB[:, kc - 8, :],
                                             in_=psc[:, :QB], func=AF.Exp)
                        nc.vector.tensor_mul(aB[:, kc - 8, :], aB[:, kc - 8, :],
                                             mq[:, kc, :])
                pav = g.ps.tile([P, 512], f32, tag="ps", name="ps_av")
                for kc in range(8):
                    nc.tensor.matmul(pav[:DH + 1, :512], g.vv[kc][:, h, :],
                                     aA[:, kc, :], start=(kc == 0), stop=False)
                for kc in range(8, 16):
                    nc.tensor.matmul(pav[:DH + 1, QB:512], g.vv[kc][:, h, :],
                                     aB[:, kc - 8, :], start=False,
                                     stop=(kc == 15))
                den = g.stats.tile([1, 512], f32, tag="den", name="den")
                nc.vector.tensor_copy(den[:], pav[DH:DH + 1, :512])
                rr = g.stats.tile([1, 512], f32, tag="rr", name="rr")
                nc.vector.reciprocal(rr[:], den[:])
                prb = g.ps.tile([P, 512], f32, tag="ps", name="ps_r")
                nc.tensor.matmul(prb[:DH, :512], g.ones64[:], rr[:],
                                 start=True, stop=True)
                rbc = g.stats.tile([DH, 512], f32, tag="rbc", name="rbc")
                nc.vector.tensor_copy(rbc[:], prb[:DH, :512])
                nc.vector.tensor_mul(out=g.hcat[pair][hs, :],
                                     in0=pav[:DH, :512], in1=rbc[:])


def _phase_cd(g):
    """Wo + residual + LN2 + FFN + output."""
    nc, tc = g.nc, g.tc
    with tc.tile_pool(name="cp", bufs=3) as cp, \
         tc.tile_pool(name="wop", bufs=1) as wop, \
         tc.tile_pool(name="x3p", bufs=1) as x3p:
        wos = wop.tile([P, CC, C], bt16, tag="wos", name="wos")
        nc.gpsimd.dma_start(wos[:], g.wo)
        x2T = [x3p.tile([P, 512], f32, tag=f"x2T{m}", name=f"x2T{m}")
               for m in range(CC)]
        for m in range(CC):
            pa = g.ps.tile([P, 512], f32, tag="ps", name="ps_o")
            for c in range(CC):
                nc.tensor.matmul(pa[:, :512], wos[:, c, m * P:(m + 1) * P],
                                 g.hcat[c][:, :], start=(c == 0),
                                 stop=(c == CC - 1))
            nc.vector.scalar_tensor_tensor(
                out=x2T[m][:, :], in0=pa[:, :512],
                scalar=g.sb_vec["bo"][:, m:m + 1], in1=g.x1f[m][:, :],
                op0=OP.add, op1=OP.add)

        x3Tb = [x3p.tile([P, 512], bt16, tag=f"x3Tb{c}", name=f"x3Tb{c}")
                for c in range(CC)]
        x3Tf = [x3p.tile([P, 512], f32, tag=f"x3Tf{c}", name=f"x3Tf{c}")
                for c in range(CC)]
        for rt in range(R // P):
            x2r = cp.tile([P, C], f32, tag="x2r", name="x2r")
            for c in range(CC):
                pt = g.ps.tile([P, 512], f32, tag="ps", name="ps_t2")
                nc.tensor.transpose(pt[:P, :P], x2T[c][:, rt * P:(rt + 1) * P],
                                    g.ident[:])
                nc.scalar.copy(out=x2r[:, c * P:(c + 1) * P], in_=pt[:P, :P])

            def wr2(c, pt, rt=rt):
                nc.scalar.activation(
                    out=x3Tb[c][:, rt * P:(rt + 1) * P], in_=pt,
                    func=AF.Identity, bias=g.be2s[:, c:c + 1],
                    scale=g.g2s[:, c:c + 1])
                nc.scalar.activation(
                    out=x3Tf[c][:, rt * P:(rt + 1) * P], in_=pt,
                    func=AF.Identity, bias=g.be2s[:, c:c + 1],
                    scale=g.g2s[:, c:c + 1])
            _layernorm_tile(g, x2r, wr2)

        # FFN
        with tc.tile_pool(name="dp", bufs=1) as dp:
            h1 = [dp.tile([P, 512], bt16, tag=f"h1_{m}", name=f"h1_{m}")
                  for m in range(MM)]
            for m in range(MM):
                w1m = g.wstr.tile([P, CC, P], bt16, tag="wstr", name="w1m")
                nc.gpsimd.dma_start(w1m[:], g.w1[m])
                p1 = g.ps.tile([P, 512], f32, tag="ps", name="ps_f1")
                for c in range(CC):
                    nc.tensor.matmul(p1[:], w1m[:, c, :], x3Tb[c][:, :],
                                     start=(c == 0), stop=(c == CC - 1))
                nc.scalar.activation(out=h1[m][:, :], in_=p1[:], func=AF.Gelu,
                                     bias=g.sb_vec["b1"][:, m:m + 1], scale=1.0)
            for oc in range(CC):
                w2a = g.wstr.tile([P, MM // 2, P], bt16, tag="wstr", name="w2a")
                nc.gpsimd.dma_start(w2a[:], g.w2[oc, :, :MM // 2])
                w2b = g.wstr.tile([P, MM // 2, P], bt16, tag="wstr", name="w2b")
                nc.gpsimd.dma_start(w2b[:], g.w2[oc, :, MM // 2:])
                p2 = g.ps.tile([P, 512], f32, tag="ps", name="ps_f2")
                for k in range(MM):
                    wt = w2a if k < MM // 2 else w2b
                    nc.tensor.matmul(p2[:], wt[:, k % (MM // 2), :], h1[k][:, :],
                                     start=(k == 0), stop=(k == MM - 1))
                ot = cp.tile([P, R], f32, tag="otile", name="otile")
                nc.vector.scalar_tensor_tensor(
                    out=ot[:], in0=p2[:], scalar=g.sb_vec["b2"][:, oc:oc + 1],
                    in1=x3Tf[oc][:, :], op0=OP.add, op1=OP.add)
                nc.sync.dma_start(g.out[oc], ot[:])


def build_kernel():
    nc = bass.Bass("TRN2", target_bir_lowering=False, num_devices=8)
    g = Ctx()
    g.nc = nc

    # Weight layouts match the SBUF tile layout exactly so every DMA is
    # contiguous per partition (>=512B elements run at full DMA rate).
    g.xc = nc.dram_tensor("xc", [RT, C], f32, kind="ExternalInput").ap()
    g.wq = nc.dram_tensor("wq", [CC, P, CC, P], bt16, kind="ExternalInput").ap()
    g.wk = nc.dram_tensor("wk", [CC, P, CC, P], bt16, kind="ExternalInput").ap()
    g.wv = nc.dram_tensor("wv", [P, CC, C], bt16, kind="ExternalInput").ap()
    g.wo = nc.dram_tensor("wo", [P, CC, C], bt16, kind="ExternalInput").ap()
    g.w1 = nc.dram_tensor("w1", [MM, P, CC, P], bt16, kind="ExternalInput").ap()
    g.w2 = nc.dram_tensor("w2", [CC, P, MM, P], bt16, kind="ExternalInput").ap()
    g.masks = nc.dram_tensor("masks", [P, 16, QB], bt16,
                             kind="ExternalInput").ap()
    vecs = {}
    for nm, n in [("bq", CC), ("bk", CC), ("bo", CC), ("b1", MM),
                  ("b2", CC), ("g1", CC), ("be1", CC), ("g2", CC), ("be2", CC)]:
        vecs[nm] = nc.dram_tensor(nm, [P, n], f32, kind="ExternalInput").ap()
    g.out = nc.dram_tensor("out", [CC, P, R], f32, kind="ExternalOutput").ap()

    with tile.TileContext(nc) as tc:
        g.tc = tc
        with tc.tile_pool(name="setup", bufs=1) as setup, \
             tc.tile_pool(name="stats", bufs=4) as stats, \
             tc.tile_pool(name="ps", bufs=8, space="PSUM") as ps, \
             tc.tile_pool(name="wstr", bufs=4) as wstr, \
             tc.tile_pool(name="x1f_p", bufs=1) as x1f_p, \
             tc.tile_pool(name="hp", bufs=1) as hp:
            g.stats, g.ps, g.wstr = stats, ps, wstr

            ident = setup.tile([P, P], f32, tag="ident", name="ident")
            make_identity(nc, ident[:])
            g.ident = ident
            g.ones64 = setup.tile([1, DH], f32, tag="ones64", name="ones64")
            nc.vector.memset(g.ones64[:], 1.0)
            g.eps_sb = setup.tile([P, 1], f32, tag="eps", name="eps")
            nc.vector.memset(g.eps_sb[:], EPS)
            g.sb_vec = {}
            for nm, ap_ in vecs.items():
                n = ap_.shape[1]
                t = setup.tile([P, n], f32, tag=f"vec_{nm}", name=f"vec_{nm}")
                nc.sync.dma_start(t[:], ap_)
                g.sb_vec[nm] = t
            g.g1s, g.be1s = g.sb_vec["g1"], g.sb_vec["be1"]
            g.g2s, g.be2s = g.sb_vec["g2"], g.sb_vec["be2"]

            g.x1f = [x1f_p.tile([P, 512], f32, tag=f"x1f{c}", name=f"x1f{c}")
                     for c in range(CC)]
            g.hcat = [hp.tile([P, 512], bt16, tag=f"hcat{c}", name=f"hcat{c}")
                      for c in range(CC)]

            with tc.tile_pool(name="kvp", bufs=1) as kvp:
                g.kT = [[kvp.tile([P, 512], bt16, tag=f"kT{m}_{rb}",
                                  name=f"kT{m}_{rb}") for rb in range(4)]
                        for m in range(CC)]
                g.vv = [kvp.tile([P, H, DH + 1], bt16, tag=f"vv{kt}",
                                 name=f"vv{kt}") for kt in range(T // P)]
                g.qT = [kvp.tile([P, 512], bt16, tag=f"qT{m}", name=f"qT{m}")
                        for m in range(CC)]
                _phase_a(g)
                _phase_b(g)
            _phase_cd(g)
    _split_sync_waits(nc)
    return nc


_NC_CACHE = None


def _get_nc():
    global _NC_CACHE
    if _NC_CACHE is None:
        _NC_CACHE = build_kernel()
    return _NC_CACHE


def _prep_shared(inputs):
    scale = DH ** -0.5
    Wq = np.asarray(inputs["Wq"], np.float32)
    Wk = np.asarray(inputs["Wk"], np.float32)
    Wv = np.asarray(inputs["Wv"], np.float32)
    Wo = np.asarray(inputs["Wo"], np.float32)
    W1 = np.asarray(inputs["W1"], np.float32)
    W2 = np.asarray(inputs["W2"], np.float32)
    bv_c = np.asarray(inputs["bv"], np.float32).reshape(C)

    def kt_layout(w):
        # [C(in), C(out)] -> [m, p, c, 128]: SBUF tile order for wkm/wqm/w1m
        cin, cout = w.shape
        return np.ascontiguousarray(
            w.reshape(cin // P, P, cout // P, P).transpose(2, 1, 0, 3)
        ).astype(bf16)

    def row_layout(w):
        # [C(in), F] -> [p, c, F]: SBUF tile order for wvs/wos
        cin = w.shape[0]
        return np.ascontiguousarray(
            w.reshape(cin // P, P, -1).transpose(1, 0, 2)).astype(bf16)

    def vec_layout(v):
        # [n*P] -> [P, n] (transposed into partition-major SBUF layout)
        n = v.size // P
        return np.ascontiguousarray(v.reshape(n, P).T).copy()

    shared = {
        "wq": kt_layout(Wq.transpose(1, 0, 2).reshape(C, C) * scale),
        "wk": kt_layout(Wk.transpose(1, 0, 2).reshape(C, C)),
        "wv": row_layout(Wv.transpose(1, 0, 2).reshape(C, C)),
        "wo": row_layout(Wo),
        "w1": kt_layout(W1),
        "w2": kt_layout(W2),
        "bq": vec_layout(np.asarray(inputs["bq"], np.float32).reshape(C)
                         * scale),
        "bk": vec_layout(np.asarray(inputs["bk"], np.float32).reshape(C)),
        "bo": vec_layout(np.asarray(inputs["bo"], np.float32) + bv_c @ Wo),
        "b1": vec_layout(np.asarray(inputs["b1"], np.float32)),
        "b2": vec_layout(np.asarray(inputs["b2"], np.float32)),
        "g1": vec_layout(np.asarray(inputs["gamma1"], np.float32)),
        "be1": vec_layout(np.asarray(inputs["beta1"], np.float32)),
        "g2": vec_layout(np.asarray(inputs["gamma2"], np.float32)),
        "be2": vec_layout(np.asarray(inputs["beta2"], np.float32)),
    }
    return shared


def _core_masks(j):
    """[128, 16, 256] bf16 0/1 masks. kc 0-7 mask the early block's columns
    (block j); kc 8-15 mask the late block's columns (block 7-j)."""
    out = np.zeros((P, 16, QB), np.float32)
    for kc in range(16):
        b = j if kc < 8 else 7 - j
        key = kc * P + np.arange(P)[:, None]          # [128, 1]
        qglob = b * QB + np.arange(QB)[None, :]       # [1, 256]
        out[:, kc, :] = (key <= qglob)
    return out.astype(bf16)


def _make_in_maps(inputs):
    x = np.asarray(inputs["x"], np.float32)
    shared = _prep_shared(inputs)
    in_maps = []
    for c in range(8):
        gg, j = c // 4, c % 4
        xb = x[gg]
        xq = np.concatenate([xb[j * QB:(j + 1) * QB],
                             xb[(7 - j) * QB:(8 - j) * QB]], 0)
        m = dict(shared)
        m["xc"] = np.ascontiguousarray(np.concatenate([xb, xq], 0))
        m["masks"] = _core_masks(j)
        in_maps.append(m)
    return in_maps


def _assemble(results):
    out = np.zeros((B, T, C), np.float32)
    for c in range(8):
        gg, j = c // 4, c % 4
        o = results[c]["out"].reshape(C, R).T  # [512, C] rows = 2 blocks
        out[gg, j * QB:(j + 1) * QB] = o[:QB]
        out[gg, (7 - j) * QB:(8 - j) * QB] = o[QB:]
    return out


def kernel(**inputs):
    in_maps = _make_in_maps(inputs)
    nc = _get_nc()
    res = run_bass_kernel_spmd(nc, in_maps, core_ids=list(range(8)))
    return _assemble(res.results)



# revision 17
# speedup vs baseline: 1.0966x; 1.0005x over previous
"""Trainium2 Bass kernel for a dense transformer block (B=2, T=2048, C=1024,
H=16, DFF=4096), distributed over 8 NeuronCores.

Sharding: 2 batch groups x 4-way query-block sharding. Core c handles batch
g=c//4 and query blocks {j, 7-j} (j=c%4) of 8 blocks of 256 rows. K/V are
computed per-core for the full batch (replicated; no collectives). Causality
is exploited statically: key-chunks 0-7 are needed by both query blocks
(masked only on block-0's columns), chunks 8-15 only by the late block.
The data-dependent causal boundary is applied with per-core 0/1 masks so one
NEFF serves all 8 cores (SPMD).

Activations are kept feature-major ("xT") so every matmul chains without
transposes; layernorm runs row-major with PE transposes between domains.
Softmax denominators ride the AV matmul as an extra ones-column of V.
"""
import numpy as np
import ml_dtypes

import concourse.bass as bass
import concourse.mybir as mybir
import concourse.tile as tile
from concourse.vector_clock import ScopedClock
from concourse.bass_utils import run_bass_kernel_spmd
from concourse.masks import make_identity

bf16 = ml_dtypes.bfloat16
f32 = mybir.dt.float32
bt16 = mybir.dt.bfloat16
AF = mybir.ActivationFunctionType
OP = mybir.AluOpType

B, T, C, H, DH, DFF = 2, 2048, 1024, 16, 64, 4096
P = 128
QB = 256            # rows per query block
R = 512             # own query rows per core
RT = T + R          # ln1 rows per core (full batch + own q rows)
CC = C // P         # 8 feature chunks
MM = DFF // P       # 32 ffn chunks
EPS = 1e-5


# ---------------------------------------------------------------------------
# The walrus build in this container rejects instructions with >1 sync wait.
# Tile's sem assignment can emit several on one instruction; split the excess
# onto same-engine NoOps placed immediately before.
def _patched_drain_and_barrier(self, tick_clock, wait_clock):
    nc = self.nc
    probe = nc.sync.nop(nofuse=True, hint="tail_wait_probe")
    wait_clock.add_sem_waits(probe.ins, ScopedClock({None: tick_clock.global_clock}))
    si = probe.ins.sync_info
    waits = list(si.on_wait) if si is not None else []
    if si is not None:
        si.on_wait = waits[:1]
    for w in waits[1:]:
        n2 = nc.sync.nop(nofuse=True, hint="tail_wait_split")
        n2.ins.sync_info = mybir.SyncInfo(on_wait=[w], on_update=[])
    nc.sync.drain()
    nc.all_engine_barrier()
    assert self.sems is not None
    popped = nc._tile_sem_poison_stack.pop()
    assert popped is self._sem_poison
    nc.clear_and_free_semaphores(list(self.sems.allocated().values()))
    nc.all_engine_barrier()


tile.TileContext._drain_and_barrier = _patched_drain_and_barrier

_MAX_WAITS = 1
_split_counter = [0]


def _split_sync_waits(nc):
    for fn in nc.m.functions:
        for bb in fn.blocks:
            new_insts = []
            for inst in bb.instructions:
                si = getattr(inst, "sync_info", None)
                lim = _MAX_WAITS
                if si is not None and si.on_wait and len(si.on_wait) > lim:
                    waits = list(si.on_wait)
                    keep = waits[-lim:]
                    excess = waits[:-lim]
                    for i in range(0, len(excess), _MAX_WAITS):
                        _split_counter[0] += 1
                        nop = mybir.InstNoOp(
                            name=f"I-wsplit-{_split_counter[0]}", ins=[], outs=[])
                        nop.engine = inst.engine
                        nop.sync_info = mybir.SyncInfo(
                            on_wait=excess[i:i + _MAX_WAITS], on_update=[])
                        new_insts.append(nop)
                    si.on_wait = keep
                new_insts.append(inst)
            bb.instructions = new_insts
# ---------------------------------------------------------------------------


class Ctx:
    pass


def _layernorm_tile(g, xt, out_writes):
    """Row-major LN of xt [128, C] (in place), then transpose chunks and run
    out_writes(c, psum_ap) for each feature chunk c."""
    nc = g.nc
    st = g.stats.tile([P, 2, 6], f32, tag="bnst", name="bnst")
    xv = xt.rearrange("p (s d) -> p s d", s=2)
    for sg in range(2):
        nc.vector.bn_stats(out=st[:, sg, :], in_=xv[:, sg, :])
    mv = g.stats.tile([P, 2], f32, tag="bnmv", name="bnmv")
    nc.vector.bn_aggr(out=mv[:], in_=st[:])
    sq = g.stats.tile([P, 1], f32, tag="bnsq", name="bnsq")
    nc.scalar.activation(out=sq[:], in_=mv[:, 1:2], func=AF.Sqrt,
                         bias=g.eps_sb[:], scale=float(C) / (C - 1))
    rstd = g.stats.tile([P, 1], f32, tag="bnrstd", name="bnrstd")
    nc.vector.reciprocal(rstd[:], sq[:])
    nc.vector.tensor_scalar(out=xt[:], in0=xt[:], scalar1=mv[:, 0:1],
                            scalar2=rstd[:], op0=OP.subtract, op1=OP.mult)
    for c in range(CC):
        pt = g.ps.tile([P, 512], f32, tag="ps", name="ps_t")
        nc.tensor.transpose(pt[:P, :P], xt[:, c * P:(c + 1) * P], g.ident[:])
        out_writes(c, pt[:P, :P])


def _ln1_block(g, x1T, rb):
    """LN1 of row-block rb (4 tiles of 128 rows) into x1T[rb]."""
    nc = g.nc
    for i in range(4):
        rt = rb * 4 + i
        r0 = i * P
        xt = g.xio.tile([P, C], f32, tag="xin", name="xin")
        nc.sync.dma_start(xt[:], g.xc[rt * P:(rt + 1) * P, :])

        def wr1(c, pt, rb=rb, r0=r0, rt=rt):
            nc.scalar.activation(
                out=x1T[rb][:, c, r0:r0 + P], in_=pt, func=AF.Identity,
                bias=g.be1s[:, c:c + 1], scale=g.g1s[:, c:c + 1])
            if rt >= T // P:
                q0 = (rt - T // P) * P
                nc.scalar.activation(
                    out=g.x1f[c][:, q0:q0 + P], in_=pt, func=AF.Identity,
                    bias=g.be1s[:, c:c + 1], scale=g.g1s[:, c:c + 1])
        _layernorm_tile(g, xt, wr1)


def _kproj_block(g, x1T, rb):
    """K^T projection of row-block rb (all 8 output chunks)."""
    nc = g.nc
    for m in range(CC):
        wkm = g.wstr.tile([P, CC, P], bt16, tag="wstr", name="wkm")
        nc.gpsimd.dma_start(wkm[:, :, :], g.wk[m])
        pk = g.ps.tile([P, 512], f32, tag="ps", name="ps_k")
        for c in range(CC):
            nc.tensor.matmul(pk[:], wkm[:, c, :], x1T[rb][:, c, :],
                             start=(c == 0), stop=(c == CC - 1))
        nc.vector.tensor_scalar(
            out=g.kT[m][rb][:, :], in0=pk[:],
            scalar1=g.sb_vec["bk"][:, m:m + 1], scalar2=None, op0=OP.add)


def _vproj_block(g, x1T, wvs, rb):
    """V projection (row-major) for the 4 key-tiles of row-block rb."""
    nc = g.nc
    for i in range(4):
        kt = rb * 4 + i
        r0 = i * P
        for half in range(2):
            pv = g.ps.tile([P, 512], f32, tag="ps", name="ps_v")
            for c in range(CC):
                nc.tensor.matmul(pv[:], x1T[rb][:, c, r0:r0 + P],
                                 wvs[:, c, half * 512:(half + 1) * 512],
                                 start=(c == 0), stop=(c == CC - 1))
            nc.vector.tensor_copy(
                out=g.vv[kt][:, half * 8:(half + 1) * 8, 0:DH],
                in_=pv.rearrange("p (h d) -> p h d", h=8))


def _phase_a(g):
    """LN1 + transpose + Q/K/V projections, software-pipelined per row-block
    so PE (projections) overlaps Act/DVE (layernorm of the next block)."""
    nc, tc = g.nc, g.tc
    with tc.tile_pool(name="x1p", bufs=1) as x1p, \
         tc.tile_pool(name="xio", bufs=3) as xio, \
         tc.tile_pool(name="wvp", bufs=1) as wvp:
        g.xio = xio
        # x1T split per 512-row block (rb 0-3 = batch, rb 4 = own q rows)
        x1T = [x1p.tile([P, CC, 512], bt16, tag=f"x1T{rb}", name=f"x1T{rb}")
               for rb in range(5)]
        for kt in range(T // P):
            nc.vector.memset(g.vv[kt][:, :, DH:DH + 1], 1.0)

        wvs = wvp.tile([P, CC, C], bt16, tag="wvs", name="wvs")
        _ln1_block(g, x1T, 0)
        nc.gpsimd.dma_start(wvs[:], g.wv)
        for rb in range(4):
            if rb + 1 < 5:
                _ln1_block(g, x1T, rb + 1)
            _kproj_block(g, x1T, rb)
            _vproj_block(g, x1T, wvs, rb)

        # Q^T projection of the own-query rows (x1T[4])
        for m in range(CC):
            wqm = g.wstr.tile([P, CC, P], bt16, tag="wstr", name="wqm")
            nc.gpsimd.dma_start(wqm[:, :, :], g.wq[m])
            pq = g.ps.tile([P, 512], f32, tag="ps", name="ps_q")
            for c in range(CC):
                nc.tensor.matmul(pq[:], wqm[:, c, :], x1T[4][:, c, :],
                                 start=(c == 0), stop=(c == CC - 1))
            nc.vector.tensor_scalar(
                out=g.qT[m][:, :], in0=pq[:],
                scalar1=g.sb_vec["bq"][:, m:m + 1], scalar2=None, op0=OP.add)


def _phase_b(g):
    """Attention, both query blocks fused on the free dim (cols 0:256 = early
    block, 256:512 = late block). Key-chunks 0-7 feed both blocks (one N=512
    matmul); chunks 8-15 feed only the late block (N=256)."""
    nc, tc = g.nc, g.tc
    with tc.tile_pool(name="mp", bufs=1) as mp, \
         tc.tile_pool(name="apl", bufs=3) as apl:
        mq = mp.tile([P, 16, QB], bt16, tag="mask", name="mask")
        nc.sync.dma_start(mq[:], g.masks)
        for pair in range(CC):
            for hl in range(2):
                h = 2 * pair + hl
                hs = slice(hl * DH, (hl + 1) * DH)
                aA = apl.tile([P, 8, 512], bt16, tag="aA", name="aA")
                aB = apl.tile([P, 8, QB], bt16, tag="aB", name="aB")
                for kc in range(16):
                    rb, k0 = kc // 4, (kc % 4) * P
                    psc = g.ps.tile([P, 512], f32, tag="ps", name="ps_s")
                    n = 512 if kc < 8 else QB
                    q0 = 0 if kc < 8 else QB
                    nc.tensor.matmul(
                        psc[:, :n], g.kT[pair][rb][hs, k0:k0 + P],
                        g.qT[pair][hs, q0:512],
                        start=True, stop=True, tile_position=(hl * DH, 0))
                    if kc < 8:
                        nc.scalar.activation(out=aA[:, kc, :], in_=psc[:, :512],
                                             func=AF.Exp)
                        nc.vector.tensor_mul(aA[:, kc, 0:QB], aA[:, kc, 0:QB],
                                             mq[:, kc, :])
                    else:
                        nc.scalar.activation(out=aB[:, kc - 8, :],
                                             in_=psc[:, :QB], func=AF.Exp)
                        nc.vector.tensor_mul(aB[:, kc - 8, :], aB[:, kc - 8, :],
                                             mq[:, kc, :])
                pav = g.ps.tile([P, 512], f32, tag="ps", name="ps_av")
                for kc in range(8):
                    nc.tensor.matmul(pav[:DH + 1, :512], g.vv[kc][:, h, :],
                                     aA[:, kc, :], start=(kc == 0), stop=False)
                for kc in range(8, 16):
                    nc.tensor.matmul(pav[:DH + 1, QB:512], g.vv[kc][:, h, :],
                                     aB[:, kc - 8, :], start=False,
                                     stop=(kc == 15))
                den = g.stats.tile([1, 512], f32, tag="den", name="den")
                nc.vector.tensor_copy(den[:], pav[DH:DH + 1, :512])
                rr = g.stats.tile([1, 512], f32, tag="rr", name="rr")
                nc.vector.reciprocal(rr[:], den[:])
                prb = g.ps.tile([P, 512], f32, tag="ps", name="ps_r")
                nc.tensor.matmul(prb[:DH, :512], g.ones64[:], rr[:],
                                 start=True, stop=True)
                rbc = g.stats.tile([DH, 512], f32, tag="rbc", name="rbc")
                nc.vector.tensor_copy(rbc[:], prb[:DH, :512])
                nc.vector.tensor_mul(out=g.hcat[pair][hs, :],
                                     in0=pav[:DH, :512], in1=rbc[:])


def _phase_cd(g):
    """Wo + residual + LN2 + FFN + output."""
    nc, tc = g.nc, g.tc
    with tc.tile_pool(name="cp", bufs=3) as cp, \
         tc.tile_pool(name="wop", bufs=1) as wop, \
         tc.tile_pool(name="x3p", bufs=1) as x3p:
        wos = wop.tile([P, CC, C], bt16, tag="wos", name="wos")
        nc.gpsimd.dma_start(wos[:], g.wo)
        x2T = [x3p.tile([P, 512], f32, tag=f"x2T{m}", name=f"x2T{m}")
               for m in range(CC)]
        for m in range(CC):
            pa = g.ps.tile([P, 512], f32, tag="ps", name="ps_o")
            for c in range(CC):
                nc.tensor.matmul(pa[:, :512], wos[:, c, m * P:(m + 1) * P],
                                 g.hcat[c][:, :], start=(c == 0),
                                 stop=(c == CC - 1))
            nc.vector.scalar_tensor_tensor(
                out=x2T[m][:, :], in0=pa[:, :512],
                scalar=g.sb_vec["bo"][:, m:m + 1], in1=g.x1f[m][:, :],
                op0=OP.add, op1=OP.add)

        x3Tb = [x3p.tile([P, 512], bt16, tag=f"x3Tb{c}", name=f"x3Tb{c}")
                for c in range(CC)]
        x3Tf = [x3p.tile([P, 512], f32, tag=f"x3Tf{c}", name=f"x3Tf{c}")
                for c in range(CC)]
        for rt in range(R // P):
            x2r = cp.tile([P, C], f32, tag="x2r", name="x2r")
            for c in range(CC):
                pt = g.ps.tile([P, 512], f32, tag="ps", name="ps_t2")
                nc.tensor.transpose(pt[:P, :P], x2T[c][:, rt * P:(rt + 1) * P],
                                    g.ident[:])
                nc.scalar.copy(out=x2r[:, c * P:(c + 1) * P], in_=pt[:P, :P])

            def wr2(c, pt, rt=rt):
                nc.scalar.activation(
                    out=x3Tb[c][:, rt * P:(rt + 1) * P], in_=pt,
                    func=AF.Identity, bias=g.be2s[:, c:c + 1],
                    scale=g.g2s[:, c:c + 1])
                nc.scalar.activation(
                    out=x3Tf[c][:, rt * P:(rt + 1) * P], in_=pt,
                    func=AF.Identity, bias=g.be2s[:, c:c + 1],
                    scale=g.g2s[:, c:c + 1])
            _layernorm_tile(g, x2r, wr2)

        # FFN
        with tc.tile_pool(name="dp", bufs=1) as dp:
            h1 = [dp.tile([P, 512], bt16, tag=f"h1_{m}", name=f"h1_{m}")
                  for m in range(MM)]
            for m in range(MM):
                w1m = g.wstr.tile([P, CC, P], bt16, tag="wstr", name="w1m")
                nc.gpsimd.dma_start(w1m[:], g.w1[m])
                p1 = g.ps.tile([P, 512], f32, tag="ps", name="ps_f1")
                for c in range(CC):
                    nc.tensor.matmul(p1[:], w1m[:, c, :], x3Tb[c][:, :],
                                     start=(c == 0), stop=(c == CC - 1))
                nc.scalar.activation(out=h1[m][:, :], in_=p1[:], func=AF.Gelu,
                                     bias=g.sb_vec["b1"][:, m:m + 1], scale=1.0)
            for oc in range(CC):
                w2a = g.wstr.tile([P, MM // 2, P], bt16, tag="wstr", name="w2a")
                nc.gpsimd.dma_start(w2a[:], g.w2[oc, :, :MM // 2])
                w2b = g.wstr.tile([P, MM // 2, P], bt16, tag="wstr", name="w2b")
                nc.gpsimd.dma_start(w2b[:], g.w2[oc, :, MM // 2:])
                p2 = g.ps.tile([P, 512], f32, tag="ps", name="ps_f2")
                for k in range(MM):
                    wt = w2a if k < MM // 2 else w2b
                    nc.tensor.matmul(p2[:], wt[:, k % (MM // 2), :], h1[k][:, :],
                                     start=(k == 0), stop=(k == MM - 1))
                ot = cp.tile([P, R], f32, tag="otile", name="otile")
                nc.vector.scalar_tensor_tensor(
                    out=ot[:], in0=p2[:], scalar=g.sb_vec["b2"][:, oc:oc + 1],
                    in1=x3Tf[oc][:, :], op0=OP.add, op1=OP.add)
                nc.sync.dma_start(g.out[oc], ot[:])


def build_kernel():
    nc = bass.Bass("TRN2", target_bir_lowering=False, num_devices=8)
    g = Ctx()
    g.nc = nc

    # Weight layouts match the SBUF tile layout exactly so every DMA is
    # contiguous per partition (>=512B elements run at full DMA rate).
    g.xc = nc.dram_tensor("xc", [RT, C], f32, kind="ExternalInput").ap()
    g.wq = nc.dram_tensor("wq", [CC, P, CC, P], bt16, kind="ExternalInput").ap()
    g.wk = nc.dram_tensor("wk", [CC, P, CC, P], bt16, kind="ExternalInput").ap()
    g.wv = nc.dram_tensor("wv", [P, CC, C], bt16, kind="ExternalInput").ap()
    g.wo = nc.dram_tensor("wo", [P, CC, C], bt16, kind="ExternalInput").ap()
    g.w1 = nc.dram_tensor("w1", [MM, P, CC, P], bt16, kind="ExternalInput").ap()
    g.w2 = nc.dram_tensor("w2", [CC, P, MM, P], bt16, kind="ExternalInput").ap()
    g.masks = nc.dram_tensor("masks", [P, 16, QB], bt16,
                             kind="ExternalInput").ap()
    # All per-feature vectors packed into one [P, 96] tensor (single DMA)
    VEC_SPECS = [("bq", CC), ("bk", CC), ("bo", CC), ("b1", MM),
                 ("b2", CC), ("g1", CC), ("be1", CC), ("g2", CC), ("be2", CC)]
    NVEC = sum(n for _, n in VEC_SPECS)
    g.vecs = nc.dram_tensor("vecs", [P, NVEC], f32, kind="ExternalInput").ap()
    g.out = nc.dram_tensor("out", [CC, P, R], f32, kind="ExternalOutput").ap()

    with tile.TileContext(nc) as tc:
        g.tc = tc
        with tc.tile_pool(name="setup", bufs=1) as setup, \
             tc.tile_pool(name="stats", bufs=4) as stats, \
             tc.tile_pool(name="ps", bufs=8, space="PSUM") as ps, \
             tc.tile_pool(name="wstr", bufs=4) as wstr, \
             tc.tile_pool(name="x1f_p", bufs=1) as x1f_p, \
             tc.tile_pool(name="hp", bufs=1) as hp:
            g.stats, g.ps, g.wstr = stats, ps, wstr

            ident = setup.tile([P, P], f32, tag="ident", name="ident")
            make_identity(nc, ident[:])
            g.ident = ident
            g.ones64 = setup.tile([1, DH], f32, tag="ones64", name="ones64")
            nc.vector.memset(g.ones64[:], 1.0)
            g.eps_sb = setup.tile([P, 1], f32, tag="eps", name="eps")
            nc.vector.memset(g.eps_sb[:], EPS)
            vt = setup.tile([P, NVEC], f32, tag="vec_all", name="vec_all")
            nc.sync.dma_start(vt[:], g.vecs)
            g.sb_vec = {}
            off = 0
            for nm, n in VEC_SPECS:
                g.sb_vec[nm] = vt[:, off:off + n]
                off += n
            g.g1s, g.be1s = g.sb_vec["g1"], g.sb_vec["be1"]
            g.g2s, g.be2s = g.sb_vec["g2"], g.sb_vec["be2"]

            g.x1f = [x1f_p.tile([P, 512], f32, tag=f"x1f{c}", name=f"x1f{c}")
                     for c in range(CC)]
            g.hcat = [hp.tile([P, 512], bt16, tag=f"hcat{c}", name=f"hcat{c}")
                      for c in range(CC)]

            with tc.tile_pool(name="kvp", bufs=1) as kvp:
                g.kT = [[kvp.tile([P, 512], bt16, tag=f"kT{m}_{rb}",
                                  name=f"kT{m}_{rb}") for rb in range(4)]
                        for m in range(CC)]
                g.vv = [kvp.tile([P, H, DH + 1], bt16, tag=f"vv{kt}",
                                 name=f"vv{kt}") for kt in range(T // P)]
                g.qT = [kvp.tile([P, 512], bt16, tag=f"qT{m}", name=f"qT{m}")
                        for m in range(CC)]
                _phase_a(g)
                _phase_b(g)
            _phase_cd(g)
    _split_sync_waits(nc)
    return nc


_NC_CACHE = None


def _get_nc():
    global _NC_CACHE
    if _NC_CACHE is None:
        _NC_CACHE = build_kernel()
    return _NC_CACHE


def _prep_shared(inputs):
    scale = DH ** -0.5
    Wq = np.asarray(inputs["Wq"], np.float32)
    Wk = np.asarray(inputs["Wk"], np.float32)
    Wv = np.asarray(inputs["Wv"], np.float32)
    Wo = np.asarray(inputs["Wo"], np.float32)
    W1 = np.asarray(inputs["W1"], np.float32)
    W2 = np.asarray(inputs["W2"], np.float32)
    bv_c = np.asarray(inputs["bv"], np.float32).reshape(C)

    def kt_layout(w):
        # [C(in), C(out)] -> [m, p, c, 128]: SBUF tile order for wkm/wqm/w1m
        cin, cout = w.shape
        return np.ascontiguousarray(
            w.reshape(cin // P, P, cout // P, P).transpose(2, 1, 0, 3)
        ).astype(bf16)

    def row_layout(w):
        # [C(in), F] -> [p, c, F]: SBUF tile order for wvs/wos
        cin = w.shape[0]
        return np.ascontiguousarray(
            w.reshape(cin // P, P, -1).transpose(1, 0, 2)).astype(bf16)

    def vec_layout(v):
        # [n*P] -> [P, n] (transposed into partition-major SBUF layout)
        n = v.size // P
        return np.ascontiguousarray(v.reshape(n, P).T).copy()

    shared = {
        "wq": kt_layout(Wq.transpose(1, 0, 2).reshape(C, C) * scale),
        "wk": kt_layout(Wk.transpose(1, 0, 2).reshape(C, C)),
        "wv": row_layout(Wv.transpose(1, 0, 2).reshape(C, C)),
        "wo": row_layout(Wo),
        "w1": kt_layout(W1),
        "w2": kt_layout(W2),
        "vecs": np.concatenate([
            vec_layout(np.asarray(inputs["bq"], np.float32).reshape(C)
                       * scale),
            vec_layout(np.asarray(inputs["bk"], np.float32).reshape(C)),
            vec_layout(np.asarray(inputs["bo"], np.float32) + bv_c @ Wo),
            vec_layout(np.asarray(inputs["b1"], np.float32)),
            vec_layout(np.asarray(inputs["b2"], np.float32)),
            vec_layout(np.asarray(inputs["gamma1"], np.float32)),
            vec_layout(np.asarray(inputs["beta1"], np.float32)),
            vec_layout(np.asarray(inputs["gamma2"], np.float32)),
            vec_layout(np.asarray(inputs["beta2"], np.float32)),
        ], axis=1),
    }
    return shared


def _core_masks(j):
    """[128, 16, 256] bf16 0/1 masks. kc 0-7 mask the early block's columns
    (block j); kc 8-15 mask the late block's columns (block 7-j)."""
    out = np.zeros((P, 16, QB), np.float32)
    for kc in range(16):
        b = j if kc < 8 else 7 - j
        key = kc * P + np.arange(P)[:, None]          # [128, 1]
        qglob = b * QB + np.arange(QB)[None, :]       # [1, 256]
        out[:, kc, :] = (key <= qglob)
    return out.astype(bf16)


def _make_in_maps(inputs):
    x = np.asarray(inputs["x"], np.float32)
    shared = _prep_shared(inputs)
    in_maps = []
    for c in range(8):
        gg, j = c // 4, c % 4
        xb = x[gg]
        xq = np.concatenate([xb[j * QB:(j + 1) * QB],
                             xb[(7 - j) * QB:(8 - j) * QB]], 0)
        m = dict(shared)
        m["xc"] = np.ascontiguousarray(np.concatenate([xb, xq], 0))
        m["masks"] = _core_masks(j)
        in_maps.append(m)
    return in_maps


def _assemble(results):
    out = np.zeros((B, T, C), np.float32)
    for c in range(8):
        gg, j = c // 4, c % 4
        o = results[c]["out"].reshape(C, R).T  # [512, C] rows = 2 blocks
        out[gg, j * QB:(j + 1) * QB] = o[:QB]
        out[gg, (7 - j) * QB:(8 - j) * QB] = o[QB:]
    return out


def kernel(**inputs):
    in_maps = _make_in_maps(inputs)
    nc = _get_nc()
    res = run_bass_kernel_spmd(nc, in_maps, core_ids=list(range(8)))
    return _assemble(res.results)



# revision 21
# speedup vs baseline: 1.1814x; 1.0774x over previous
"""Trainium2 Bass kernel for a dense transformer block (B=2, T=2048, C=1024,
H=16, DFF=4096), distributed over 8 NeuronCores.

Sharding: 2 batch groups x 4-way query-block sharding. Core c handles batch
g=c//4 and query blocks {j, 7-j} (j=c%4) of 8 blocks of 256 rows. K/V are
computed per-core for the full batch (replicated; no collectives). Causality
is exploited statically: key-chunks 0-7 are needed by both query blocks
(masked only on block-0's columns), chunks 8-15 only by the late block.
The data-dependent causal boundary is applied with per-core 0/1 masks so one
NEFF serves all 8 cores (SPMD).

Activations are kept feature-major ("xT") so every matmul chains without
transposes; layernorm runs row-major with PE transposes between domains.
Softmax denominators ride the AV matmul as an extra ones-column of V.
"""
import numpy as np
import ml_dtypes

import concourse.bass as bass
import concourse.mybir as mybir
import concourse.tile as tile
from concourse.vector_clock import ScopedClock
from concourse.bass_utils import run_bass_kernel_spmd
from concourse.masks import make_identity

bf16 = ml_dtypes.bfloat16
f32 = mybir.dt.float32
bt16 = mybir.dt.bfloat16
AF = mybir.ActivationFunctionType
OP = mybir.AluOpType

B, T, C, H, DH, DFF = 2, 2048, 1024, 16, 64, 4096
P = 128
QB = 256            # rows per query block
R = 512             # own query rows per core
RT = T + R          # ln1 rows per core (full batch + own q rows)
CC = C // P         # 8 feature chunks
MM = DFF // P       # 32 ffn chunks
EPS = 1e-5


# ---------------------------------------------------------------------------
# The walrus build in this container rejects instructions with >1 sync wait.
# Tile's sem assignment can emit several on one instruction; split the excess
# onto same-engine NoOps placed immediately before.
def _patched_drain_and_barrier(self, tick_clock, wait_clock):
    nc = self.nc
    probe = nc.sync.nop(nofuse=True, hint="tail_wait_probe")
    wait_clock.add_sem_waits(probe.ins, ScopedClock({None: tick_clock.global_clock}))
    si = probe.ins.sync_info
    waits = list(si.on_wait) if si is not None else []
    if si is not None:
        si.on_wait = waits[:1]
    for w in waits[1:]:
        n2 = nc.sync.nop(nofuse=True, hint="tail_wait_split")
        n2.ins.sync_info = mybir.SyncInfo(on_wait=[w], on_update=[])
    nc.sync.drain()
    nc.all_engine_barrier()
    assert self.sems is not None
    popped = nc._tile_sem_poison_stack.pop()
    assert popped is self._sem_poison
    nc.clear_and_free_semaphores(list(self.sems.allocated().values()))
    nc.all_engine_barrier()


tile.TileContext._drain_and_barrier = _patched_drain_and_barrier

_MAX_WAITS = 1
_split_counter = [0]


def _split_sync_waits(nc):
    for fn in nc.m.functions:
        for bb in fn.blocks:
            new_insts = []
            for inst in bb.instructions:
                si = getattr(inst, "sync_info", None)
                lim = _MAX_WAITS
                if si is not None and si.on_wait and len(si.on_wait) > lim:
                    waits = list(si.on_wait)
                    keep = waits[-lim:]
                    excess = waits[:-lim]
                    for i in range(0, len(excess), _MAX_WAITS):
                        _split_counter[0] += 1
                        nop = mybir.InstNoOp(
                            name=f"I-wsplit-{_split_counter[0]}", ins=[], outs=[])
                        nop.engine = inst.engine
                        nop.sync_info = mybir.SyncInfo(
                            on_wait=excess[i:i + _MAX_WAITS], on_update=[])
                        new_insts.append(nop)
                    si.on_wait = keep
                new_insts.append(inst)
            bb.instructions = new_insts
# ---------------------------------------------------------------------------


class Ctx:
    pass


def _layernorm_pre(g, xt):
    """Row-major LN of xt [128, C] in place (DVE + one tiny Act sqrt)."""
    nc = g.nc
    st = g.stats.tile([P, 2, 6], f32, tag="bnst", name="bnst")
    xv = xt.rearrange("p (s d) -> p s d", s=2)
    for sg in range(2):
        nc.vector.bn_stats(out=st[:, sg, :], in_=xv[:, sg, :])
    mv = g.stats.tile([P, 2], f32, tag="bnmv", name="bnmv")
    nc.vector.bn_aggr(out=mv[:], in_=st[:])
    sq = g.stats.tile([P, 1], f32, tag="bnsq", name="bnsq")
    nc.scalar.activation(out=sq[:], in_=mv[:, 1:2], func=AF.Sqrt,
                         bias=g.eps_sb[:], scale=float(C) / (C - 1))
    rstd = g.stats.tile([P, 1], f32, tag="bnrstd", name="bnrstd")
    nc.vector.reciprocal(rstd[:], sq[:])
    nc.vector.tensor_scalar(out=xt[:], in0=xt[:], scalar1=mv[:, 0:1],
                            scalar2=rstd[:], op0=OP.subtract, op1=OP.mult)


def _layernorm_post(g, xt, out_writes):
    """Transpose normalized xt per feature chunk; out_writes(c, psum_ap)."""
    nc = g.nc
    for c in range(CC):
        pt = g.ps.tile([P, 512], f32, tag="ps", name="ps_t")
        nc.tensor.transpose(pt[:P, :P], xt[:, c * P:(c + 1) * P], g.ident[:])
        out_writes(c, pt[:P, :P])


def _layernorm_tile(g, xt, out_writes):
    _layernorm_pre(g, xt)
    _layernorm_post(g, xt, out_writes)


def _ln1_pre(g, rt):
    """DMA + row-major LN of tile rt; returns the normalized xt tile."""
    nc = g.nc
    xt = g.xio.tile([P, C], f32, tag="xin", name="xin")
    nc.sync.dma_start(xt[:], g.xc[rt * P:(rt + 1) * P, :])
    _layernorm_pre(g, xt)
    return xt


def _ln1_post(g, x1T, xt, rt):
    nc = g.nc
    rb, r0 = rt // 4, (rt % 4) * P

    def wr1(c, pt):
        nc.scalar.activation(
            out=x1T[rb][:, c, r0:r0 + P], in_=pt, func=AF.Identity,
            bias=g.be1s[:, c:c + 1], scale=g.g1s[:, c:c + 1])
        if rt >= T // P:
            q0 = (rt - T // P) * P
            nc.scalar.activation(
                out=g.x1f[c][:, q0:q0 + P], in_=pt, func=AF.Identity,
                bias=g.be1s[:, c:c + 1], scale=g.g1s[:, c:c + 1])
    _layernorm_post(g, xt, wr1)


def _kproj_chunk(g, x1T, rb, m):
    """K^T projection of row-block rb, output chunk m."""
    nc = g.nc
    wkm = g.wstr.tile([P, CC, P], bt16, tag="wstr", name="wkm")
    nc.gpsimd.dma_start(wkm[:, :, :], g.wk[m])
    pk = g.ps.tile([P, 512], f32, tag="ps", name="ps_k")
    for c in range(CC):
        nc.tensor.matmul(pk[:], wkm[:, c, :], x1T[rb][:, c, :],
                         start=(c == 0), stop=(c == CC - 1))
    nc.vector.tensor_scalar(
        out=g.kT[m][rb][:, :], in0=pk[:],
        scalar1=g.sb_vec["bk"][:, m:m + 1], scalar2=None, op0=OP.add)


def _vproj_kt(g, x1T, wvs, kt):
    """V projection (row-major, both 512-col halves) for key-tile kt."""
    nc = g.nc
    rb, r0 = kt // 4, (kt % 4) * P
    for half in range(2):
        pv = g.ps.tile([P, 512], f32, tag="ps", name="ps_v")
        for c in range(CC):
            nc.tensor.matmul(pv[:], x1T[rb][:, c, r0:r0 + P],
                             wvs[:, c, half * 512:(half + 1) * 512],
                             start=(c == 0), stop=(c == CC - 1))
        nc.vector.tensor_copy(
            out=g.vv[kt][:, half * 8:(half + 1) * 8, 0:DH],
            in_=pv.rearrange("p (h d) -> p h d", h=8))


def _phase_a(g):
    """LN1 + transpose + Q/K/V projections, software-pipelined at tile
    granularity: LN stats (DVE) run ahead; each LN transpose batch is
    staggered between projection matmul chunks of the previous row-block so
    the in-order PE stream never waits long."""
    nc, tc = g.nc, g.tc
    with tc.tile_pool(name="x1p", bufs=1) as x1p, \
         tc.tile_pool(name="xio", bufs=3) as xio, \
         tc.tile_pool(name="wvp", bufs=1) as wvp:
        g.xio = xio
        # x1T split per 512-row block (rb 0-3 = batch, rb 4 = own q rows)
        x1T = [x1p.tile([P, CC, 512], bt16, tag=f"x1T{rb}", name=f"x1T{rb}")
               for rb in range(5)]
        for kt in range(T // P):
            nc.vector.memset(g.vv[kt][:, :, DH:DH + 1], 1.0)

        wvs = wvp.tile([P, CC, C], bt16, tag="wvs", name="wvs")
        # Prologue: LN of row-block 0 (no projections to overlap with yet)
        xts = [_ln1_pre(g, rt) for rt in range(2)]
        nc.gpsimd.dma_start(wvs[:], g.wv)
        _ln1_post(g, x1T, xts[0], 0)
        xts.append(_ln1_pre(g, 2))
        _ln1_post(g, x1T, xts[1], 1)
        xts.append(_ln1_pre(g, 3))
        _ln1_post(g, x1T, xts[2], 2)
        _ln1_post(g, x1T, xts[3], 3)

        # Steady state: projections of rb overlap LN of rb+1.
        # PE emission order per rb: K(m0) T(t0) K(m1) T(t1) K(m2) T(t2)
        # K(m3) T(t3) K(m4..7) V(kt*4); LN-pre(t_i) is emitted just before
        # K(m_i) so DVE stats run one matmul-chunk ahead of the transpose.
        for rb in range(4):
            for m in range(CC):
                if m < 4:
                    xt = _ln1_pre(g, (rb + 1) * 4 + m)
                _kproj_chunk(g, x1T, rb, m)
                if m < 4:
                    _ln1_post(g, x1T, xt, (rb + 1) * 4 + m)
            for i in range(4):
                _vproj_kt(g, x1T, wvs, rb * 4 + i)

        # Q^T projection of the own-query rows (x1T[4])
        for m in range(CC):
            wqm = g.wstr.tile([P, CC, P], bt16, tag="wstr", name="wqm")
            nc.gpsimd.dma_start(wqm[:, :, :], g.wq[m])
            pq = g.ps.tile([P, 512], f32, tag="ps", name="ps_q")
            for c in range(CC):
                nc.tensor.matmul(pq[:], wqm[:, c, :], x1T[4][:, c, :],
                                 start=(c == 0), stop=(c == CC - 1))
            nc.vector.tensor_scalar(
                out=g.qT[m][:, :], in0=pq[:],
                scalar1=g.sb_vec["bq"][:, m:m + 1], scalar2=None, op0=OP.add)


def _phase_b(g):
    """Attention, both query blocks fused on the free dim (cols 0:256 = early
    block, 256:512 = late block). Key-chunks 0-7 feed both blocks (one N=512
    matmul); chunks 8-15 feed only the late block (N=256)."""
    nc, tc = g.nc, g.tc
    with tc.tile_pool(name="mp", bufs=1) as mp, \
         tc.tile_pool(name="apl", bufs=3) as apl:
        mq = mp.tile([P, 16, QB], bt16, tag="mask", name="mask")
        nc.sync.dma_start(mq[:], g.masks)
        for pair in range(CC):
            for hl in range(2):
                h = 2 * pair + hl
                hs = slice(hl * DH, (hl + 1) * DH)
                aA = apl.tile([P, 8, 512], bt16, tag="aA", name="aA")
                aB = apl.tile([P, 8, QB], bt16, tag="aB", name="aB")
                for kc in range(16):
                    rb, k0 = kc // 4, (kc % 4) * P
                    psc = g.ps.tile([P, 512], f32, tag="ps", name="ps_s")
                    n = 512 if kc < 8 else QB
                    q0 = 0 if kc < 8 else QB
                    nc.tensor.matmul(
                        psc[:, :n], g.kT[pair][rb][hs, k0:k0 + P],
                        g.qT[pair][hs, q0:512],
                        start=True, stop=True, tile_position=(hl * DH, 0))
                    if kc < 8:
                        nc.scalar.activation(out=aA[:, kc, :], in_=psc[:, :512],
                                             func=AF.Exp)
                        nc.vector.tensor_mul(aA[:, kc, 0:QB], aA[:, kc, 0:QB],
                                             mq[:, kc, :])
                    else:
                        nc.scalar.activation(out=aB[:, kc - 8, :],
                                             in_=psc[:, :QB], func=AF.Exp)
                        nc.vector.tensor_mul(aB[:, kc - 8, :], aB[:, kc - 8, :],
                                             mq[:, kc, :])
                pav = g.ps.tile([P, 512], f32, tag="ps", name="ps_av")
                for kc in range(8):
                    nc.tensor.matmul(pav[:DH + 1, :512], g.vv[kc][:, h, :],
                                     aA[:, kc, :], start=(kc == 0), stop=False)
                for kc in range(8, 16):
                    nc.tensor.matmul(pav[:DH + 1, QB:512], g.vv[kc][:, h, :],
                                     aB[:, kc - 8, :], start=False,
                                     stop=(kc == 15))
                den = g.stats.tile([1, 512], f32, tag="den", name="den")
                nc.vector.tensor_copy(den[:], pav[DH:DH + 1, :512])
                rr = g.stats.tile([1, 512], f32, tag="rr", name="rr")
                nc.vector.reciprocal(rr[:], den[:])
                prb = g.ps.tile([P, 512], f32, tag="ps", name="ps_r")
                nc.tensor.matmul(prb[:DH, :512], g.ones64[:], rr[:],
                                 start=True, stop=True)
                rbc = g.stats.tile([DH, 512], f32, tag="rbc", name="rbc")
                nc.vector.tensor_copy(rbc[:], prb[:DH, :512])
                nc.vector.tensor_mul(out=g.hcat[pair][hs, :],
                                     in0=pav[:DH, :512], in1=rbc[:])


def _phase_cd(g):
    """Wo + residual + LN2 + FFN + output."""
    nc, tc = g.nc, g.tc
    with tc.tile_pool(name="cp", bufs=3) as cp, \
         tc.tile_pool(name="wop", bufs=1) as wop, \
         tc.tile_pool(name="x3p", bufs=1) as x3p:
        wos = wop.tile([P, CC, C], bt16, tag="wos", name="wos")
        nc.gpsimd.dma_start(wos[:], g.wo)
        x2T = [x3p.tile([P, 512], f32, tag=f"x2T{m}", name=f"x2T{m}")
               for m in range(CC)]
        for m in range(CC):
            pa = g.ps.tile([P, 512], f32, tag="ps", name="ps_o")
            for c in range(CC):
                nc.tensor.matmul(pa[:, :512], wos[:, c, m * P:(m + 1) * P],
                                 g.hcat[c][:, :], start=(c == 0),
                                 stop=(c == CC - 1))
            nc.vector.scalar_tensor_tensor(
                out=x2T[m][:, :], in0=pa[:, :512],
                scalar=g.sb_vec["bo"][:, m:m + 1], in1=g.x1f[m][:, :],
                op0=OP.add, op1=OP.add)

        x3Tb = [x3p.tile([P, 512], bt16, tag=f"x3Tb{c}", name=f"x3Tb{c}")
                for c in range(CC)]
        x3Tf = [x3p.tile([P, 512], f32, tag=f"x3Tf{c}", name=f"x3Tf{c}")
                for c in range(CC)]
        for rt in range(R // P):
            x2r = cp.tile([P, C], f32, tag="x2r", name="x2r")
            for c in range(CC):
                pt = g.ps.tile([P, 512], f32, tag="ps", name="ps_t2")
                nc.tensor.transpose(pt[:P, :P], x2T[c][:, rt * P:(rt + 1) * P],
                                    g.ident[:])
                nc.scalar.copy(out=x2r[:, c * P:(c + 1) * P], in_=pt[:P, :P])

            def wr2(c, pt, rt=rt):
                nc.scalar.activation(
                    out=x3Tb[c][:, rt * P:(rt + 1) * P], in_=pt,
                    func=AF.Identity, bias=g.be2s[:, c:c + 1],
                    scale=g.g2s[:, c:c + 1])
                nc.scalar.activation(
                    out=x3Tf[c][:, rt * P:(rt + 1) * P], in_=pt,
                    func=AF.Identity, bias=g.be2s[:, c:c + 1],
                    scale=g.g2s[:, c:c + 1])
            _layernorm_tile(g, x2r, wr2)

        # FFN
        with tc.tile_pool(name="dp", bufs=1) as dp:
            h1 = [dp.tile([P, 512], bt16, tag=f"h1_{m}", name=f"h1_{m}")
                  for m in range(MM)]
            for m in range(MM):
                w1m = g.wstr.tile([P, CC, P], bt16, tag="wstr", name="w1m")
                nc.gpsimd.dma_start(w1m[:], g.w1[m])
                p1 = g.ps.tile([P, 512], f32, tag="ps", name="ps_f1")
                for c in range(CC):
                    nc.tensor.matmul(p1[:], w1m[:, c, :], x3Tb[c][:, :],
                                     start=(c == 0), stop=(c == CC - 1))
                nc.scalar.activation(out=h1[m][:, :], in_=p1[:], func=AF.Gelu,
                                     bias=g.sb_vec["b1"][:, m:m + 1], scale=1.0)
            for oc in range(CC):
                w2a = g.wstr.tile([P, MM // 2, P], bt16, tag="wstr", name="w2a")
                nc.gpsimd.dma_start(w2a[:], g.w2[oc, :, :MM // 2])
                w2b = g.wstr.tile([P, MM // 2, P], bt16, tag="wstr", name="w2b")
                nc.gpsimd.dma_start(w2b[:], g.w2[oc, :, MM // 2:])
                p2 = g.ps.tile([P, 512], f32, tag="ps", name="ps_f2")
                for k in range(MM):
                    wt = w2a if k < MM // 2 else w2b
                    nc.tensor.matmul(p2[:], wt[:, k % (MM // 2), :], h1[k][:, :],
                                     start=(k == 0), stop=(k == MM - 1))
                ot = cp.tile([P, R], f32, tag="otile", name="otile")
                nc.vector.scalar_tensor_tensor(
                    out=ot[:], in0=p2[:], scalar=g.sb_vec["b2"][:, oc:oc + 1],
                    in1=x3Tf[oc][:, :], op0=OP.add, op1=OP.add)
                nc.sync.dma_start(g.out[oc], ot[:])


def build_kernel():
    nc = bass.Bass("TRN2", target_bir_lowering=False, num_devices=8)
    g = Ctx()
    g.nc = nc

    # Weight layouts match the SBUF tile layout exactly so every DMA is
    # contiguous per partition (>=512B elements run at full DMA rate).
    g.xc = nc.dram_tensor("xc", [RT, C], f32, kind="ExternalInput").ap()
    g.wq = nc.dram_tensor("wq", [CC, P, CC, P], bt16, kind="ExternalInput").ap()
    g.wk = nc.dram_tensor("wk", [CC, P, CC, P], bt16, kind="ExternalInput").ap()
    g.wv = nc.dram_tensor("wv", [P, CC, C], bt16, kind="ExternalInput").ap()
    g.wo = nc.dram_tensor("wo", [P, CC, C], bt16, kind="ExternalInput").ap()
    g.w1 = nc.dram_tensor("w1", [MM, P, CC, P], bt16, kind="ExternalInput").ap()
    g.w2 = nc.dram_tensor("w2", [CC, P, MM, P], bt16, kind="ExternalInput").ap()
    g.masks = nc.dram_tensor("masks", [P, 16, QB], bt16,
                             kind="ExternalInput").ap()
    # All per-feature vectors packed into one [P, 96] tensor (single DMA)
    VEC_SPECS = [("bq", CC), ("bk", CC), ("bo", CC), ("b1", MM),
                 ("b2", CC), ("g1", CC), ("be1", CC), ("g2", CC), ("be2", CC)]
    NVEC = sum(n for _, n in VEC_SPECS)
    g.vecs = nc.dram_tensor("vecs", [P, NVEC], f32, kind="ExternalInput").ap()
    g.out = nc.dram_tensor("out", [CC, P, R], f32, kind="ExternalOutput").ap()

    with tile.TileContext(nc) as tc:
        g.tc = tc
        with tc.tile_pool(name="setup", bufs=1) as setup, \
             tc.tile_pool(name="stats", bufs=4) as stats, \
             tc.tile_pool(name="ps", bufs=8, space="PSUM") as ps, \
             tc.tile_pool(name="wstr", bufs=4) as wstr, \
             tc.tile_pool(name="x1f_p", bufs=1) as x1f_p, \
             tc.tile_pool(name="hp", bufs=1) as hp:
            g.stats, g.ps, g.wstr = stats, ps, wstr

            ident = setup.tile([P, P], f32, tag="ident", name="ident")
            make_identity(nc, ident[:])
            g.ident = ident
            g.ones64 = setup.tile([1, DH], f32, tag="ones64", name="ones64")
            nc.vector.memset(g.ones64[:], 1.0)
            g.eps_sb = setup.tile([P, 1], f32, tag="eps", name="eps")
            nc.vector.memset(g.eps_sb[:], EPS)
            vt = setup.tile([P, NVEC], f32, tag="vec_all", name="vec_all")
            nc.sync.dma_start(vt[:], g.vecs)
            g.sb_vec = {}
            off = 0
            for nm, n in VEC_SPECS:
                g.sb_vec[nm] = vt[:, off:off + n]
                off += n
            g.g1s, g.be1s = g.sb_vec["g1"], g.sb_vec["be1"]
            g.g2s, g.be2s = g.sb_vec["g2"], g.sb_vec["be2"]

            g.x1f = [x1f_p.tile([P, 512], f32, tag=f"x1f{c}", name=f"x1f{c}")
                     for c in range(CC)]
            g.hcat = [hp.tile([P, 512], bt16, tag=f"hcat{c}", name=f"hcat{c}")
                      for c in range(CC)]

            with tc.tile_pool(name="kvp", bufs=1) as kvp:
                g.kT = [[kvp.tile([P, 512], bt16, tag=f"kT{m}_{rb}",
                                  name=f"kT{m}_{rb}") for rb in range(4)]
                        for m in range(CC)]
                g.vv = [kvp.tile([P, H, DH + 1], bt16, tag=f"vv{kt}",
                                 name=f"vv{kt}") for kt in range(T // P)]
                g.qT = [kvp.tile([P, 512], bt16, tag=f"qT{m}", name=f"qT{m}")
                        for m in range(CC)]
                _phase_a(g)
                _phase_b(g)
            _phase_cd(g)
    _split_sync_waits(nc)
    return nc


_NC_CACHE = None


def _get_nc():
    global _NC_CACHE
    if _NC_CACHE is None:
        _NC_CACHE = build_kernel()
    return _NC_CACHE


def _prep_shared(inputs):
    scale = DH ** -0.5
    Wq = np.asarray(inputs["Wq"], np.float32)
    Wk = np.asarray(inputs["Wk"], np.float32)
    Wv = np.asarray(inputs["Wv"], np.float32)
    Wo = np.asarray(inputs["Wo"], np.float32)
    W1 = np.asarray(inputs["W1"], np.float32)
    W2 = np.asarray(inputs["W2"], np.float32)
    bv_c = np.asarray(inputs["bv"], np.float32).reshape(C)

    def kt_layout(w):
        # [C(in), C(out)] -> [m, p, c, 128]: SBUF tile order for wkm/wqm/w1m
        cin, cout = w.shape
        return np.ascontiguousarray(
            w.reshape(cin // P, P, cout // P, P).transpose(2, 1, 0, 3)
        ).astype(bf16)

    def row_layout(w):
        # [C(in), F] -> [p, c, F]: SBUF tile order for wvs/wos
        cin = w.shape[0]
        return np.ascontiguousarray(
            w.reshape(cin // P, P, -1).transpose(1, 0, 2)).astype(bf16)

    def vec_layout(v):
        # [n*P] -> [P, n] (transposed into partition-major SBUF layout)
        n = v.size // P
        return np.ascontiguousarray(v.reshape(n, P).T).copy()

    shared = {
        "wq": kt_layout(Wq.transpose(1, 0, 2).reshape(C, C) * scale),
        "wk": kt_layout(Wk.transpose(1, 0, 2).reshape(C, C)),
        "wv": row_layout(Wv.transpose(1, 0, 2).reshape(C, C)),
        "wo": row_layout(Wo),
        "w1": kt_layout(W1),
        "w2": kt_layout(W2),
        "vecs": np.concatenate([
            vec_layout(np.asarray(inputs["bq"], np.float32).reshape(C)
                       * scale),
            vec_layout(np.asarray(inputs["bk"], np.float32).reshape(C)),
            vec_layout(np.asarray(inputs["bo"], np.float32) + bv_c @ Wo),
            vec_layout(np.asarray(inputs["b1"], np.float32)),
            vec_layout(np.asarray(inputs["b2"], np.float32)),
            vec_layout(np.asarray(inputs["gamma1"], np.float32)),
            vec_layout(np.asarray(inputs["beta1"], np.float32)),
            vec_layout(np.asarray(inputs["gamma2"], np.float32)),
            vec_layout(np.asarray(inputs["beta2"], np.float32)),
        ], axis=1),
    }
    return shared


def _core_masks(j):
    """[128, 16, 256] bf16 0/1 masks. kc 0-7 mask the early block's columns
    (block j); kc 8-15 mask the late block's columns (block 7-j)."""
    out = np.zeros((P, 16, QB), np.float32)
    for kc in range(16):
        b = j if kc < 8 else 7 - j
        key = kc * P + np.arange(P)[:, None]          # [128, 1]
        qglob = b * QB + np.arange(QB)[None, :]       # [1, 256]
        out[:, kc, :] = (key <= qglob)
    return out.astype(bf16)


def _make_in_maps(inputs):
    x = np.asarray(inputs["x"], np.float32)
    shared = _prep_shared(inputs)
    in_maps = []
    for c in range(8):
        gg, j = c // 4, c % 4
        xb = x[gg]
        xq = np.concatenate([xb[j * QB:(j + 1) * QB],
                             xb[(7 - j) * QB:(8 - j) * QB]], 0)
        m = dict(shared)
        m["xc"] = np.ascontiguousarray(np.concatenate([xb, xq], 0))
        m["masks"] = _core_masks(j)
        in_maps.append(m)
    return in_maps


def _assemble(results):
    out = np.zeros((B, T, C), np.float32)
    for c in range(8):
        gg, j = c // 4, c % 4
        o = results[c]["out"].reshape(C, R).T  # [512, C] rows = 2 blocks
        out[gg, j * QB:(j + 1) * QB] = o[:QB]
        out[gg, (7 - j) * QB:(8 - j) * QB] = o[QB:]
    return out


def kernel(**inputs):
    in_maps = _make_in_maps(inputs)
    nc = _get_nc()
    res = run_bass_kernel_spmd(nc, in_maps, core_ids=list(range(8)))
    return _assemble(res.results)



# revision 35
# speedup vs baseline: 1.3862x; 1.1733x over previous
"""Trainium2 Bass kernel for a dense transformer block (B=2, T=2048, C=1024,
H=16, DFF=4096), distributed over 8 NeuronCores.

Sharding: 2 batch groups x 4-way query-block sharding. Core c handles batch
g=c//4 and query blocks {j, 7-j} (j=c%4) of 8 blocks of 256 rows. K/V are
computed per-core for the full batch (replicated; no collectives). Causality
is exploited statically: key-chunks 0-7 are needed by both query blocks
(masked only on block-0's columns), chunks 8-15 only by the late block.
The data-dependent causal boundary is applied with per-core 0/1 masks so one
NEFF serves all 8 cores (SPMD).

Activations are kept feature-major ("xT") so every matmul chains without
transposes; layernorm runs row-major with PE transposes between domains.
Softmax denominators ride the AV matmul as an extra ones-column of V.
"""
import numpy as np
import ml_dtypes

import concourse.bass as bass
import concourse.mybir as mybir
import concourse.tile as tile
from concourse.vector_clock import ScopedClock
from concourse.bass_utils import run_bass_kernel_spmd
from concourse.masks import make_identity

bf16 = ml_dtypes.bfloat16
f32 = mybir.dt.float32
bt16 = mybir.dt.bfloat16
AF = mybir.ActivationFunctionType
OP = mybir.AluOpType

B, T, C, H, DH, DFF = 2, 2048, 1024, 16, 64, 4096
P = 128
QB = 256            # rows per query block
R = 512             # own query rows per core
RT = T + R          # ln1 rows per core (full batch + own q rows)
CC = C // P         # 8 feature chunks
MM = DFF // P       # 32 ffn chunks
EPS = 1e-5


# ---------------------------------------------------------------------------
# The walrus build in this container rejects instructions with >1 sync wait.
# Tile's sem assignment can emit several on one instruction; split the excess
# onto same-engine NoOps placed immediately before.
def _patched_drain_and_barrier(self, tick_clock, wait_clock):
    nc = self.nc
    probe = nc.sync.nop(nofuse=True, hint="tail_wait_probe")
    wait_clock.add_sem_waits(probe.ins, ScopedClock({None: tick_clock.global_clock}))
    si = probe.ins.sync_info
    waits = list(si.on_wait) if si is not None else []
    if si is not None:
        si.on_wait = waits[:1]
    for w in waits[1:]:
        n2 = nc.sync.nop(nofuse=True, hint="tail_wait_split")
        n2.ins.sync_info = mybir.SyncInfo(on_wait=[w], on_update=[])
    nc.sync.drain()
    nc.all_engine_barrier()
    assert self.sems is not None
    popped = nc._tile_sem_poison_stack.pop()
    assert popped is self._sem_poison
    nc.clear_and_free_semaphores(list(self.sems.allocated().values()))
    nc.all_engine_barrier()


tile.TileContext._drain_and_barrier = _patched_drain_and_barrier

_MAX_WAITS = 1
_split_counter = [0]


def _split_sync_waits(nc):
    for fn in nc.m.functions:
        for bb in fn.blocks:
            new_insts = []
            for inst in bb.instructions:
                si = getattr(inst, "sync_info", None)
                lim = _MAX_WAITS
                if si is not None and si.on_wait and len(si.on_wait) > lim:
                    waits = list(si.on_wait)
                    keep = waits[-lim:]
                    excess = waits[:-lim]
                    for i in range(0, len(excess), _MAX_WAITS):
                        _split_counter[0] += 1
                        nop = mybir.InstNoOp(
                            name=f"I-wsplit-{_split_counter[0]}", ins=[], outs=[])
                        nop.engine = inst.engine
                        nop.sync_info = mybir.SyncInfo(
                            on_wait=excess[i:i + _MAX_WAITS], on_update=[])
                        new_insts.append(nop)
                    si.on_wait = keep
                new_insts.append(inst)
            bb.instructions = new_insts
# ---------------------------------------------------------------------------


class Ctx:
    pass


def _layernorm_pre(g, xt):
    """Row-major LN of xt [128, C] in place (DVE + one tiny Act sqrt)."""
    nc = g.nc
    st = g.stats.tile([P, 2, 6], f32, tag="bnst", name="bnst")
    xv = xt.rearrange("p (s d) -> p s d", s=2)
    for sg in range(2):
        nc.vector.bn_stats(out=st[:, sg, :], in_=xv[:, sg, :])
    mv = g.stats.tile([P, 2], f32, tag="bnmv", name="bnmv")
    nc.vector.bn_aggr(out=mv[:], in_=st[:])
    sq = g.stats.tile([P, 1], f32, tag="bnsq", name="bnsq")
    nc.scalar.activation(out=sq[:], in_=mv[:, 1:2], func=AF.Sqrt,
                         bias=g.eps_sb[:], scale=float(C) / (C - 1))
    rstd = g.stats.tile([P, 1], f32, tag="bnrstd", name="bnrstd")
    nc.vector.reciprocal(rstd[:], sq[:])
    nc.vector.tensor_scalar(out=xt[:], in0=xt[:], scalar1=mv[:, 0:1],
                            scalar2=rstd[:], op0=OP.subtract, op1=OP.mult)


def _layernorm_post(g, xt, out_writes):
    """Transpose normalized xt per feature chunk; out_writes(c, psum_ap)."""
    nc = g.nc
    for c in range(CC):
        pt = g.ps.tile([P, 512], f32, tag="ps", name="ps_t")
        nc.tensor.transpose(pt[:P, :P], xt[:, c * P:(c + 1) * P], g.ident[:])
        out_writes(c, pt[:P, :P])


def _layernorm_tile(g, xt, out_writes):
    _layernorm_pre(g, xt)
    _layernorm_post(g, xt, out_writes)


def _ln1_pre(g, rt):
    """DMA + row-major LN of tile rt; returns the normalized xt tile."""
    nc = g.nc
    xt = g.xio.tile([P, C], f32, tag="xin", name="xin")
    nc.sync.dma_start(xt[:], g.xc[rt * P:(rt + 1) * P, :])
    _layernorm_pre(g, xt)
    return xt


def _ln1_post(g, x1T, xt, rt):
    nc = g.nc
    rb, r0 = rt // 4, (rt % 4) * P

    def wr1(c, pt):
        nc.scalar.activation(
            out=x1T[rb][:, c, r0:r0 + P], in_=pt, func=AF.Identity,
            bias=g.be1s[:, c:c + 1], scale=g.g1s[:, c:c + 1])
        if rt >= T // P:
            q0 = (rt - T // P) * P
            nc.scalar.activation(
                out=g.x1f[c][:, q0:q0 + P], in_=pt, func=AF.Identity,
                bias=g.be1s[:, c:c + 1], scale=g.g1s[:, c:c + 1])
    _layernorm_post(g, xt, wr1)


def _kproj_chunk(g, x1T, rb, m):
    """K^T projection of row-block rb, output chunk m."""
    nc = g.nc
    wkm = g.wstr.tile([P, CC, P], bt16, tag="wstr", name="wkm")
    nc.gpsimd.dma_start(wkm[:, :, :], g.wk[m])
    pk = g.ps.tile([P, 512], f32, tag="ps", name="ps_k")
    for c in range(CC):
        nc.tensor.matmul(pk[:], wkm[:, c, :], x1T[rb][:, c, :],
                         start=(c == 0), stop=(c == CC - 1))
    nc.vector.tensor_scalar(
        out=g.kT[m][rb][:, :], in0=pk[:],
        scalar1=g.sb_vec["bk"][:, m:m + 1], scalar2=None, op0=OP.add)


def _vproj_kt(g, x1T, wvs, kt):
    """V projection (row-major, both 512-col halves) for key-tile kt.
    Writes vv[kt] scaled by the per-core late-relevance bit svl[kt], and (for
    kt < 8) vvE[kt] scaled by the early-relevance bit sve[kt]."""
    nc = g.nc
    rb, r0 = kt // 4, (kt % 4) * P
    for half in range(2):
        pv = g.ps.tile([P, 512], f32, tag="ps", name="ps_v")
        for c in range(CC):
            nc.tensor.matmul(pv[:], x1T[rb][:, c, r0:r0 + P],
                             wvs[:, c, half * 512:(half + 1) * 512],
                             start=(c == 0), stop=(c == CC - 1))
        pvh = pv.rearrange("p (h d) -> p h d", h=8)
        nc.vector.tensor_scalar(
            out=g.vv[kt][:, half * 8:(half + 1) * 8, 0:DH], in0=pvh,
            scalar1=g.sb_vec["svl"][:, kt:kt + 1], scalar2=None, op0=OP.mult)
        if kt < 8:
            nc.vector.tensor_scalar(
                out=g.vvE[kt][:, half * 8:(half + 1) * 8, 0:DH], in0=pvh,
                scalar1=g.sb_vec["sve"][:, kt:kt + 1], scalar2=None,
                op0=OP.mult)


def _phase_a(g):
    """LN1 + transpose + Q/K/V projections, software-pipelined at tile
    granularity: LN stats (DVE) run ahead; each LN transpose batch is
    staggered between projection matmul chunks of the previous row-block so
    the in-order PE stream never waits long."""
    nc, tc = g.nc, g.tc
    with tc.tile_pool(name="x1p", bufs=1) as x1p, \
         tc.tile_pool(name="xio", bufs=3) as xio, \
         tc.tile_pool(name="wvp", bufs=1) as wvp:
        g.xio = xio
        # x1T split per 512-row block (rb 0-3 = batch, rb 4 = own q rows)
        x1T = [x1p.tile([P, CC, 512], bt16, tag=f"x1T{rb}", name=f"x1T{rb}")
               for rb in range(5)]
        # ones columns (softmax denominator), scaled by per-slot relevance
        for kt in range(T // P):
            nc.vector.memset(g.vv[kt][:, :, DH:DH + 1], 1.0)
            nc.vector.tensor_scalar(
                out=g.vv[kt][:, :, DH:DH + 1], in0=g.vv[kt][:, :, DH:DH + 1],
                scalar1=g.sb_vec["svl"][:, kt:kt + 1], scalar2=None,
                op0=OP.mult)
        for kt in range(8):
            nc.vector.memset(g.vvE[kt][:, :, DH:DH + 1], 1.0)
            nc.vector.tensor_scalar(
                out=g.vvE[kt][:, :, DH:DH + 1],
                in0=g.vvE[kt][:, :, DH:DH + 1],
                scalar1=g.sb_vec["sve"][:, kt:kt + 1], scalar2=None,
                op0=OP.mult)

        wvs = wvp.tile([P, CC, C], bt16, tag="wvs", name="wvs")
        # Prologue: LN of row-block 0 (no projections to overlap with yet)
        xts = [_ln1_pre(g, rt) for rt in range(2)]
        nc.gpsimd.dma_start(wvs[:], g.wv)
        _ln1_post(g, x1T, xts[0], 0)
        xts.append(_ln1_pre(g, 2))
        _ln1_post(g, x1T, xts[1], 1)
        xts.append(_ln1_pre(g, 3))
        _ln1_post(g, x1T, xts[2], 2)
        _ln1_post(g, x1T, xts[3], 3)

        # Steady state: projections of rb overlap LN of rb+1.
        # PE emission order per rb: K(m0) T(t0) K(m1) T(t1) K(m2) T(t2)
        # K(m3) T(t3) K(m4..7) V(kt*4); LN-pre(t_i) is emitted just before
        # K(m_i) so DVE stats run one matmul-chunk ahead of the transpose.
        for rb in range(4):
            for m in range(CC):
                if m < 4:
                    xt = _ln1_pre(g, (rb + 1) * 4 + m)
                _kproj_chunk(g, x1T, rb, m)
                if m < 4:
                    _ln1_post(g, x1T, xt, (rb + 1) * 4 + m)
            for i in range(4):
                _vproj_kt(g, x1T, wvs, rb * 4 + i)

        # Q^T projection of the own-query rows (x1T[4])
        for m in range(CC):
            wqm = g.wstr.tile([P, CC, P], bt16, tag="wstr", name="wqm")
            nc.gpsimd.dma_start(wqm[:, :, :], g.wq[m])
            pq = g.ps.tile([P, 512], f32, tag="ps", name="ps_q")
            for c in range(CC):
                nc.tensor.matmul(pq[:], wqm[:, c, :], x1T[4][:, c, :],
                                 start=(c == 0), stop=(c == CC - 1))
            nc.vector.tensor_scalar(
                out=g.qT[m][:, :], in0=pq[:],
                scalar1=g.sb_vec["bq"][:, m:m + 1], scalar2=None, op0=OP.add)


def _phase_b(g):
    """Attention, both query blocks fused on the free dim (cols 0:256 = early
    block, 256:512 = late block). Key chunks arrive permuted per core so the
    causally-partial (diagonal) chunks sit at slots {0,1} (early) / {8,9}
    (late); all other slots need no elementwise mask because the V copies
    (vvE for the early half, vv for the late half) are zeroed per-slot when
    that chunk is causally irrelevant, nulling both numerator and the
    ones-column denominator. exp runs on 2-PSUM-bank batches; the softmax
    denominator reciprocal is broadcast on the idle GpSimd engine."""
    nc, tc = g.nc, g.tc
    with tc.tile_pool(name="mp", bufs=1) as mp, \
         tc.tile_pool(name="apl", bufs=2) as apl, \
         tc.tile_pool(name="rcp", bufs=2) as rcp, \
         tc.tile_pool(name="scp", bufs=3, space="PSUM") as scp, \
         tc.tile_pool(name="pvp", bufs=2, space="PSUM") as pvp:
        mq = mp.tile([P, 4, QB], bt16, tag="mask", name="mask")
        nc.sync.dma_start(mq[:], g.masks)
        for pair in range(CC):
            for hl in range(2):
                h = 2 * pair + hl
                hs = slice(hl * DH, (hl + 1) * DH)
                aA = apl.tile([P, 8, 512], bt16, tag="aA", name="aA")
                aB = apl.tile([P, 8, QB], bt16, tag="aB", name="aB")
                # scores slots 0..7 (512 wide): 2 slots per 2-bank psum tile
                for t4 in range(4):
                    psc = scp.tile([P, 1024], f32, tag="sc", name="ps_s")
                    for k in range(2):
                        s = 2 * t4 + k
                        rb, k0 = s // 4, (s % 4) * P
                        nc.tensor.matmul(
                            psc[:, k * 512:(k + 1) * 512],
                            g.kT[pair][rb][hs, k0:k0 + P],
                            g.qT[pair][hs, 0:512], start=True, stop=True,
                            tile_position=(hl * DH, 0))
                    nc.scalar.activation(out=aA[:, 2 * t4:2 * t4 + 2, :],
                                         in_=psc[:], func=AF.Exp)
                # slots 8..15 (late block only, 256 wide): 4 per psum tile
                for t4 in range(2):
                    psc = scp.tile([P, 1024], f32, tag="sc", name="ps_sB")
                    for k in range(4):
                        s = 8 + 4 * t4 + k
                        rb, k0 = s // 4, (s % 4) * P
                        nc.tensor.matmul(
                            psc[:, k * QB:(k + 1) * QB],
                            g.kT[pair][rb][hs, k0:k0 + P],
                            g.qT[pair][hs, QB:512], start=True, stop=True,
                            tile_position=(hl * DH, 0))
                    nc.scalar.activation(out=aB[:, 4 * t4:4 * t4 + 4, :],
                                         in_=psc[:], func=AF.Exp)
                # elementwise causal masks only on the diagonal slots
                nc.vector.tensor_mul(aA[:, 0:2, 0:QB], aA[:, 0:2, 0:QB],
                                     mq[:, 0:2, :])
                nc.vector.tensor_mul(aB[:, 0:2, :], aB[:, 0:2, :],
                                     mq[:, 2:4, :])
                pav = pvp.tile([P, 512], f32, tag="pav", name="ps_av")
                for s in range(8):
                    nc.tensor.matmul(pav[:DH + 1, 0:QB], g.vvE[s][:, h, :],
                                     aA[:, s, 0:QB], start=(s == 0),
                                     stop=(s == 7))
                for s in range(8):
                    nc.tensor.matmul(pav[:DH + 1, QB:512], g.vv[s][:, h, :],
                                     aA[:, s, QB:512], start=(s == 0),
                                     stop=False)
                for s in range(8, 16):
                    nc.tensor.matmul(pav[:DH + 1, QB:512], g.vv[s][:, h, :],
                                     aB[:, s - 8, :], start=False,
                                     stop=(s == 15))
                den = g.stats.tile([1, 512], f32, tag="den", name="den")
                nc.vector.tensor_copy(den[:], pav[DH:DH + 1, :512])
                rr = g.stats.tile([1, 512], bt16, tag="rr", name="rr")
                with nc.allow_low_precision(reason="softmax denom in bf16"):
                    nc.vector.reciprocal(rr[:], den[:])
                prb = pvp.tile([P, 512], f32, tag="pav", name="ps_r")
                nc.tensor.matmul(prb[:DH, :512], g.ones64[:], rr[:],
                                 start=True, stop=True)
                rbc = rcp.tile([DH, 512], bt16, tag="rbc", name="rbc")
                with nc.allow_low_precision(reason="softmax denom in bf16"):
                    nc.vector.tensor_copy(rbc[:], prb[:DH, :512])
                nc.vector.tensor_mul(out=g.hcat[pair][hs, :],
                                     in0=pav[:DH, :512], in1=rbc[:])


def _phase_cd(g):
    """Wo + residual + LN2 + FFN + output."""
    nc, tc = g.nc, g.tc
    with tc.tile_pool(name="cp", bufs=3) as cp, \
         tc.tile_pool(name="wop", bufs=1) as wop, \
         tc.tile_pool(name="x3p", bufs=1) as x3p:
        wos = wop.tile([P, CC, C], bt16, tag="wos", name="wos")
        nc.gpsimd.dma_start(wos[:], g.wo)
        x2T = [x3p.tile([P, 512], f32, tag=f"x2T{m}", name=f"x2T{m}")
               for m in range(CC)]
        for m in range(CC):
            pa = g.ps.tile([P, 512], f32, tag="ps", name="ps_o")
            for c in range(CC):
                nc.tensor.matmul(pa[:, :512], wos[:, c, m * P:(m + 1) * P],
                                 g.hcat[c][:, :], start=(c == 0),
                                 stop=(c == CC - 1))
            nc.vector.scalar_tensor_tensor(
                out=x2T[m][:, :], in0=pa[:, :512],
                scalar=g.sb_vec["bo"][:, m:m + 1], in1=g.x1f[m][:, :],
                op0=OP.add, op1=OP.add)

        x3Tb = [x3p.tile([P, 512], bt16, tag=f"x3Tb{c}", name=f"x3Tb{c}")
                for c in range(CC)]
        x3Tf = [x3p.tile([P, 512], f32, tag=f"x3Tf{c}", name=f"x3Tf{c}")
                for c in range(CC)]
        for rt in range(R // P):
            x2r = cp.tile([P, C], f32, tag="x2r", name="x2r")
            for c in range(CC):
                pt = g.ps.tile([P, 512], f32, tag="ps", name="ps_t2")
                nc.tensor.transpose(pt[:P, :P], x2T[c][:, rt * P:(rt + 1) * P],
                                    g.ident[:])
                nc.scalar.copy(out=x2r[:, c * P:(c + 1) * P], in_=pt[:P, :P])

            def wr2(c, pt, rt=rt):
                nc.scalar.activation(
                    out=x3Tb[c][:, rt * P:(rt + 1) * P], in_=pt,
                    func=AF.Identity, bias=g.be2s[:, c:c + 1],
                    scale=g.g2s[:, c:c + 1])
                nc.scalar.activation(
                    out=x3Tf[c][:, rt * P:(rt + 1) * P], in_=pt,
                    func=AF.Identity, bias=g.be2s[:, c:c + 1],
                    scale=g.g2s[:, c:c + 1])
            _layernorm_tile(g, x2r, wr2)

        # FFN
        with tc.tile_pool(name="dp", bufs=1) as dp:
            h1 = [dp.tile([P, 512], bt16, tag=f"h1_{m}", name=f"h1_{m}")
                  for m in range(MM)]
            for m in range(MM):
                w1m = g.wstr.tile([P, CC, P], bt16, tag="wstr", name="w1m")
                nc.gpsimd.dma_start(w1m[:], g.w1[m])
                p1 = g.ps.tile([P, 512], f32, tag="ps", name="ps_f1")
                for c in range(CC):
                    nc.tensor.matmul(p1[:], w1m[:, c, :], x3Tb[c][:, :],
                                     start=(c == 0), stop=(c == CC - 1))
                nc.scalar.activation(out=h1[m][:, :], in_=p1[:], func=AF.Gelu,
                                     bias=g.sb_vec["b1"][:, m:m + 1], scale=1.0)
            for oc in range(CC):
                w2a = g.wstr.tile([P, MM // 2, P], bt16, tag="wstr", name="w2a")
                nc.gpsimd.dma_start(w2a[:], g.w2[oc, :, :MM // 2])
                w2b = g.wstr.tile([P, MM // 2, P], bt16, tag="wstr", name="w2b")
                nc.gpsimd.dma_start(w2b[:], g.w2[oc, :, MM // 2:])
                p2 = g.ps.tile([P, 512], f32, tag="ps", name="ps_f2")
                for k in range(MM):
                    wt = w2a if k < MM // 2 else w2b
                    nc.tensor.matmul(p2[:], wt[:, k % (MM // 2), :], h1[k][:, :],
                                     start=(k == 0), stop=(k == MM - 1))
                ot = cp.tile([P, R], f32, tag="otile", name="otile")
                nc.vector.scalar_tensor_tensor(
                    out=ot[:], in0=p2[:], scalar=g.sb_vec["b2"][:, oc:oc + 1],
                    in1=x3Tf[oc][:, :], op0=OP.add, op1=OP.add)
                nc.sync.dma_start(g.out[oc], ot[:])


def build_kernel():
    nc = bass.Bass("TRN2", target_bir_lowering=False, num_devices=8)
    g = Ctx()
    g.nc = nc

    # Weight layouts match the SBUF tile layout exactly so every DMA is
    # contiguous per partition (>=512B elements run at full DMA rate).
    g.xc = nc.dram_tensor("xc", [RT, C], f32, kind="ExternalInput").ap()
    g.wq = nc.dram_tensor("wq", [CC, P, CC, P], bt16, kind="ExternalInput").ap()
    g.wk = nc.dram_tensor("wk", [CC, P, CC, P], bt16, kind="ExternalInput").ap()
    g.wv = nc.dram_tensor("wv", [P, CC, C], bt16, kind="ExternalInput").ap()
    g.wo = nc.dram_tensor("wo", [P, CC, C], bt16, kind="ExternalInput").ap()
    g.w1 = nc.dram_tensor("w1", [MM, P, CC, P], bt16, kind="ExternalInput").ap()
    g.w2 = nc.dram_tensor("w2", [CC, P, MM, P], bt16, kind="ExternalInput").ap()
    g.masks = nc.dram_tensor("masks", [P, 4, QB], bt16,
                             kind="ExternalInput").ap()
    # All per-feature vectors packed into one tensor (single DMA); svl/sve
    # are per-core per-slot causal-relevance bits for the V-zeroing scheme.
    VEC_SPECS = [("bq", CC), ("bk", CC), ("bo", CC), ("b1", MM),
                 ("b2", CC), ("g1", CC), ("be1", CC), ("g2", CC), ("be2", CC),
                 ("svl", 16), ("sve", 8)]
    NVEC = sum(n for _, n in VEC_SPECS)
    g.vecs = nc.dram_tensor("vecs", [P, NVEC], f32, kind="ExternalInput").ap()
    g.out = nc.dram_tensor("out", [CC, P, R], f32, kind="ExternalOutput").ap()

    with tile.TileContext(nc) as tc:
        g.tc = tc
        with tc.tile_pool(name="setup", bufs=1) as setup, \
             tc.tile_pool(name="stats", bufs=4) as stats, \
             tc.tile_pool(name="wstr", bufs=4) as wstr, \
             tc.tile_pool(name="x1f_p", bufs=1) as x1f_p, \
             tc.tile_pool(name="hp", bufs=1) as hp:
            g.stats, g.wstr = stats, wstr

            ident = setup.tile([P, P], f32, tag="ident", name="ident")
            make_identity(nc, ident[:])
            g.ident = ident
            g.ones64 = setup.tile([1, DH], bt16, tag="ones64", name="ones64")
            nc.vector.memset(g.ones64[:], 1.0)
            g.eps_sb = setup.tile([P, 1], f32, tag="eps", name="eps")
            nc.vector.memset(g.eps_sb[:], EPS)
            vt = setup.tile([P, NVEC], f32, tag="vec_all", name="vec_all")
            nc.sync.dma_start(vt[:], g.vecs)
            g.sb_vec = {}
            off = 0
            for nm, n in VEC_SPECS:
                g.sb_vec[nm] = vt[:, off:off + n]
                off += n
            g.g1s, g.be1s = g.sb_vec["g1"], g.sb_vec["be1"]
            g.g2s, g.be2s = g.sb_vec["g2"], g.sb_vec["be2"]

            g.x1f = [x1f_p.tile([P, 512], bt16, tag=f"x1f{c}", name=f"x1f{c}")
                     for c in range(CC)]
            g.hcat = [hp.tile([P, 512], bt16, tag=f"hcat{c}", name=f"hcat{c}")
                      for c in range(CC)]

            with tc.tile_pool(name="kvp", bufs=1) as kvp:
                g.kT = [[kvp.tile([P, 512], bt16, tag=f"kT{m}_{rb}",
                                  name=f"kT{m}_{rb}") for rb in range(4)]
                        for m in range(CC)]
                g.vv = [kvp.tile([P, H, DH + 1], bt16, tag=f"vv{kt}",
                                 name=f"vv{kt}") for kt in range(T // P)]
                g.vvE = [kvp.tile([P, H, DH + 1], bt16, tag=f"vvE{kt}",
                                  name=f"vvE{kt}") for kt in range(8)]
                g.qT = [kvp.tile([P, 512], bt16, tag=f"qT{m}", name=f"qT{m}")
                        for m in range(CC)]
                with tc.tile_pool(name="psA", bufs=6, space="PSUM") as psA:
                    g.ps = psA
                    _phase_a(g)
                _phase_b(g)
            with tc.tile_pool(name="psC", bufs=8, space="PSUM") as psC:
                g.ps = psC
                _phase_cd(g)
    _split_sync_waits(nc)
    return nc


_NC_CACHE = None


def _get_nc():
    global _NC_CACHE
    if _NC_CACHE is None:
        _NC_CACHE = build_kernel()
    return _NC_CACHE


def _prep_shared(inputs):
    scale = DH ** -0.5
    Wq = np.asarray(inputs["Wq"], np.float32)
    Wk = np.asarray(inputs["Wk"], np.float32)
    Wv = np.asarray(inputs["Wv"], np.float32)
    Wo = np.asarray(inputs["Wo"], np.float32)
    W1 = np.asarray(inputs["W1"], np.float32)
    W2 = np.asarray(inputs["W2"], np.float32)
    bv_c = np.asarray(inputs["bv"], np.float32).reshape(C)

    def kt_layout(w):
        # [C(in), C(out)] -> [m, p, c, 128]: SBUF tile order for wkm/wqm/w1m
        cin, cout = w.shape
        return np.ascontiguousarray(
            w.reshape(cin // P, P, cout // P, P).transpose(2, 1, 0, 3)
        ).astype(bf16)

    def row_layout(w):
        # [C(in), F] -> [p, c, F]: SBUF tile order for wvs/wos
        cin = w.shape[0]
        return np.ascontiguousarray(
            w.reshape(cin // P, P, -1).transpose(1, 0, 2)).astype(bf16)

    def vec_layout(v):
        # [n*P] -> [P, n] (transposed into partition-major SBUF layout)
        n = v.size // P
        return np.ascontiguousarray(v.reshape(n, P).T).copy()

    shared = {
        "wq": kt_layout(Wq.transpose(1, 0, 2).reshape(C, C) * scale),
        "wk": kt_layout(Wk.transpose(1, 0, 2).reshape(C, C)),
        "wv": row_layout(Wv.transpose(1, 0, 2).reshape(C, C)),
        "wo": row_layout(Wo),
        "w1": kt_layout(W1),
        "w2": kt_layout(W2),
        "vecs_base": np.concatenate([
            vec_layout(np.asarray(inputs["bq"], np.float32).reshape(C)
                       * scale),
            vec_layout(np.asarray(inputs["bk"], np.float32).reshape(C)),
            vec_layout(np.asarray(inputs["bo"], np.float32) + bv_c @ Wo),
            vec_layout(np.asarray(inputs["b1"], np.float32)),
            vec_layout(np.asarray(inputs["b2"], np.float32)),
            vec_layout(np.asarray(inputs["gamma1"], np.float32)),
            vec_layout(np.asarray(inputs["beta1"], np.float32)),
            vec_layout(np.asarray(inputs["gamma2"], np.float32)),
            vec_layout(np.asarray(inputs["beta2"], np.float32)),
        ], axis=1),
    }
    return shared


def _core_perm(j):
    """Slot -> key-chunk permutation: diagonal (causally partial) chunks of
    the early block at slots {0,1}, of the late block at slots {8,9}."""
    fixed = {0: 2 * j, 1: 2 * j + 1, 8: 14 - 2 * j, 9: 15 - 2 * j}
    rest = [c for c in range(16) if c not in fixed.values()]
    perm = []
    for s in range(16):
        perm.append(fixed[s] if s in fixed else rest.pop(0))
    return perm


def _core_masks(j, perm):
    """[128, 4, 256] bf16 0/1 masks for the diagonal slots: entries 0,1 mask
    slots 0,1 against the early block's columns; entries 2,3 mask slots 8,9
    against the late block's columns."""
    out = np.zeros((P, 4, QB), np.float32)
    key_p = np.arange(P)[:, None]
    col = np.arange(QB)[None, :]
    for i in range(2):
        out[:, i, :] = (perm[i] * P + key_p <= j * QB + col)
        out[:, 2 + i, :] = (perm[8 + i] * P + key_p <= (7 - j) * QB + col)
    return out.astype(bf16)


def _core_sv(j, perm):
    """Per-slot relevance bits. svl[s]: late half of slot s is causally live
    (diagonal late slots 8,9 use masks instead -> 1). sve[s]: early half of
    slot s is fully live (diagonal early slots 0,1 use masks -> 1)."""
    svl = np.zeros(16, np.float32)
    sve = np.zeros(8, np.float32)
    for s in range(16):
        if s in (8, 9) or perm[s] < 14 - 2 * j:
            svl[s] = 1.0
    for s in range(8):
        if s in (0, 1) or perm[s] < 2 * j:
            sve[s] = 1.0
    return svl, sve


def _make_in_maps(inputs):
    x = np.asarray(inputs["x"], np.float32)
    shared = _prep_shared(inputs)
    vecs_base = shared.pop("vecs_base")
    in_maps = []
    for c in range(8):
        gg, j = c // 4, c % 4
        perm = _core_perm(j)
        xb = x[gg]
        xbp = np.concatenate([xb[p * P:(p + 1) * P] for p in perm], 0)
        xq = np.concatenate([xb[j * QB:(j + 1) * QB],
                             xb[(7 - j) * QB:(8 - j) * QB]], 0)
        svl, sve = _core_sv(j, perm)
        m = dict(shared)
        m["xc"] = np.ascontiguousarray(np.concatenate([xbp, xq], 0))
        m["masks"] = _core_masks(j, perm)
        m["vecs"] = np.ascontiguousarray(np.concatenate([
            vecs_base,
            np.broadcast_to(svl[None, :], (P, 16)),
            np.broadcast_to(sve[None, :], (P, 8)),
        ], axis=1))
        in_maps.append(m)
    return in_maps


def _assemble(results):
    out = np.zeros((B, T, C), np.float32)
    for c in range(8):
        gg, j = c // 4, c % 4
        o = results[c]["out"].reshape(C, R).T  # [512, C] rows = 2 blocks
        out[gg, j * QB:(j + 1) * QB] = o[:QB]
        out[gg, (7 - j) * QB:(8 - j) * QB] = o[QB:]
    return out


def kernel(**inputs):
    in_maps = _make_in_maps(inputs)
    nc = _get_nc()
    res = run_bass_kernel_spmd(nc, in_maps, core_ids=list(range(8)))
    return _assemble(res.results)



# revision 48
# speedup vs baseline: 1.5333x; 1.1061x over previous
"""Trainium2 Bass kernel for a dense transformer block (B=2, T=2048, C=1024,
H=16, DFF=4096), distributed over 8 NeuronCores.

Sharding: 2 batch groups x 4-way query-block sharding. Core c handles batch
g=c//4 and query blocks {j, 7-j} (j=c%4) of 8 blocks of 256 rows. K/V are
computed per-core for the full batch (replicated; no collectives). Causality
is exploited statically: key-chunks 0-7 are needed by both query blocks
(masked only on block-0's columns), chunks 8-15 only by the late block.
The data-dependent causal boundary is applied with per-core 0/1 masks so one
NEFF serves all 8 cores (SPMD).

Activations are kept feature-major ("xT") so every matmul chains without
transposes; layernorm runs row-major with PE transposes between domains.
Softmax denominators ride the AV matmul as an extra ones-column of V.
"""
import numpy as np
import ml_dtypes

import concourse.bass as bass
import concourse.mybir as mybir
import concourse.tile as tile
from concourse.vector_clock import ScopedClock
from concourse.bass_utils import run_bass_kernel_spmd
from concourse.masks import make_identity

bf16 = ml_dtypes.bfloat16
fp8 = ml_dtypes.float8_e4m3
f32 = mybir.dt.float32
bt16 = mybir.dt.bfloat16
f8 = mybir.dt.float8e4
AF = mybir.ActivationFunctionType
OP = mybir.AluOpType

B, T, C, H, DH, DFF = 2, 2048, 1024, 16, 64, 4096
P = 128
QB = 256            # rows per query block
R = 512             # own query rows per core
RT = T + R          # ln1 rows per core (full batch + own q rows)
CC = C // P         # 8 feature chunks
MM = DFF // P       # 32 ffn chunks
EPS = 1e-5


# ---------------------------------------------------------------------------
# The walrus build in this container rejects instructions with >1 sync wait.
# Tile's sem assignment can emit several on one instruction; split the excess
# onto same-engine NoOps placed immediately before.
def _patched_drain_and_barrier(self, tick_clock, wait_clock):
    nc = self.nc
    probe = nc.sync.nop(nofuse=True, hint="tail_wait_probe")
    wait_clock.add_sem_waits(probe.ins, ScopedClock({None: tick_clock.global_clock}))
    si = probe.ins.sync_info
    waits = list(si.on_wait) if si is not None else []
    if si is not None:
        si.on_wait = waits[:1]
    for w in waits[1:]:
        n2 = nc.sync.nop(nofuse=True, hint="tail_wait_split")
        n2.ins.sync_info = mybir.SyncInfo(on_wait=[w], on_update=[])
    nc.sync.drain()
    nc.all_engine_barrier()
    assert self.sems is not None
    popped = nc._tile_sem_poison_stack.pop()
    assert popped is self._sem_poison
    nc.clear_and_free_semaphores(list(self.sems.allocated().values()))
    nc.all_engine_barrier()


tile.TileContext._drain_and_barrier = _patched_drain_and_barrier

_MAX_WAITS = 1
_split_counter = [0]


def _split_sync_waits(nc):
    for fn in nc.m.functions:
        for bb in fn.blocks:
            new_insts = []
            for inst in bb.instructions:
                si = getattr(inst, "sync_info", None)
                lim = _MAX_WAITS
                if si is not None and si.on_wait and len(si.on_wait) > lim:
                    waits = list(si.on_wait)
                    keep = waits[-lim:]
                    excess = waits[:-lim]
                    for i in range(0, len(excess), _MAX_WAITS):
                        _split_counter[0] += 1
                        nop = mybir.InstNoOp(
                            name=f"I-wsplit-{_split_counter[0]}", ins=[], outs=[])
                        nop.engine = inst.engine
                        nop.sync_info = mybir.SyncInfo(
                            on_wait=excess[i:i + _MAX_WAITS], on_update=[])
                        new_insts.append(nop)
                    si.on_wait = keep
                new_insts.append(inst)
            bb.instructions = new_insts
# ---------------------------------------------------------------------------


class Ctx:
    pass


def _layernorm_pre(g, xt):
    """Row-major LN of xt [128, C] in place (DVE + one tiny Act sqrt)."""
    nc = g.nc
    st = g.stats.tile([P, 2, 6], f32, tag="bnst", name="bnst")
    xv = xt.rearrange("p (s d) -> p s d", s=2)
    for sg in range(2):
        nc.vector.bn_stats(out=st[:, sg, :], in_=xv[:, sg, :])
    mv = g.stats.tile([P, 2], f32, tag="bnmv", name="bnmv")
    nc.vector.bn_aggr(out=mv[:], in_=st[:])
    sq = g.stats.tile([P, 1], f32, tag="bnsq", name="bnsq")
    nc.scalar.activation(out=sq[:], in_=mv[:, 1:2], func=AF.Sqrt,
                         bias=g.eps_sb[:], scale=float(C) / (C - 1))
    rstd = g.stats.tile([P, 1], f32, tag="bnrstd", name="bnrstd")
    nc.vector.reciprocal(rstd[:], sq[:])
    nc.vector.tensor_scalar(out=xt[:], in0=xt[:], scalar1=mv[:, 0:1],
                            scalar2=rstd[:], op0=OP.subtract, op1=OP.mult)


def _layernorm_post(g, xt, out_writes):
    """Transpose normalized xt per feature chunk; out_writes(c, psum_ap)."""
    nc = g.nc
    for c in range(CC):
        pt = g.ps.tile([P, 512], f32, tag="ps", name="ps_t")
        nc.tensor.transpose(pt[:P, :P], xt[:, c * P:(c + 1) * P], g.ident[:])
        out_writes(c, pt[:P, :P])


def _layernorm_tile(g, xt, out_writes):
    _layernorm_pre(g, xt)
    _layernorm_post(g, xt, out_writes)


def _ln1_pre(g, rt):
    """DMA + row-major LN of tile rt; returns the normalized xt tile."""
    nc = g.nc
    xt = g.xio.tile([P, C], f32, tag="xin", name="xin")
    nc.sync.dma_start(xt[:], g.xc[rt * P:(rt + 1) * P, :])
    _layernorm_pre(g, xt)
    return xt


def _ln1_post(g, x1T, xt, rt):
    nc = g.nc
    rb, r0 = rt // 4, (rt % 4) * P

    def wr1(c, pt):
        nc.scalar.activation(
            out=x1T[rb][:, c, r0:r0 + P], in_=pt, func=AF.Identity,
            bias=g.be1s[:, c:c + 1], scale=g.g1s[:, c:c + 1])
        if rt >= T // P:
            q0 = (rt - T // P) * P
            nc.scalar.activation(
                out=g.x1f[c][:, q0:q0 + P], in_=pt, func=AF.Identity,
                bias=g.be1s[:, c:c + 1], scale=g.g1s[:, c:c + 1])
    _layernorm_post(g, xt, wr1)


def _kproj_chunk(g, x1T, rb, m):
    """K^T projection of row-block rb, output chunk m."""
    nc = g.nc
    wkm = g.wstr.tile([P, CC, P], bt16, tag="wstr", name="wkm")
    nc.gpsimd.dma_start(wkm[:, :, :], g.wk[m])
    pk = g.ps.tile([P, 512], f32, tag="ps", name="ps_k")
    for c in range(CC):
        nc.tensor.matmul(pk[:], wkm[:, c, :], x1T[rb][:, c, :],
                         start=(c == 0), stop=(c == CC - 1))
    nc.vector.tensor_scalar(
        out=g.kT[m][rb][:, :], in0=pk[:],
        scalar1=g.sb_vec["bk"][:, m:m + 1], scalar2=None, op0=OP.add)


def _vproj_kt(g, x1T, wvs, kt):
    """V projection (row-major, both 512-col halves) for key-tile kt.
    Writes vv[kt] scaled by the per-core late-relevance bit svl[kt], and (for
    kt < 8) vvE[kt] scaled by the early-relevance bit sve[kt]."""
    nc = g.nc
    rb, r0 = kt // 4, (kt % 4) * P
    for half in range(2):
        pv = g.ps.tile([P, 512], f32, tag="ps", name="ps_v")
        for c in range(CC):
            nc.tensor.matmul(pv[:], x1T[rb][:, c, r0:r0 + P],
                             wvs[:, c, half * 512:(half + 1) * 512],
                             start=(c == 0), stop=(c == CC - 1))
        pvh = pv.rearrange("p (h d) -> p h d", h=8)
        nc.vector.tensor_scalar(
            out=g.vv[kt][:, half * 8:(half + 1) * 8, 0:DH], in0=pvh,
            scalar1=g.sb_vec["svl"][:, kt:kt + 1], scalar2=None, op0=OP.mult)
        if kt < 8:
            nc.vector.tensor_scalar(
                out=g.vvE[kt][:, half * 8:(half + 1) * 8, 0:DH], in0=pvh,
                scalar1=g.sb_vec["sve"][:, kt:kt + 1], scalar2=None,
                op0=OP.mult)


def _phase_a(g):
    """LN1 + transpose + Q/K/V projections, software-pipelined at tile
    granularity: LN stats (DVE) run ahead; each LN transpose batch is
    staggered between projection matmul chunks of the previous row-block so
    the in-order PE stream never waits long."""
    nc, tc = g.nc, g.tc
    with tc.tile_pool(name="x1p", bufs=1) as x1p, \
         tc.tile_pool(name="xio", bufs=3) as xio, \
         tc.tile_pool(name="wvp", bufs=1) as wvp:
        g.xio = xio
        # x1T split per 512-row block (rb 0-3 = batch, rb 4 = own q rows)
        x1T = [x1p.tile([P, CC, 512], bt16, tag=f"x1T{rb}", name=f"x1T{rb}")
               for rb in range(5)]
        # ones columns (softmax denominator), scaled by per-slot relevance
        for kt in range(T // P):
            nc.vector.memset(g.vv[kt][:, :, DH:DH + 1], 1.0)
            nc.vector.tensor_scalar(
                out=g.vv[kt][:, :, DH:DH + 1], in0=g.vv[kt][:, :, DH:DH + 1],
                scalar1=g.sb_vec["svl"][:, kt:kt + 1], scalar2=None,
                op0=OP.mult)
        for kt in range(8):
            nc.vector.memset(g.vvE[kt][:, :, DH:DH + 1], 1.0)
            nc.vector.tensor_scalar(
                out=g.vvE[kt][:, :, DH:DH + 1],
                in0=g.vvE[kt][:, :, DH:DH + 1],
                scalar1=g.sb_vec["sve"][:, kt:kt + 1], scalar2=None,
                op0=OP.mult)

        wvs = wvp.tile([P, CC, C], bt16, tag="wvs", name="wvs")
        # Prologue: LN of row-block 0 (no projections to overlap with yet)
        xts = [_ln1_pre(g, rt) for rt in range(2)]
        nc.gpsimd.dma_start(wvs[:], g.wv)
        _ln1_post(g, x1T, xts[0], 0)
        xts.append(_ln1_pre(g, 2))
        _ln1_post(g, x1T, xts[1], 1)
        xts.append(_ln1_pre(g, 3))
        _ln1_post(g, x1T, xts[2], 2)
        _ln1_post(g, x1T, xts[3], 3)

        # Steady state: projections of rb overlap LN of rb+1.
        # PE emission order per rb: K(m0) T(t0) K(m1) T(t1) K(m2) T(t2)
        # K(m3) T(t3) K(m4..7) V(kt*4); LN-pre(t_i) is emitted just before
        # K(m_i) so DVE stats run one matmul-chunk ahead of the transpose.
        for rb in range(4):
            for m in range(CC):
                if m < 4:
                    xt = _ln1_pre(g, (rb + 1) * 4 + m)
                _kproj_chunk(g, x1T, rb, m)
                if m < 4:
                    _ln1_post(g, x1T, xt, (rb + 1) * 4 + m)
            for i in range(4):
                _vproj_kt(g, x1T, wvs, rb * 4 + i)

        # Q^T projection of the own-query rows (x1T[4])
        for m in range(CC):
            wqm = g.wstr.tile([P, CC, P], bt16, tag="wstr", name="wqm")
            nc.gpsimd.dma_start(wqm[:, :, :], g.wq[m])
            pq = g.ps.tile([P, 512], f32, tag="ps", name="ps_q")
            for c in range(CC):
                nc.tensor.matmul(pq[:], wqm[:, c, :], x1T[4][:, c, :],
                                 start=(c == 0), stop=(c == CC - 1))
            nc.vector.tensor_scalar(
                out=g.qT[m][:, :], in0=pq[:],
                scalar1=g.sb_vec["bq"][:, m:m + 1], scalar2=None, op0=OP.add)


def _phase_b(g):
    """Attention, both query blocks fused on the free dim (cols 0:256 = early
    block, 256:512 = late block). Key chunks arrive permuted per core so the
    causally-partial (diagonal) chunks sit at slots {0,1} (early) / {8,9}
    (late); all other slots need no elementwise mask because the V copies
    (vvE for the early half, vv for the late half) are zeroed per-slot when
    that chunk is causally irrelevant, nulling both numerator and the
    ones-column denominator. exp runs on 2-PSUM-bank batches; the softmax
    denominator reciprocal is broadcast on the idle GpSimd engine."""
    nc, tc = g.nc, g.tc
    with tc.tile_pool(name="mp", bufs=1) as mp, \
         tc.tile_pool(name="apl", bufs=2) as apl, \
         tc.tile_pool(name="rcp", bufs=2) as rcp, \
         tc.tile_pool(name="scp", bufs=3, space="PSUM") as scp, \
         tc.tile_pool(name="pvp", bufs=2, space="PSUM") as pvp:
        mq = mp.tile([P, 4, QB], bt16, tag="mask", name="mask")
        nc.sync.dma_start(mq[:], g.masks)
        for pair in range(CC):
            for hl in range(2):
                h = 2 * pair + hl
                hs = slice(hl * DH, (hl + 1) * DH)
                aA = apl.tile([P, 8, 512], bt16, tag="aA", name="aA")
                aB = apl.tile([P, 8, QB], bt16, tag="aB", name="aB")
                # scores slots 0..7 (512 wide): 2 slots per 2-bank psum tile
                for t4 in range(4):
                    psc = scp.tile([P, 1024], f32, tag="sc", name="ps_s")
                    for k in range(2):
                        s = 2 * t4 + k
                        rb, k0 = s // 4, (s % 4) * P
                        nc.tensor.matmul(
                            psc[:, k * 512:(k + 1) * 512],
                            g.kT[pair][rb][hs, k0:k0 + P],
                            g.qT[pair][hs, 0:512], start=True, stop=True,
                            tile_position=(hl * DH, 0))
                    nc.scalar.activation(out=aA[:, 2 * t4:2 * t4 + 2, :],
                                         in_=psc[:], func=AF.Exp)
                # slots 8..15 (late block only, 256 wide): 4 per psum tile
                for t4 in range(2):
                    psc = scp.tile([P, 1024], f32, tag="sc", name="ps_sB")
                    for k in range(4):
                        s = 8 + 4 * t4 + k
                        rb, k0 = s // 4, (s % 4) * P
                        nc.tensor.matmul(
                            psc[:, k * QB:(k + 1) * QB],
                            g.kT[pair][rb][hs, k0:k0 + P],
                            g.qT[pair][hs, QB:512], start=True, stop=True,
                            tile_position=(hl * DH, 0))
                    nc.scalar.activation(out=aB[:, 4 * t4:4 * t4 + 4, :],
                                         in_=psc[:], func=AF.Exp)
                # elementwise causal masks only on the diagonal slots
                nc.vector.tensor_mul(aA[:, 0:2, 0:QB], aA[:, 0:2, 0:QB],
                                     mq[:, 0:2, :])
                nc.vector.tensor_mul(aB[:, 0:2, :], aB[:, 0:2, :],
                                     mq[:, 2:4, :])
                pav = pvp.tile([P, 512], f32, tag="pav", name="ps_av")
                for s in range(8):
                    nc.tensor.matmul(pav[:DH + 1, 0:QB], g.vvE[s][:, h, :],
                                     aA[:, s, 0:QB], start=(s == 0),
                                     stop=(s == 7))
                for s in range(8):
                    nc.tensor.matmul(pav[:DH + 1, QB:512], g.vv[s][:, h, :],
                                     aA[:, s, QB:512], start=(s == 0),
                                     stop=False)
                for s in range(8, 16):
                    nc.tensor.matmul(pav[:DH + 1, QB:512], g.vv[s][:, h, :],
                                     aB[:, s - 8, :], start=False,
                                     stop=(s == 15))
                den = g.stats.tile([1, 512], f32, tag="den", name="den")
                nc.vector.tensor_copy(den[:], pav[DH:DH + 1, :512])
                rr = g.stats.tile([1, 512], bt16, tag="rr", name="rr")
                with nc.allow_low_precision(reason="softmax denom in bf16"):
                    nc.vector.reciprocal(rr[:], den[:])
                prb = pvp.tile([P, 512], f32, tag="pav", name="ps_r")
                nc.tensor.matmul(prb[:DH, :512], g.ones64[:], rr[:],
                                 start=True, stop=True)
                rbc = rcp.tile([DH, 512], bt16, tag="rbc", name="rbc")
                with nc.allow_low_precision(reason="softmax denom in bf16"):
                    nc.vector.tensor_copy(rbc[:], prb[:DH, :512])
                nc.vector.tensor_mul(out=g.hcat[pair][hs, :],
                                     in0=pav[:DH, :512], in1=rbc[:])


def _phase_cd(g):
    """Wo + residual + LN2 + FFN (fp8 DoubleRow) + output.

    FFN weights are pre-scaled by 64 on the host so fp8e4 quantization stays
    in the normal range; the 1/64 rides the gelu's scale operand (FFN1) and
    the epilogue's scalar multiply (FFN2). b2 is pre-folded into the f32
    residual copy of ln2(x) (bias be2f = beta2 + b2)."""
    nc, tc = g.nc, g.tc
    FSC = 1.0 / 64
    with tc.tile_pool(name="cp", bufs=3) as cp, \
         tc.tile_pool(name="wop", bufs=1) as wop, \
         tc.tile_pool(name="x3p", bufs=1) as x3p:
        wos = wop.tile([P, CC, C], bt16, tag="wos", name="wos")
        nc.gpsimd.dma_start(wos[:], g.wo)
        x2T = [x3p.tile([P, 512], f32, tag=f"x2T{m}", name=f"x2T{m}")
               for m in range(CC)]
        for m in range(CC):
            pa = g.ps.tile([P, 512], f32, tag="ps", name="ps_o")
            for c in range(CC):
                nc.tensor.matmul(pa[:, :512], wos[:, c, m * P:(m + 1) * P],
                                 g.hcat[c][:, :], start=(c == 0),
                                 stop=(c == CC - 1))
            nc.vector.scalar_tensor_tensor(
                out=x2T[m][:, :], in0=pa[:, :512],
                scalar=g.sb_vec["bo"][:, m:m + 1], in1=g.x1f[m][:, :],
                op0=OP.add, op1=OP.add)

        # x3T8[kp]: ln2(x) in fp8, DoubleRow-interleaved over feature pairs;
        # x3L8: the x8-scaled fp8 quantization residual (split precision)
        x3T8 = [x3p.tile([P, 2, 512], f8, tag=f"x3T8{c}", name=f"x3T8{c}")
                for c in range(CC // 2)]
        x3L8 = [x3p.tile([P, 2, 512], f8, tag=f"x3L8{c}", name=f"x3L8{c}")
                for c in range(CC // 2)]
        x3Tf = [x3p.tile([P, 512], f32, tag=f"x3Tf{c}", name=f"x3Tf{c}")
                for c in range(CC)]
        for rt in range(R // P):
            x2r = cp.tile([P, C], f32, tag="x2r", name="x2r")
            for c in range(CC):
                pt = g.ps.tile([P, 512], f32, tag="ps", name="ps_t2")
                nc.tensor.transpose(pt[:P, :P], x2T[c][:, rt * P:(rt + 1) * P],
                                    g.ident[:])
                nc.scalar.copy(out=x2r[:, c * P:(c + 1) * P], in_=pt[:P, :P])

            def wr2(c, pt, rt=rt):
                with nc.allow_low_precision(reason="ffn input quantize fp8"):
                    nc.scalar.activation(
                        out=x3T8[c // 2][:, c % 2, rt * P:(rt + 1) * P],
                        in_=pt, func=AF.Identity, bias=g.be2s[:, c:c + 1],
                        scale=g.g2s[:, c:c + 1])
                nc.scalar.activation(
                    out=x3Tf[c][:, rt * P:(rt + 1) * P], in_=pt,
                    func=AF.Identity, bias=g.sb_vec["be2f"][:, c:c + 1],
                    scale=g.g2s[:, c:c + 1])
            _layernorm_tile(g, x2r, wr2)

        # x3 fp8 residual: x3lo = fp8(8 * (x3 - b2 - dequant(x3hi)))
        for c in range(CC):
            dt_ = cp.tile([P, 512], bt16, tag="dres", name="dres")
            with nc.allow_low_precision(reason="fp8 split residual"):
                nc.vector.scalar_tensor_tensor(
                    out=dt_[:], in0=x3Tf[c][:, :],
                    scalar=g.sb_vec["b2"][:, c:c + 1],
                    in1=x3T8[c // 2][:, c % 2, :],
                    op0=OP.subtract, op1=OP.subtract)
                nc.vector.tensor_scalar(
                    out=x3L8[c // 2][:, c % 2, :], in0=dt_[:], scalar1=8.0,
                    scalar2=None, op0=OP.mult)

        # FFN in fp8 DoubleRow (256-row contraction per matmul)
        with tc.tile_pool(name="dp", bufs=1) as dp:
            h1p = [dp.tile([P, 2, 512], f8, tag=f"h1_{m}", name=f"h1_{m}")
                   for m in range(MM // 2)]
            for m in range(MM):
                w1m = g.wstr.tile([P, 3, 4, 2, P], f8, tag="wstr", name="w1m")
                nc.gpsimd.dma_start(w1m[:], g.w1[m])
                p1 = g.ps.tile([P, 512], f32, tag="ps", name="ps_f1")
                for kp in range(4):
                    nc.tensor.matmul(p1[:], w1m[:, 0, kp], x3T8[kp][:],
                                     perf_mode=mybir.MatmulPerfMode.DoubleRow,
                                     start=(kp == 0), stop=False)
                    nc.tensor.matmul(p1[:], w1m[:, 1, kp], x3T8[kp][:],
                                     perf_mode=mybir.MatmulPerfMode.DoubleRow,
                                     start=False, stop=False)
                    nc.tensor.matmul(p1[:], w1m[:, 2, kp], x3L8[kp][:],
                                     perf_mode=mybir.MatmulPerfMode.DoubleRow,
                                     start=False, stop=(kp == 3))
                with nc.allow_low_precision(reason="ffn hidden fp8"):
                    nc.scalar.activation(
                        out=h1p[m // 2][:, m % 2, :], in_=p1[:], func=AF.Gelu,
                        bias=g.sb_vec["b1"][:, m:m + 1], scale=FSC)
            for oc in range(CC):
                w2m = g.wstr.tile([P, 16, 2, P], f8, tag="wstr", name="w2m")
                nc.gpsimd.dma_start(w2m[:], g.w2[oc])
                p2 = g.ps.tile([P, 512], f32, tag="ps", name="ps_f2")
                for kp in range(16):
                    nc.tensor.matmul(p2[:], w2m[:, kp], h1p[kp][:],
                                     perf_mode=mybir.MatmulPerfMode.DoubleRow,
                                     start=(kp == 0), stop=(kp == 15))
                ot = cp.tile([P, R], f32, tag="otile", name="otile")
                nc.vector.scalar_tensor_tensor(
                    out=ot[:], in0=p2[:], scalar=FSC,
                    in1=x3Tf[oc][:, :], op0=OP.mult, op1=OP.add)
                nc.sync.dma_start(g.out[oc], ot[:])


def build_kernel():
    nc = bass.Bass("TRN2", target_bir_lowering=False, num_devices=8)
    g = Ctx()
    g.nc = nc

    # Weight layouts match the SBUF tile layout exactly so every DMA is
    # contiguous per partition (>=512B elements run at full DMA rate).
    g.xc = nc.dram_tensor("xc", [RT, C], f32, kind="ExternalInput").ap()
    g.wq = nc.dram_tensor("wq", [CC, P, CC, P], bt16, kind="ExternalInput").ap()
    g.wk = nc.dram_tensor("wk", [CC, P, CC, P], bt16, kind="ExternalInput").ap()
    g.wv = nc.dram_tensor("wv", [P, CC, C], bt16, kind="ExternalInput").ap()
    g.wo = nc.dram_tensor("wo", [P, CC, C], bt16, kind="ExternalInput").ap()
    # w1 packs three fp8 copies (hi*64, residual*64, hi*8) for the 3-term
    # split-precision FFN1: x3hi@W1hi + x3hi@W1lo + x3lo@W1hi8, all scale 64.
    g.w1 = nc.dram_tensor("w1", [MM, P, 3, 4, 2, P], f8,
                          kind="ExternalInput").ap()
    g.w2 = nc.dram_tensor("w2", [CC, P, 16, 2, P], f8,
                          kind="ExternalInput").ap()
    g.masks = nc.dram_tensor("masks", [P, 4, QB], bt16,
                             kind="ExternalInput").ap()
    # All per-feature vectors packed into one tensor (single DMA); svl/sve
    # are per-core per-slot causal-relevance bits for the V-zeroing scheme.
    VEC_SPECS = [("bq", CC), ("bk", CC), ("bo", CC), ("b1", MM),
                 ("b2", CC), ("g1", CC), ("be1", CC), ("g2", CC), ("be2", CC),
                 ("be2f", CC), ("svl", 16), ("sve", 8)]
    NVEC = sum(n for _, n in VEC_SPECS)
    g.vecs = nc.dram_tensor("vecs", [P, NVEC], f32, kind="ExternalInput").ap()
    g.out = nc.dram_tensor("out", [CC, P, R], f32, kind="ExternalOutput").ap()

    with tile.TileContext(nc) as tc:
        g.tc = tc
        with tc.tile_pool(name="setup", bufs=1) as setup, \
             tc.tile_pool(name="stats", bufs=4) as stats, \
             tc.tile_pool(name="wstr", bufs=4) as wstr, \
             tc.tile_pool(name="x1f_p", bufs=1) as x1f_p, \
             tc.tile_pool(name="hp", bufs=1) as hp:
            g.stats, g.wstr = stats, wstr

            ident = setup.tile([P, P], f32, tag="ident", name="ident")
            make_identity(nc, ident[:])
            g.ident = ident
            g.ones64 = setup.tile([1, DH], bt16, tag="ones64", name="ones64")
            nc.vector.memset(g.ones64[:], 1.0)
            g.eps_sb = setup.tile([P, 1], f32, tag="eps", name="eps")
            nc.vector.memset(g.eps_sb[:], EPS)
            vt = setup.tile([P, NVEC], f32, tag="vec_all", name="vec_all")
            nc.sync.dma_start(vt[:], g.vecs)
            g.sb_vec = {}
            off = 0
            for nm, n in VEC_SPECS:
                g.sb_vec[nm] = vt[:, off:off + n]
                off += n
            g.g1s, g.be1s = g.sb_vec["g1"], g.sb_vec["be1"]
            g.g2s, g.be2s = g.sb_vec["g2"], g.sb_vec["be2"]

            g.x1f = [x1f_p.tile([P, 512], bt16, tag=f"x1f{c}", name=f"x1f{c}")
                     for c in range(CC)]
            g.hcat = [hp.tile([P, 512], bt16, tag=f"hcat{c}", name=f"hcat{c}")
                      for c in range(CC)]

            with tc.tile_pool(name="kvp", bufs=1) as kvp:
                g.kT = [[kvp.tile([P, 512], bt16, tag=f"kT{m}_{rb}",
                                  name=f"kT{m}_{rb}") for rb in range(4)]
                        for m in range(CC)]
                g.vv = [kvp.tile([P, H, DH + 1], bt16, tag=f"vv{kt}",
                                 name=f"vv{kt}") for kt in range(T // P)]
                g.vvE = [kvp.tile([P, H, DH + 1], bt16, tag=f"vvE{kt}",
                                  name=f"vvE{kt}") for kt in range(8)]
                g.qT = [kvp.tile([P, 512], bt16, tag=f"qT{m}", name=f"qT{m}")
                        for m in range(CC)]
                with tc.tile_pool(name="psA", bufs=6, space="PSUM") as psA:
                    g.ps = psA
                    _phase_a(g)
                _phase_b(g)
            with tc.tile_pool(name="psC", bufs=8, space="PSUM") as psC:
                g.ps = psC
                _phase_cd(g)
    _split_sync_waits(nc)
    return nc


_NC_CACHE = None


def _get_nc():
    global _NC_CACHE
    if _NC_CACHE is None:
        _NC_CACHE = build_kernel()
    return _NC_CACHE


def _prep_shared(inputs):
    scale = DH ** -0.5
    Wq = np.asarray(inputs["Wq"], np.float32)
    Wk = np.asarray(inputs["Wk"], np.float32)
    Wv = np.asarray(inputs["Wv"], np.float32)
    Wo = np.asarray(inputs["Wo"], np.float32)
    W1 = np.asarray(inputs["W1"], np.float32)
    W2 = np.asarray(inputs["W2"], np.float32)
    bv_c = np.asarray(inputs["bv"], np.float32).reshape(C)

    def kt_layout(w):
        # [C(in), C(out)] -> [m, p, c, 128]: SBUF tile order for wkm/wqm
        cin, cout = w.shape
        return np.ascontiguousarray(
            w.reshape(cin // P, P, cout // P, P).transpose(2, 1, 0, 3)
        ).astype(bf16)

    def dr_layout(w, s=64.0):
        # [K, F] -> [m, p, kp, i, 128]: fp8 DoubleRow-interleaved, x`s` scaled
        k, fdim = w.shape
        return np.ascontiguousarray(
            (w * s).reshape(k // 256, 2, P, fdim // P, P)
            .transpose(3, 2, 0, 1, 4)).astype(fp8)

    def dr3_layout(w):
        # Three stacked fp8 DoubleRow copies: hi*64, residual*64, hi*8
        hi_mat = (w * 64.0).astype(fp8).astype(np.float32) / 64.0
        return np.ascontiguousarray(np.stack(
            [dr_layout(w, 64.0), dr_layout(w - hi_mat, 64.0),
             dr_layout(w, 8.0)], axis=2))

    def row_layout(w):
        # [C(in), F] -> [p, c, F]: SBUF tile order for wvs/wos
        cin = w.shape[0]
        return np.ascontiguousarray(
            w.reshape(cin // P, P, -1).transpose(1, 0, 2)).astype(bf16)

    def vec_layout(v):
        # [n*P] -> [P, n] (transposed into partition-major SBUF layout)
        n = v.size // P
        return np.ascontiguousarray(v.reshape(n, P).T).copy()

    shared = {
        "wq": kt_layout(Wq.transpose(1, 0, 2).reshape(C, C) * scale),
        "wk": kt_layout(Wk.transpose(1, 0, 2).reshape(C, C)),
        "wv": row_layout(Wv.transpose(1, 0, 2).reshape(C, C)),
        "wo": row_layout(Wo),
        "w1": dr3_layout(W1),
        "w2": dr_layout(W2),
        "vecs_base": np.concatenate([
            vec_layout(np.asarray(inputs["bq"], np.float32).reshape(C)
                       * scale),
            vec_layout(np.asarray(inputs["bk"], np.float32).reshape(C)),
            vec_layout(np.asarray(inputs["bo"], np.float32) + bv_c @ Wo),
            vec_layout(np.asarray(inputs["b1"], np.float32)),
            vec_layout(np.asarray(inputs["b2"], np.float32)),
            vec_layout(np.asarray(inputs["gamma1"], np.float32)),
            vec_layout(np.asarray(inputs["beta1"], np.float32)),
            vec_layout(np.asarray(inputs["gamma2"], np.float32)),
            vec_layout(np.asarray(inputs["beta2"], np.float32)),
            vec_layout(np.asarray(inputs["beta2"], np.float32)
                       + np.asarray(inputs["b2"], np.float32)),
        ], axis=1),
    }
    return shared


def _core_perm(j):
    """Slot -> key-chunk permutation: diagonal (causally partial) chunks of
    the early block at slots {0,1}, of the late block at slots {8,9}."""
    fixed = {0: 2 * j, 1: 2 * j + 1, 8: 14 - 2 * j, 9: 15 - 2 * j}
    rest = [c for c in range(16) if c not in fixed.values()]
    perm = []
    for s in range(16):
        perm.append(fixed[s] if s in fixed else rest.pop(0))
    return perm


def _core_masks(j, perm):
    """[128, 4, 256] bf16 0/1 masks for the diagonal slots: entries 0,1 mask
    slots 0,1 against the early block's columns; entries 2,3 mask slots 8,9
    against the late block's columns."""
    out = np.zeros((P, 4, QB), np.float32)
    key_p = np.arange(P)[:, None]
    col = np.arange(QB)[None, :]
    for i in range(2):
        out[:, i, :] = (perm[i] * P + key_p <= j * QB + col)
        out[:, 2 + i, :] = (perm[8 + i] * P + key_p <= (7 - j) * QB + col)
    return out.astype(bf16)


def _core_sv(j, perm):
    """Per-slot relevance bits. svl[s]: late half of slot s is causally live
    (diagonal late slots 8,9 use masks instead -> 1). sve[s]: early half of
    slot s is fully live (diagonal early slots 0,1 use masks -> 1)."""
    svl = np.zeros(16, np.float32)
    sve = np.zeros(8, np.float32)
    for s in range(16):
        if s in (8, 9) or perm[s] < 14 - 2 * j:
            svl[s] = 1.0
    for s in range(8):
        if s in (0, 1) or perm[s] < 2 * j:
            sve[s] = 1.0
    return svl, sve


def _make_in_maps(inputs):
    x = np.asarray(inputs["x"], np.float32)
    shared = _prep_shared(inputs)
    vecs_base = shared.pop("vecs_base")
    in_maps = []
    for c in range(8):
        gg, j = c // 4, c % 4
        perm = _core_perm(j)
        xb = x[gg]
        xbp = np.concatenate([xb[p * P:(p + 1) * P] for p in perm], 0)
        xq = np.concatenate([xb[j * QB:(j + 1) * QB],
                             xb[(7 - j) * QB:(8 - j) * QB]], 0)
        svl, sve = _core_sv(j, perm)
        m = dict(shared)
        m["xc"] = np.ascontiguousarray(np.concatenate([xbp, xq], 0))
        m["masks"] = _core_masks(j, perm)
        m["vecs"] = np.ascontiguousarray(np.concatenate([
            vecs_base,
            np.broadcast_to(svl[None, :], (P, 16)),
            np.broadcast_to(sve[None, :], (P, 8)),
        ], axis=1))
        in_maps.append(m)
    return in_maps


def _assemble(results):
    out = np.zeros((B, T, C), np.float32)
    for c in range(8):
        gg, j = c // 4, c % 4
        o = results[c]["out"].reshape(C, R).T  # [512, C] rows = 2 blocks
        out[gg, j * QB:(j + 1) * QB] = o[:QB]
        out[gg, (7 - j) * QB:(8 - j) * QB] = o[QB:]
    return out


def kernel(**inputs):
    in_maps = _make_in_maps(inputs)
    nc = _get_nc()
    res = run_bass_kernel_spmd(nc, in_maps, core_ids=list(range(8)))
    return _assemble(res.results)



# revision 71
# speedup vs baseline: 1.6178x; 1.0551x over previous
"""Trainium2 Bass kernel for a dense transformer block (B=2, T=2048, C=1024,
H=16, DFF=4096), distributed over 8 NeuronCores.

Sharding: 2 batch groups x 4-way query-block sharding. Core c handles batch
g=c//4 and query blocks {j, 7-j} (j=c%4) of 8 blocks of 256 rows. K/V are
computed per-core for the full batch (replicated; no collectives). Causality
is exploited statically: key-chunks 0-7 are needed by both query blocks
(masked only on block-0's columns), chunks 8-15 only by the late block.
The data-dependent causal boundary is applied with per-core 0/1 masks so one
NEFF serves all 8 cores (SPMD).

Activations are kept feature-major ("xT") so every matmul chains without
transposes; layernorm runs row-major with PE transposes between domains.
Softmax denominators ride the AV matmul as an extra ones-column of V.
"""
import numpy as np
import ml_dtypes

import concourse.bass as bass
import concourse.mybir as mybir
import concourse.tile as tile
from concourse.vector_clock import ScopedClock
from concourse.bass_utils import run_bass_kernel_spmd
from concourse.masks import make_identity

bf16 = ml_dtypes.bfloat16
fp8 = ml_dtypes.float8_e4m3
f32 = mybir.dt.float32
bt16 = mybir.dt.bfloat16
f8 = mybir.dt.float8e4
AF = mybir.ActivationFunctionType
OP = mybir.AluOpType

B, T, C, H, DH, DFF = 2, 2048, 1024, 16, 64, 4096
P = 128
QB = 256            # rows per query block
R = 512             # own query rows per core
RT = T + R          # ln1 rows per core (full batch + own q rows)
CC = C // P         # 8 feature chunks
MM = DFF // P       # 32 ffn chunks
EPS = 1e-5


# ---------------------------------------------------------------------------
# The walrus build in this container rejects instructions with >1 sync wait.
# Tile's sem assignment can emit several on one instruction; split the excess
# onto same-engine NoOps placed immediately before.
def _patched_drain_and_barrier(self, tick_clock, wait_clock):
    nc = self.nc
    probe = nc.sync.nop(nofuse=True, hint="tail_wait_probe")
    wait_clock.add_sem_waits(probe.ins, ScopedClock({None: tick_clock.global_clock}))
    si = probe.ins.sync_info
    waits = list(si.on_wait) if si is not None else []
    if si is not None:
        si.on_wait = waits[:1]
    for w in waits[1:]:
        n2 = nc.sync.nop(nofuse=True, hint="tail_wait_split")
        n2.ins.sync_info = mybir.SyncInfo(on_wait=[w], on_update=[])
    nc.sync.drain()
    nc.all_engine_barrier()
    assert self.sems is not None
    popped = nc._tile_sem_poison_stack.pop()
    assert popped is self._sem_poison
    nc.clear_and_free_semaphores(list(self.sems.allocated().values()))
    nc.all_engine_barrier()


tile.TileContext._drain_and_barrier = _patched_drain_and_barrier

_MAX_WAITS = 1
_split_counter = [0]


def _split_sync_waits(nc):
    for fn in nc.m.functions:
        for bb in fn.blocks:
            new_insts = []
            for inst in bb.instructions:
                si = getattr(inst, "sync_info", None)
                lim = _MAX_WAITS
                if si is not None and si.on_wait and len(si.on_wait) > lim:
                    waits = list(si.on_wait)
                    keep = waits[-lim:]
                    excess = waits[:-lim]
                    for i in range(0, len(excess), _MAX_WAITS):
                        _split_counter[0] += 1
                        nop = mybir.InstNoOp(
                            name=f"I-wsplit-{_split_counter[0]}", ins=[], outs=[])
                        nop.engine = inst.engine
                        nop.sync_info = mybir.SyncInfo(
                            on_wait=excess[i:i + _MAX_WAITS], on_update=[])
                        new_insts.append(nop)
                    si.on_wait = keep
                new_insts.append(inst)
            bb.instructions = new_insts
# ---------------------------------------------------------------------------


class Ctx:
    pass


def _layernorm_pre(g, xt):
    """Row-major LN of xt [128, C] in place (DVE + one tiny Act sqrt)."""
    nc = g.nc
    st = g.stats.tile([P, 2, 6], f32, tag="bnst", name="bnst")
    xv = xt.rearrange("p (s d) -> p s d", s=2)
    for sg in range(2):
        nc.vector.bn_stats(out=st[:, sg, :], in_=xv[:, sg, :])
    mv = g.stats.tile([P, 2], f32, tag="bnmv", name="bnmv")
    nc.vector.bn_aggr(out=mv[:], in_=st[:])
    sq = g.stats.tile([P, 1], f32, tag="bnsq", name="bnsq")
    nc.scalar.activation(out=sq[:], in_=mv[:, 1:2], func=AF.Sqrt,
                         bias=g.eps_sb[:], scale=float(C) / (C - 1))
    rstd = g.stats.tile([P, 1], f32, tag="bnrstd", name="bnrstd")
    nc.vector.reciprocal(rstd[:], sq[:])
    nc.vector.tensor_scalar(out=xt[:], in0=xt[:], scalar1=mv[:, 0:1],
                            scalar2=rstd[:], op0=OP.subtract, op1=OP.mult)


def _layernorm_post(g, xt, out_writes):
    """Transpose normalized xt per feature chunk; out_writes(c, psum_ap)."""
    nc = g.nc
    for c in range(CC):
        pt = g.ps.tile([P, 512], f32, tag="ps", name="ps_t")
        nc.tensor.transpose(pt[:P, :P], xt[:, c * P:(c + 1) * P], g.ident[:])
        out_writes(c, pt[:P, :P])


def _layernorm_tile(g, xt, out_writes):
    _layernorm_pre(g, xt)
    _layernorm_post(g, xt, out_writes)


def _ln1_pre(g, rt):
    """DMA + row-major LN of tile rt; returns the normalized xt tile."""
    nc = g.nc
    xt = g.xio.tile([P, C], f32, tag="xin", name="xin")
    nc.sync.dma_start(xt[:], g.xc[rt * P:(rt + 1) * P, :])
    _layernorm_pre(g, xt)
    return xt


def _ln1_post(g, x1T, xt, rt):
    nc = g.nc
    rb, r0 = rt // 4, (rt % 4) * P

    def wr1(c, pt):
        nc.scalar.activation(
            out=x1T[rb][:, c, r0:r0 + P], in_=pt, func=AF.Identity,
            bias=g.be1s[:, c:c + 1], scale=g.g1s[:, c:c + 1])
        if rt >= T // P:
            q0 = (rt - T // P) * P
            nc.scalar.activation(
                out=g.x1f[c][:, q0:q0 + P], in_=pt, func=AF.Identity,
                bias=g.be1s[:, c:c + 1], scale=g.g1s[:, c:c + 1])
    _layernorm_post(g, xt, wr1)


def _kproj_chunk(g, x1T, rb, m):
    """K^T projection of row-block rb, output chunk m."""
    nc = g.nc
    wkm = g.wstr.tile([P, CC, P], bt16, tag="wstr", name="wkm")
    nc.gpsimd.dma_start(wkm[:, :, :], g.wk[m])
    pk = g.ps.tile([P, 512], f32, tag="ps", name="ps_k")
    for c in range(CC):
        nc.tensor.matmul(pk[:], wkm[:, c, :], x1T[rb][:, c, :],
                         start=(c == 0), stop=(c == CC - 1))
    nc.vector.tensor_scalar(
        out=g.kT[m][rb][:, :], in0=pk[:],
        scalar1=g.sb_vec["bk"][:, m:m + 1], scalar2=None, op0=OP.add)


def _vproj_kt(g, x1T, wvs, kt):
    """V projection (row-major, both 512-col halves) for key-tile kt.
    Writes vv[kt] scaled by the per-core late-relevance bit svl[kt], and (for
    kt < 8) vvE[kt] scaled by the early-relevance bit sve[kt]."""
    nc = g.nc
    rb, r0 = kt // 4, (kt % 4) * P
    for half in range(2):
        pv = g.ps.tile([P, 512], f32, tag="ps", name="ps_v")
        for c in range(CC):
            nc.tensor.matmul(pv[:], x1T[rb][:, c, r0:r0 + P],
                             wvs[:, c, half * 512:(half + 1) * 512],
                             start=(c == 0), stop=(c == CC - 1))
        pvh = pv.rearrange("p (h d) -> p h d", h=8)
        nc.vector.tensor_scalar(
            out=g.vv[kt][:, half * 8:(half + 1) * 8, 0:DH], in0=pvh,
            scalar1=g.sb_vec["svl"][:, kt:kt + 1], scalar2=None, op0=OP.mult)
        if kt < 8:
            nc.vector.tensor_scalar(
                out=g.vvE[kt][:, half * 8:(half + 1) * 8, 0:DH], in0=pvh,
                scalar1=g.sb_vec["sve"][:, kt:kt + 1], scalar2=None,
                op0=OP.mult)


def _phase_a(g):
    """LN1 + transpose + Q/K/V projections, software-pipelined at tile
    granularity: LN stats (DVE) run ahead; each LN transpose batch is
    staggered between projection matmul chunks of the previous row-block so
    the in-order PE stream never waits long."""
    nc, tc = g.nc, g.tc
    with tc.tile_pool(name="x1p", bufs=1) as x1p, \
         tc.tile_pool(name="xio", bufs=3) as xio, \
         tc.tile_pool(name="wvp", bufs=1) as wvp:
        g.xio = xio
        # x1T split per 512-row block (rb 0-3 = batch, rb 4 = own q rows)
        x1T = [x1p.tile([P, CC, 512], bt16, tag=f"x1T{rb}", name=f"x1T{rb}")
               for rb in range(5)]
        wvs = wvp.tile([P, CC, C], bt16, tag="wvs", name="wvs")
        # Prologue: LN of row-block 0 (no projections to overlap with yet)
        xts = [_ln1_pre(g, rt) for rt in range(3)]
        _ln1_post(g, x1T, xts[0], 0)
        xts.append(_ln1_pre(g, 3))
        nc.gpsimd.dma_start(wvs[:], g.wv)
        _ln1_post(g, x1T, xts[1], 1)
        _ln1_post(g, x1T, xts[2], 2)
        _ln1_post(g, x1T, xts[3], 3)

        # ones columns (softmax denominator), scaled by per-slot relevance
        for kt in range(T // P):
            nc.vector.memset(g.vv[kt][:, :, DH:DH + 1], 1.0)
            nc.vector.tensor_scalar(
                out=g.vv[kt][:, :, DH:DH + 1], in0=g.vv[kt][:, :, DH:DH + 1],
                scalar1=g.sb_vec["svl"][:, kt:kt + 1], scalar2=None,
                op0=OP.mult)
        for kt in range(8):
            nc.vector.memset(g.vvE[kt][:, :, DH:DH + 1], 1.0)
            nc.vector.tensor_scalar(
                out=g.vvE[kt][:, :, DH:DH + 1],
                in0=g.vvE[kt][:, :, DH:DH + 1],
                scalar1=g.sb_vec["sve"][:, kt:kt + 1], scalar2=None,
                op0=OP.mult)

        # Steady state: projections of rb overlap LN of rb+1.
        # PE emission order per rb: K(m0) T(t0) K(m1) T(t1) K(m2) T(t2)
        # K(m3) T(t3) K(m4..7) V(kt*4); LN-pre(t_i) is emitted just before
        # K(m_i) so DVE stats run one matmul-chunk ahead of the transpose.
        for rb in range(4):
            for m in range(CC):
                if m < 4:
                    xt = _ln1_pre(g, (rb + 1) * 4 + m)
                _kproj_chunk(g, x1T, rb, m)
                if m < 4:
                    _ln1_post(g, x1T, xt, (rb + 1) * 4 + m)
            for i in range(4):
                _vproj_kt(g, x1T, wvs, rb * 4 + i)

        # Q^T projection of the own-query rows (x1T[4])
        for m in range(CC):
            wqm = g.wstr.tile([P, CC, P], bt16, tag="wstr", name="wqm")
            nc.gpsimd.dma_start(wqm[:, :, :], g.wq[m])
            pq = g.ps.tile([P, 512], f32, tag="ps", name="ps_q")
            for c in range(CC):
                nc.tensor.matmul(pq[:], wqm[:, c, :], x1T[4][:, c, :],
                                 start=(c == 0), stop=(c == CC - 1))
            nc.vector.tensor_scalar(
                out=g.qT[m][:, :], in0=pq[:],
                scalar1=g.sb_vec["bq"][:, m:m + 1], scalar2=None, op0=OP.add)


def _phase_b(g):
    """Attention, both query blocks fused on the free dim (cols 0:256 = early
    block, 256:512 = late block). Key chunks arrive permuted per core so the
    causally-partial (diagonal) chunks sit at slots {0,1} (early) / {8,9}
    (late); all other slots need no elementwise mask because the V copies
    (vvE for the early half, vv for the late half) are zeroed per-slot when
    that chunk is causally irrelevant, nulling both numerator and the
    ones-column denominator. exp runs on 2-PSUM-bank batches; the softmax
    denominator reciprocal is broadcast on the idle GpSimd engine."""
    nc, tc = g.nc, g.tc
    with tc.tile_pool(name="mp", bufs=1) as mp, \
         tc.tile_pool(name="apl", bufs=2) as apl, \
         tc.tile_pool(name="rcp", bufs=2) as rcp, \
         tc.tile_pool(name="hp", bufs=1) as hp, \
         tc.tile_pool(name="wop", bufs=1) as wop, \
         tc.tile_pool(name="scp", bufs=3, space="PSUM") as scp, \
         tc.tile_pool(name="pvp", bufs=2, space="PSUM") as pvp:
        g.hcat = [hp.tile([P, 512], bt16, tag=f"hcat{c}", name=f"hcat{c}")
                  for c in range(CC)]
        mq = mp.tile([P, 4, QB], bt16, tag="mask", name="mask")
        nc.sync.dma_start(mq[:], g.masks)
        # prefetch the Wo weights during attention (DMA engine is idle here)
        wos = wop.tile([P, CC, C], bt16, tag="wos", name="wos")
        nc.gpsimd.dma_start(wos[:], g.wo)
        for pair in range(CC):
            for hl in range(2):
                h = 2 * pair + hl
                hs = slice(hl * DH, (hl + 1) * DH)
                aA = apl.tile([P, 8, 512], bt16, tag="aA", name="aA")
                aB = apl.tile([P, 8, QB], bt16, tag="aB", name="aB")
                # scores slots 0..7 (512 wide): 2 slots per 2-bank psum tile
                for t4 in range(4):
                    psc = scp.tile([P, 1024], f32, tag="sc", name="ps_s")
                    for k in range(2):
                        s = 2 * t4 + k
                        rb, k0 = s // 4, (s % 4) * P
                        nc.tensor.matmul(
                            psc[:, k * 512:(k + 1) * 512],
                            g.kT[pair][rb][hs, k0:k0 + P],
                            g.qT[pair][hs, 0:512], start=True, stop=True,
                            tile_position=(hl * DH, 0))
                    nc.scalar.activation(out=aA[:, 2 * t4:2 * t4 + 2, :],
                                         in_=psc[:], func=AF.Exp)
                # slots 8..15 (late block only, 256 wide): 4 per psum tile
                for t4 in range(2):
                    psc = scp.tile([P, 1024], f32, tag="sc", name="ps_sB")
                    for k in range(4):
                        s = 8 + 4 * t4 + k
                        rb, k0 = s // 4, (s % 4) * P
                        nc.tensor.matmul(
                            psc[:, k * QB:(k + 1) * QB],
                            g.kT[pair][rb][hs, k0:k0 + P],
                            g.qT[pair][hs, QB:512], start=True, stop=True,
                            tile_position=(hl * DH, 0))
                    nc.scalar.activation(out=aB[:, 4 * t4:4 * t4 + 4, :],
                                         in_=psc[:], func=AF.Exp)
                # elementwise causal masks only on the diagonal slots
                nc.vector.tensor_mul(aA[:, 0:2, 0:QB], aA[:, 0:2, 0:QB],
                                     mq[:, 0:2, :])
                nc.vector.tensor_mul(aB[:, 0:2, :], aB[:, 0:2, :],
                                     mq[:, 2:4, :])
                pav = pvp.tile([P, 512], f32, tag="pav", name="ps_av")
                for s in range(8):
                    nc.tensor.matmul(pav[:DH + 1, 0:QB], g.vvE[s][:, h, :],
                                     aA[:, s, 0:QB], start=(s == 0),
                                     stop=(s == 7))
                for s in range(8):
                    nc.tensor.matmul(pav[:DH + 1, QB:512], g.vv[s][:, h, :],
                                     aA[:, s, QB:512], start=(s == 0),
                                     stop=False)
                for s in range(8, 16):
                    nc.tensor.matmul(pav[:DH + 1, QB:512], g.vv[s][:, h, :],
                                     aB[:, s - 8, :], start=False,
                                     stop=(s == 15))
                den = g.stats.tile([1, 512], f32, tag="den", name="den")
                nc.vector.tensor_copy(den[:], pav[DH:DH + 1, :512])
                rr = g.stats.tile([1, 512], bt16, tag="rr", name="rr")
                with nc.allow_low_precision(reason="softmax denom in bf16"):
                    nc.vector.reciprocal(rr[:], den[:])
                prb = pvp.tile([P, 512], f32, tag="pav", name="ps_r")
                nc.tensor.matmul(prb[:DH, :512], g.ones64[:], rr[:],
                                 start=True, stop=True)
                rbc = rcp.tile([DH, 512], bt16, tag="rbc", name="rbc")
                with nc.allow_low_precision(reason="softmax denom in bf16"):
                    nc.vector.tensor_copy(rbc[:], prb[:DH, :512])
                nc.vector.tensor_mul(out=g.hcat[pair][hs, :],
                                     in0=pav[:DH, :512], in1=rbc[:])

        # Wo projection + residual (uses hcat/x1f/wos before they die)
        for m in range(CC):
            pa = pvp.tile([P, 512], f32, tag="pav", name="ps_o")
            for c in range(CC):
                nc.tensor.matmul(pa[:, :512], wos[:, c, m * P:(m + 1) * P],
                                 g.hcat[c][:, :], start=(c == 0),
                                 stop=(c == CC - 1))
            with nc.allow_low_precision(reason="x2 residual in bf16"):
                nc.vector.scalar_tensor_tensor(
                    out=g.x2T[m][:, :], in0=pa[:, :512],
                    scalar=g.sb_vec["bo"][:, m:m + 1], in1=g.x1f[m][:, :],
                    op0=OP.add, op1=OP.add)


def _phase_d(g):
    """LN2 + FFN (fp8 DoubleRow) + output.

    FFN weights are pre-scaled by 64 on the host so fp8e4 quantization stays
    in the normal range; the 1/64 rides the gelu's scale operand (FFN1) and
    the epilogue's scalar multiply (FFN2). b2 is pre-folded into the f32
    residual copy of ln2(x) (bias be2f = beta2 + b2)."""
    nc, tc = g.nc, g.tc
    FSC = 1.0 / 64
    x2T = g.x2T
    with tc.tile_pool(name="cp", bufs=3) as cp, \
         tc.tile_pool(name="psB", bufs=2, space="PSUM") as psB, \
         tc.tile_pool(name="x3p", bufs=1) as x3p:
        g.psB = psB
        # x3T8[kp]: ln2(x) in fp8, DoubleRow-interleaved over feature pairs;
        # x3L8: the x8-scaled fp8 quantization residual (split precision)
        x3T8 = [x3p.tile([P, 2, 512], f8, tag=f"x3T8{c}", name=f"x3T8{c}")
                for c in range(CC // 2)]
        x3L8 = [x3p.tile([P, 2, 512], f8, tag=f"x3L8{c}", name=f"x3L8{c}")
                for c in range(CC // 2)]
        x3Tf = [x3p.tile([P, 512], f32, tag=f"x3Tf{c}", name=f"x3Tf{c}")
                for c in range(CC)]

        def ln2_in(rt):
            x2r = cp.tile([P, C], f32, tag="x2r", name="x2r")
            for c in range(CC):
                pt = g.psB.tile([P, 512], bt16, tag="psb", name="ps_t2")
                nc.tensor.transpose(pt[:P, :P], x2T[c][:, rt * P:(rt + 1) * P],
                                    g.identb[:])
                nc.scalar.copy(out=x2r[:, c * P:(c + 1) * P], in_=pt[:P, :P])
            _layernorm_pre(g, x2r)
            return x2r

        def ln2_out(x2r, rt):
            def wr2(c, pt):
                with nc.allow_low_precision(reason="ffn input quantize fp8"):
                    nc.scalar.activation(
                        out=x3T8[c // 2][:, c % 2, rt * P:(rt + 1) * P],
                        in_=pt, func=AF.Identity, bias=g.be2s[:, c:c + 1],
                        scale=g.g2s[:, c:c + 1])
                nc.scalar.activation(
                    out=x3Tf[c][:, rt * P:(rt + 1) * P], in_=pt,
                    func=AF.Identity, bias=g.sb_vec["be2f"][:, c:c + 1],
                    scale=g.g2s[:, c:c + 1])
            _layernorm_post(g, x2r, wr2)

        # software-pipelined: PE transposes of tile rt+1 overlap Act/DVE of rt
        x2rs = [ln2_in(0), ln2_in(1), ln2_in(2)]
        ln2_out(x2rs[0], 0)
        x2rs.append(ln2_in(3))
        ln2_out(x2rs[1], 1)
        ln2_out(x2rs[2], 2)
        ln2_out(x2rs[3], 3)

        # x3 fp8 residual: x3lo = fp8(8 * (x3 - b2 - dequant(x3hi)))
        for c in range(CC):
            dt_ = cp.tile([P, 512], bt16, tag="dres", name="dres")
            with nc.allow_low_precision(reason="fp8 split residual"):
                nc.vector.scalar_tensor_tensor(
                    out=dt_[:], in0=x3Tf[c][:, :],
                    scalar=g.sb_vec["b2"][:, c:c + 1],
                    in1=x3T8[c // 2][:, c % 2, :],
                    op0=OP.subtract, op1=OP.subtract)
                nc.vector.tensor_scalar(
                    out=x3L8[c // 2][:, c % 2, :], in0=dt_[:], scalar1=8.0,
                    scalar2=None, op0=OP.mult)

        # FFN in fp8 DoubleRow (256-row contraction per matmul)
        with tc.tile_pool(name="dp", bufs=1) as dp:
            h1p = [dp.tile([P, 2, 512], f8, tag=f"h1_{m}", name=f"h1_{m}")
                   for m in range(MM // 2)]
            for m in range(MM):
                w1m = g.wstr.tile([P, 3, 4, 2, P], f8, tag="wstr", name="w1m")
                nc.gpsimd.dma_start(w1m[:], g.w1[m])
                p1 = g.ps.tile([P, 512], f32, tag="ps", name="ps_f1")
                for kp in range(4):
                    nc.tensor.matmul(p1[:], w1m[:, 0, kp], x3T8[kp][:],
                                     perf_mode=mybir.MatmulPerfMode.DoubleRow,
                                     start=(kp == 0), stop=False)
                    nc.tensor.matmul(p1[:], w1m[:, 1, kp], x3T8[kp][:],
                                     perf_mode=mybir.MatmulPerfMode.DoubleRow,
                                     start=False, stop=False)
                    nc.tensor.matmul(p1[:], w1m[:, 2, kp], x3L8[kp][:],
                                     perf_mode=mybir.MatmulPerfMode.DoubleRow,
                                     start=False, stop=(kp == 3))
                with nc.allow_low_precision(reason="ffn hidden fp8"):
                    nc.scalar.activation(
                        out=h1p[m // 2][:, m % 2, :], in_=p1[:], func=AF.Gelu,
                        bias=g.sb_vec["b1"][:, m:m + 1], scale=FSC)
            for oc in range(CC):
                w2m = g.wstr.tile([P, 16, 2, P], f8, tag="wstr", name="w2m")
                nc.gpsimd.dma_start(w2m[:], g.w2[oc])
                p2 = g.ps.tile([P, 512], f32, tag="ps", name="ps_f2")
                for kp in range(16):
                    nc.tensor.matmul(p2[:], w2m[:, kp], h1p[kp][:],
                                     perf_mode=mybir.MatmulPerfMode.DoubleRow,
                                     start=(kp == 0), stop=(kp == 15))
                ot = cp.tile([P, R], f32, tag="otile", name="otile")
                nc.vector.scalar_tensor_tensor(
                    out=ot[:], in0=p2[:], scalar=FSC,
                    in1=x3Tf[oc][:, :], op0=OP.mult, op1=OP.add)
                nc.sync.dma_start(g.out[oc], ot[:])


def build_kernel():
    nc = bass.Bass("TRN2", target_bir_lowering=False, num_devices=8)
    g = Ctx()
    g.nc = nc

    # Weight layouts match the SBUF tile layout exactly so every DMA is
    # contiguous per partition (>=512B elements run at full DMA rate).
    g.xc = nc.dram_tensor("xc", [RT, C], f32, kind="ExternalInput").ap()
    g.wq = nc.dram_tensor("wq", [CC, P, CC, P], bt16, kind="ExternalInput").ap()
    g.wk = nc.dram_tensor("wk", [CC, P, CC, P], bt16, kind="ExternalInput").ap()
    g.wv = nc.dram_tensor("wv", [P, CC, C], bt16, kind="ExternalInput").ap()
    g.wo = nc.dram_tensor("wo", [P, CC, C], bt16, kind="ExternalInput").ap()
    # w1 packs three fp8 copies (hi*64, residual*64, hi*8) for the 3-term
    # split-precision FFN1: x3hi@W1hi + x3hi@W1lo + x3lo@W1hi8, all scale 64.
    g.w1 = nc.dram_tensor("w1", [MM, P, 3, 4, 2, P], f8,
                          kind="ExternalInput").ap()
    g.w2 = nc.dram_tensor("w2", [CC, P, 16, 2, P], f8,
                          kind="ExternalInput").ap()
    g.masks = nc.dram_tensor("masks", [P, 4, QB], bt16,
                             kind="ExternalInput").ap()
    # All per-feature vectors packed into one tensor (single DMA); svl/sve
    # are per-core per-slot causal-relevance bits for the V-zeroing scheme.
    VEC_SPECS = [("bq", CC), ("bk", CC), ("bo", CC), ("b1", MM),
                 ("b2", CC), ("g1", CC), ("be1", CC), ("g2", CC), ("be2", CC),
                 ("be2f", CC), ("svl", 16), ("sve", 8)]
    NVEC = sum(n for _, n in VEC_SPECS)
    g.vecs = nc.dram_tensor("vecs", [P, NVEC], f32, kind="ExternalInput").ap()
    g.out = nc.dram_tensor("out", [CC, P, R], f32, kind="ExternalOutput").ap()

    with tile.TileContext(nc) as tc:
        g.tc = tc
        with tc.tile_pool(name="setup", bufs=1) as setup, \
             tc.tile_pool(name="stats", bufs=4) as stats, \
             tc.tile_pool(name="wstr", bufs=4) as wstr, \
             tc.tile_pool(name="x1f_p", bufs=1) as x1f_p:
            g.stats, g.wstr = stats, wstr

            ident = setup.tile([P, P], f32, tag="ident", name="ident")
            make_identity(nc, ident[:])
            g.ident = ident
            identb = setup.tile([P, P], bt16, tag="identb", name="identb")
            with nc.allow_low_precision(reason="identity matrix cast"):
                nc.vector.tensor_copy(identb[:], ident[:])
            g.identb = identb
            g.ones64 = setup.tile([1, DH], bt16, tag="ones64", name="ones64")
            nc.vector.memset(g.ones64[:], 1.0)
            g.eps_sb = setup.tile([P, 1], f32, tag="eps", name="eps")
            nc.vector.memset(g.eps_sb[:], EPS)
            vt = setup.tile([P, NVEC], f32, tag="vec_all", name="vec_all")
            nc.sync.dma_start(vt[:], g.vecs)
            g.sb_vec = {}
            off = 0
            for nm, n in VEC_SPECS:
                g.sb_vec[nm] = vt[:, off:off + n]
                off += n
            g.g1s, g.be1s = g.sb_vec["g1"], g.sb_vec["be1"]
            g.g2s, g.be2s = g.sb_vec["g2"], g.sb_vec["be2"]

            g.x1f = [x1f_p.tile([P, 512], bt16, tag=f"x1f{c}", name=f"x1f{c}")
                     for c in range(CC)]

            with tc.tile_pool(name="x2p", bufs=1) as x2p:
                g.x2T = [x2p.tile([P, 512], bt16, tag=f"x2T{m}",
                                  name=f"x2T{m}") for m in range(CC)]
                with tc.tile_pool(name="kvp", bufs=1) as kvp:
                    g.kT = [[kvp.tile([P, 512], bt16, tag=f"kT{m}_{rb}",
                                      name=f"kT{m}_{rb}") for rb in range(4)]
                            for m in range(CC)]
                    g.vv = [kvp.tile([P, H, DH + 1], bt16, tag=f"vv{kt}",
                                     name=f"vv{kt}") for kt in range(T // P)]
                    g.vvE = [kvp.tile([P, H, DH + 1], bt16, tag=f"vvE{kt}",
                                      name=f"vvE{kt}") for kt in range(8)]
                    g.qT = [kvp.tile([P, 512], bt16, tag=f"qT{m}",
                                     name=f"qT{m}") for m in range(CC)]
                    with tc.tile_pool(name="psA", bufs=8, space="PSUM") as psA:
                        g.ps = psA
                        _phase_a(g)
                    _phase_b(g)
                with tc.tile_pool(name="psC", bufs=6, space="PSUM") as psC:
                    g.ps = psC
                    _phase_d(g)
    _split_sync_waits(nc)
    return nc


_NC_CACHE = None


def _get_nc():
    global _NC_CACHE
    if _NC_CACHE is None:
        _NC_CACHE = build_kernel()
    return _NC_CACHE


def _prep_shared(inputs):
    scale = DH ** -0.5
    Wq = np.asarray(inputs["Wq"], np.float32)
    Wk = np.asarray(inputs["Wk"], np.float32)
    Wv = np.asarray(inputs["Wv"], np.float32)
    Wo = np.asarray(inputs["Wo"], np.float32)
    W1 = np.asarray(inputs["W1"], np.float32)
    W2 = np.asarray(inputs["W2"], np.float32)
    bv_c = np.asarray(inputs["bv"], np.float32).reshape(C)

    def kt_layout(w):
        # [C(in), C(out)] -> [m, p, c, 128]: SBUF tile order for wkm/wqm
        cin, cout = w.shape
        return np.ascontiguousarray(
            w.reshape(cin // P, P, cout // P, P).transpose(2, 1, 0, 3)
        ).astype(bf16)

    def dr_layout(w, s=64.0):
        # [K, F] -> [m, p, kp, i, 128]: fp8 DoubleRow-interleaved, x`s` scaled
        k, fdim = w.shape
        return np.ascontiguousarray(
            (w * s).reshape(k // 256, 2, P, fdim // P, P)
            .transpose(3, 2, 0, 1, 4)).astype(fp8)

    def dr3_layout(w):
        # Three stacked fp8 DoubleRow copies: hi*64, residual*64, hi*8
        hi_mat = (w * 64.0).astype(fp8).astype(np.float32) / 64.0
        return np.ascontiguousarray(np.stack(
            [dr_layout(w, 64.0), dr_layout(w - hi_mat, 64.0),
             dr_layout(w, 8.0)], axis=2))

    def row_layout(w):
        # [C(in), F] -> [p, c, F]: SBUF tile order for wvs/wos
        cin = w.shape[0]
        return np.ascontiguousarray(
            w.reshape(cin // P, P, -1).transpose(1, 0, 2)).astype(bf16)

    def vec_layout(v):
        # [n*P] -> [P, n] (transposed into partition-major SBUF layout)
        n = v.size // P
        return np.ascontiguousarray(v.reshape(n, P).T).copy()

    shared = {
        "wq": kt_layout(Wq.transpose(1, 0, 2).reshape(C, C) * scale),
        "wk": kt_layout(Wk.transpose(1, 0, 2).reshape(C, C)),
        "wv": row_layout(Wv.transpose(1, 0, 2).reshape(C, C)),
        "wo": row_layout(Wo),
        "w1": dr3_layout(W1),
        "w2": dr_layout(W2),
        "vecs_base": np.concatenate([
            vec_layout(np.asarray(inputs["bq"], np.float32).reshape(C)
                       * scale),
            vec_layout(np.asarray(inputs["bk"], np.float32).reshape(C)),
            vec_layout(np.asarray(inputs["bo"], np.float32) + bv_c @ Wo),
            vec_layout(np.asarray(inputs["b1"], np.float32)),
            vec_layout(np.asarray(inputs["b2"], np.float32)),
            vec_layout(np.asarray(inputs["gamma1"], np.float32)),
            vec_layout(np.asarray(inputs["beta1"], np.float32)),
            vec_layout(np.asarray(inputs["gamma2"], np.float32)),
            vec_layout(np.asarray(inputs["beta2"], np.float32)),
            vec_layout(np.asarray(inputs["beta2"], np.float32)
                       + np.asarray(inputs["b2"], np.float32)),
        ], axis=1),
    }
    return shared


def _core_perm(j):
    """Slot -> key-chunk permutation: diagonal (causally partial) chunks of
    the early block at slots {0,1}, of the late block at slots {8,9}."""
    fixed = {0: 2 * j, 1: 2 * j + 1, 8: 14 - 2 * j, 9: 15 - 2 * j}
    rest = [c for c in range(16) if c not in fixed.values()]
    perm = []
    for s in range(16):
        perm.append(fixed[s] if s in fixed else rest.pop(0))
    return perm


def _core_masks(j, perm):
    """[128, 4, 256] bf16 0/1 masks for the diagonal slots: entries 0,1 mask
    slots 0,1 against the early block's columns; entries 2,3 mask slots 8,9
    against the late block's columns."""
    out = np.zeros((P, 4, QB), np.float32)
    key_p = np.arange(P)[:, None]
    col = np.arange(QB)[None, :]
    for i in range(2):
        out[:, i, :] = (perm[i] * P + key_p <= j * QB + col)
        out[:, 2 + i, :] = (perm[8 + i] * P + key_p <= (7 - j) * QB + col)
    return out.astype(bf16)


def _core_sv(j, perm):
    """Per-slot relevance bits. svl[s]: late half of slot s is causally live
    (diagonal late slots 8,9 use masks instead -> 1). sve[s]: early half of
    slot s is fully live (diagonal early slots 0,1 use masks -> 1)."""
    svl = np.zeros(16, np.float32)
    sve = np.zeros(8, np.float32)
    for s in range(16):
        if s in (8, 9) or perm[s] < 14 - 2 * j:
            svl[s] = 1.0
    for s in range(8):
        if s in (0, 1) or perm[s] < 2 * j:
            sve[s] = 1.0
    return svl, sve


def _make_in_maps(inputs):
    x = np.asarray(inputs["x"], np.float32)
    shared = _prep_shared(inputs)
    vecs_base = shared.pop("vecs_base")
    in_maps = []
    for c in range(8):
        gg, j = c // 4, c % 4
        perm = _core_perm(j)
        xb = x[gg]
        xbp = np.concatenate([xb[p * P:(p + 1) * P] for p in perm], 0)
        xq = np.concatenate([xb[j * QB:(j + 1) * QB],
                             xb[(7 - j) * QB:(8 - j) * QB]], 0)
        svl, sve = _core_sv(j, perm)
        m = dict(shared)
        m["xc"] = np.ascontiguousarray(np.concatenate([xbp, xq], 0))
        m["masks"] = _core_masks(j, perm)
        m["vecs"] = np.ascontiguousarray(np.concatenate([
            vecs_base,
            np.broadcast_to(svl[None, :], (P, 16)),
            np.broadcast_to(sve[None, :], (P, 8)),
        ], axis=1))
        in_maps.append(m)
    return in_maps


def _assemble(results):
    out = np.zeros((B, T, C), np.float32)
    for c in range(8):
        gg, j = c // 4, c % 4
        o = results[c]["out"].reshape(C, R).T  # [512, C] rows = 2 blocks
        out[gg, j * QB:(j + 1) * QB] = o[:QB]
        out[gg, (7 - j) * QB:(8 - j) * QB] = o[QB:]
    return out


def kernel(**inputs):
    in_maps = _make_in_maps(inputs)
    nc = _get_nc()
    res = run_bass_kernel_spmd(nc, in_maps, core_ids=list(range(8)))
    return _assemble(res.results)



# revision 77
# speedup vs baseline: 1.6719x; 1.0335x over previous
"""Trainium2 Bass kernel for a dense transformer block (B=2, T=2048, C=1024,
H=16, DFF=4096), distributed over 8 NeuronCores.

Sharding: 2 batch groups x 4-way query-block sharding. Core c handles batch
g=c//4 and query blocks {j, 7-j} (j=c%4) of 8 blocks of 256 rows. K/V are
computed per-core for the full batch (replicated; no collectives). Causality
is exploited statically: key-chunks 0-7 are needed by both query blocks
(masked only on block-0's columns), chunks 8-15 only by the late block.
The data-dependent causal boundary is applied with per-core 0/1 masks so one
NEFF serves all 8 cores (SPMD).

Activations are kept feature-major ("xT") so every matmul chains without
transposes; layernorm runs row-major with PE transposes between domains.
Softmax denominators ride the AV matmul as an extra ones-column of V.
"""
import numpy as np
import ml_dtypes

import concourse.bass as bass
import concourse.mybir as mybir
import concourse.tile as tile
from concourse.vector_clock import ScopedClock
from concourse.bass_utils import run_bass_kernel_spmd
from concourse.masks import make_identity

bf16 = ml_dtypes.bfloat16
fp8 = ml_dtypes.float8_e4m3
f32 = mybir.dt.float32
bt16 = mybir.dt.bfloat16
f8 = mybir.dt.float8e4
AF = mybir.ActivationFunctionType
OP = mybir.AluOpType

B, T, C, H, DH, DFF = 2, 2048, 1024, 16, 64, 4096
P = 128
QB = 256            # rows per query block
R = 512             # own query rows per core
RT = T + R          # ln1 rows per core (full batch + own q rows)
CC = C // P         # 8 feature chunks
MM = DFF // P       # 32 ffn chunks
EPS = 1e-5


# ---------------------------------------------------------------------------
# The walrus build in this container rejects instructions with >1 sync wait.
# Tile's sem assignment can emit several on one instruction; split the excess
# onto same-engine NoOps placed immediately before.
def _patched_drain_and_barrier(self, tick_clock, wait_clock):
    nc = self.nc
    probe = nc.sync.nop(nofuse=True, hint="tail_wait_probe")
    wait_clock.add_sem_waits(probe.ins, ScopedClock({None: tick_clock.global_clock}))
    si = probe.ins.sync_info
    waits = list(si.on_wait) if si is not None else []
    if si is not None:
        si.on_wait = waits[:1]
    for w in waits[1:]:
        n2 = nc.sync.nop(nofuse=True, hint="tail_wait_split")
        n2.ins.sync_info = mybir.SyncInfo(on_wait=[w], on_update=[])
    nc.sync.drain()
    nc.all_engine_barrier()
    assert self.sems is not None
    popped = nc._tile_sem_poison_stack.pop()
    assert popped is self._sem_poison
    nc.clear_and_free_semaphores(list(self.sems.allocated().values()))
    nc.all_engine_barrier()


tile.TileContext._drain_and_barrier = _patched_drain_and_barrier

_MAX_WAITS = 1
_split_counter = [0]


def _split_sync_waits(nc):
    for fn in nc.m.functions:
        for bb in fn.blocks:
            new_insts = []
            for inst in bb.instructions:
                si = getattr(inst, "sync_info", None)
                lim = _MAX_WAITS
                if si is not None and si.on_wait and len(si.on_wait) > lim:
                    waits = list(si.on_wait)
                    keep = waits[-lim:]
                    excess = waits[:-lim]
                    for i in range(0, len(excess), _MAX_WAITS):
                        _split_counter[0] += 1
                        nop = mybir.InstNoOp(
                            name=f"I-wsplit-{_split_counter[0]}", ins=[], outs=[])
                        nop.engine = inst.engine
                        nop.sync_info = mybir.SyncInfo(
                            on_wait=excess[i:i + _MAX_WAITS], on_update=[])
                        new_insts.append(nop)
                    si.on_wait = keep
                new_insts.append(inst)
            bb.instructions = new_insts
# ---------------------------------------------------------------------------


class Ctx:
    pass


def _layernorm_pre(g, xt):
    """Row-major LN of xt [128, C] in place (DVE + one tiny Act sqrt)."""
    nc = g.nc
    st = g.stats.tile([P, 2, 6], f32, tag="bnst", name="bnst")
    xv = xt.rearrange("p (s d) -> p s d", s=2)
    for sg in range(2):
        nc.vector.bn_stats(out=st[:, sg, :], in_=xv[:, sg, :])
    mv = g.stats.tile([P, 2], f32, tag="bnmv", name="bnmv")
    nc.vector.bn_aggr(out=mv[:], in_=st[:])
    sq = g.stats.tile([P, 1], f32, tag="bnsq", name="bnsq")
    nc.scalar.activation(out=sq[:], in_=mv[:, 1:2], func=AF.Sqrt,
                         bias=g.eps_sb[:], scale=float(C) / (C - 1))
    rstd = g.stats.tile([P, 1], f32, tag="bnrstd", name="bnrstd")
    nc.vector.reciprocal(rstd[:], sq[:])
    nc.vector.tensor_scalar(out=xt[:], in0=xt[:], scalar1=mv[:, 0:1],
                            scalar2=rstd[:], op0=OP.subtract, op1=OP.mult)


def _layernorm_post(g, xt, out_writes):
    """Transpose normalized xt per feature chunk; out_writes(c, psum_ap)."""
    nc = g.nc
    for c in range(CC):
        pt = g.ps.tile([P, 512], f32, tag="ps", name="ps_t")
        nc.tensor.transpose(pt[:P, :P], xt[:, c * P:(c + 1) * P], g.ident[:])
        out_writes(c, pt[:P, :P])


def _layernorm_tile(g, xt, out_writes):
    _layernorm_pre(g, xt)
    _layernorm_post(g, xt, out_writes)


def _ln1_pre(g, rt):
    """DMA + row-major LN of tile rt; returns the normalized xt tile."""
    nc = g.nc
    xt = g.xio.tile([P, C], f32, tag="xin", name="xin")
    nc.sync.dma_start(xt[:], g.xc[rt * P:(rt + 1) * P, :])
    _layernorm_pre(g, xt)
    return xt


def _ln1_post(g, x1T, xt, rt):
    nc = g.nc
    rb, r0 = rt // 4, (rt % 4) * P

    def wr1(c, pt):
        nc.scalar.activation(
            out=x1T[rb][:, c, r0:r0 + P], in_=pt, func=AF.Identity,
            bias=g.be1s[:, c:c + 1], scale=g.g1s[:, c:c + 1])
        if rt >= T // P:
            q0 = (rt - T // P) * P
            nc.scalar.activation(
                out=g.x1f[c][:, q0:q0 + P], in_=pt, func=AF.Identity,
                bias=g.be1s[:, c:c + 1], scale=g.g1s[:, c:c + 1])
    _layernorm_post(g, xt, wr1)


def _wk_fetch(g, m):
    wkm = g.wstr.tile([P, CC, P], bt16, tag="wstr", name="wkm")
    g.nc.gpsimd.dma_start(wkm[:, :, :], g.wk[m])
    return wkm


def _kproj_chunk(g, x1T, rb, m, wkm=None):
    """K^T projection of row-block rb, output chunk m."""
    nc = g.nc
    if wkm is None:
        wkm = _wk_fetch(g, m)
    pk = g.ps.tile([P, 512], f32, tag="ps", name="ps_k")
    for c in range(CC):
        nc.tensor.matmul(pk[:], wkm[:, c, :], x1T[rb][:, c, :],
                         start=(c == 0), stop=(c == CC - 1))
    nc.vector.tensor_scalar(
        out=g.kT[m][rb][:, :], in0=pk[:],
        scalar1=g.sb_vec["bk"][:, m:m + 1], scalar2=None, op0=OP.add)


def _vproj_kt(g, x1T, wvs, kt):
    """V projection (row-major, both 512-col halves) for key-tile kt.
    Writes vv[kt] scaled by the per-core late-relevance bit svl[kt], and (for
    kt < 8) vvE[kt] scaled by the early-relevance bit sve[kt]."""
    nc = g.nc
    rb, r0 = kt // 4, (kt % 4) * P
    for half in range(2):
        pv = g.ps.tile([P, 512], f32, tag="ps", name="ps_v")
        for c in range(CC):
            nc.tensor.matmul(pv[:], x1T[rb][:, c, r0:r0 + P],
                             wvs[:, c, half * 512:(half + 1) * 512],
                             start=(c == 0), stop=(c == CC - 1))
        pvh = pv.rearrange("p (h d) -> p h d", h=8)
        nc.vector.tensor_scalar(
            out=g.vv[kt][:, half * 8:(half + 1) * 8, 0:DH], in0=pvh,
            scalar1=g.sb_vec["svl"][:, kt:kt + 1], scalar2=None, op0=OP.mult)
        if kt < 8:
            nc.vector.tensor_scalar(
                out=g.vvE[kt][:, half * 8:(half + 1) * 8, 0:DH], in0=pvh,
                scalar1=g.sb_vec["sve"][:, kt:kt + 1], scalar2=None,
                op0=OP.mult)


def _phase_a(g):
    """LN1 + transpose + Q/K/V projections, software-pipelined at tile
    granularity: LN stats (DVE) run ahead; each LN transpose batch is
    staggered between projection matmul chunks of the previous row-block so
    the in-order PE stream never waits long."""
    nc, tc = g.nc, g.tc
    with tc.tile_pool(name="x1p", bufs=1) as x1p, \
         tc.tile_pool(name="xio", bufs=3) as xio, \
         tc.tile_pool(name="wvp", bufs=1) as wvp:
        g.xio = xio
        # x1T split per 512-row block (rb 0-3 = batch, rb 4 = own q rows)
        x1T = [x1p.tile([P, CC, 512], bt16, tag=f"x1T{rb}", name=f"x1T{rb}")
               for rb in range(5)]
        wvs = wvp.tile([P, CC, C], bt16, tag="wvs", name="wvs")
        # Prologue: LN of row-block 0 (no projections to overlap with yet)
        xts = [_ln1_pre(g, rt) for rt in range(3)]
        _ln1_post(g, x1T, xts[0], 0)
        xts.append(_ln1_pre(g, 3))
        _ln1_post(g, x1T, xts[1], 1)
        _ln1_post(g, x1T, xts[2], 2)
        _ln1_post(g, x1T, xts[3], 3)

        # ones columns (softmax denominator), scaled by per-slot relevance
        for kt in range(T // P):
            nc.vector.memset(g.vv[kt][:, :, DH:DH + 1], 1.0)
            nc.vector.tensor_scalar(
                out=g.vv[kt][:, :, DH:DH + 1], in0=g.vv[kt][:, :, DH:DH + 1],
                scalar1=g.sb_vec["svl"][:, kt:kt + 1], scalar2=None,
                op0=OP.mult)
        for kt in range(8):
            nc.vector.memset(g.vvE[kt][:, :, DH:DH + 1], 1.0)
            nc.vector.tensor_scalar(
                out=g.vvE[kt][:, :, DH:DH + 1],
                in0=g.vvE[kt][:, :, DH:DH + 1],
                scalar1=g.sb_vec["sve"][:, kt:kt + 1], scalar2=None,
                op0=OP.mult)

        # Steady state: projections of rb overlap LN of rb+1.
        # PE emission order per rb: K(m0) T(t0) K(m1) T(t1) K(m2) T(t2)
        # K(m3) T(t3) K(m4..7) V(kt*4); LN-pre(t_i) is emitted just before
        # K(m_i) so DVE stats run one matmul-chunk ahead of the transpose.
        # The first two K-weight chunks of rb+1 are prefetched before the
        # V block so the next iteration's PE stream starts without a DMA
        # stall; wvs (V weights) is fetched late enough not to block the
        # startup x-tile loads.
        wk_pre = [_wk_fetch(g, 0), _wk_fetch(g, 1)]
        for rb in range(4):
            for m in range(CC):
                if rb == 0 and m == 3:
                    nc.gpsimd.dma_start(wvs[:], g.wv)
                if m < 4:
                    xt = _ln1_pre(g, (rb + 1) * 4 + m)
                _kproj_chunk(g, x1T, rb, m,
                             wkm=wk_pre[m] if m < 2 else None)
                if m < 4:
                    _ln1_post(g, x1T, xt, (rb + 1) * 4 + m)
            if rb < 3:
                wk_pre = [_wk_fetch(g, 0), _wk_fetch(g, 1)]
            for i in range(4):
                _vproj_kt(g, x1T, wvs, rb * 4 + i)

        # Q^T projection of the own-query rows (x1T[4])
        for m in range(CC):
            wqm = g.wstr.tile([P, CC, P], bt16, tag="wstr", name="wqm")
            nc.gpsimd.dma_start(wqm[:, :, :], g.wq[m])
            pq = g.ps.tile([P, 512], f32, tag="ps", name="ps_q")
            for c in range(CC):
                nc.tensor.matmul(pq[:], wqm[:, c, :], x1T[4][:, c, :],
                                 start=(c == 0), stop=(c == CC - 1))
            nc.vector.tensor_scalar(
                out=g.qT[m][:, :], in0=pq[:],
                scalar1=g.sb_vec["bq"][:, m:m + 1], scalar2=None, op0=OP.add)


def _phase_b(g):
    """Attention, both query blocks fused on the free dim (cols 0:256 = early
    block, 256:512 = late block). Key chunks arrive permuted per core so the
    causally-partial (diagonal) chunks sit at slots {0,1} (early) / {8,9}
    (late); all other slots need no elementwise mask because the V copies
    (vvE for the early half, vv for the late half) are zeroed per-slot when
    that chunk is causally irrelevant, nulling both numerator and the
    ones-column denominator. exp runs on 2-PSUM-bank batches; the softmax
    denominator reciprocal is broadcast on the idle GpSimd engine."""
    nc, tc = g.nc, g.tc
    with tc.tile_pool(name="mp", bufs=1) as mp, \
         tc.tile_pool(name="apl", bufs=2) as apl, \
         tc.tile_pool(name="rcp", bufs=2) as rcp, \
         tc.tile_pool(name="hp", bufs=1) as hp, \
         tc.tile_pool(name="wop", bufs=1) as wop, \
         tc.tile_pool(name="scp", bufs=3, space="PSUM") as scp, \
         tc.tile_pool(name="pvp", bufs=2, space="PSUM") as pvp:
        g.hcat = [hp.tile([P, 512], bt16, tag=f"hcat{c}", name=f"hcat{c}")
                  for c in range(CC)]
        mq = mp.tile([P, 4, QB], bt16, tag="mask", name="mask")
        nc.sync.dma_start(mq[:], g.masks)
        # prefetch the Wo weights during attention (DMA engine is idle here)
        wos = wop.tile([P, CC, C], bt16, tag="wos", name="wos")
        nc.gpsimd.dma_start(wos[:], g.wo)
        for pair in range(CC):
            for hl in range(2):
                h = 2 * pair + hl
                hs = slice(hl * DH, (hl + 1) * DH)
                aA = apl.tile([P, 8, 512], bt16, tag="aA", name="aA")
                aB = apl.tile([P, 8, QB], bt16, tag="aB", name="aB")
                # scores slots 0..7 (512 wide): 2 slots per 2-bank psum tile
                for t4 in range(4):
                    psc = scp.tile([P, 1024], f32, tag="sc", name="ps_s")
                    for k in range(2):
                        s = 2 * t4 + k
                        rb, k0 = s // 4, (s % 4) * P
                        nc.tensor.matmul(
                            psc[:, k * 512:(k + 1) * 512],
                            g.kT[pair][rb][hs, k0:k0 + P],
                            g.qT[pair][hs, 0:512], start=True, stop=True,
                            tile_position=(hl * DH, 0))
                    nc.scalar.activation(out=aA[:, 2 * t4:2 * t4 + 2, :],
                                         in_=psc[:], func=AF.Exp)
                # slots 8..15 (late block only, 256 wide): 4 per psum tile
                for t4 in range(2):
                    psc = scp.tile([P, 1024], f32, tag="sc", name="ps_sB")
                    for k in range(4):
                        s = 8 + 4 * t4 + k
                        rb, k0 = s // 4, (s % 4) * P
                        nc.tensor.matmul(
                            psc[:, k * QB:(k + 1) * QB],
                            g.kT[pair][rb][hs, k0:k0 + P],
                            g.qT[pair][hs, QB:512], start=True, stop=True,
                            tile_position=(hl * DH, 0))
                    nc.scalar.activation(out=aB[:, 4 * t4:4 * t4 + 4, :],
                                         in_=psc[:], func=AF.Exp)
                # elementwise causal masks only on the diagonal slots
                nc.vector.tensor_mul(aA[:, 0:2, 0:QB], aA[:, 0:2, 0:QB],
                                     mq[:, 0:2, :])
                nc.vector.tensor_mul(aB[:, 0:2, :], aB[:, 0:2, :],
                                     mq[:, 2:4, :])
                pav = pvp.tile([P, 512], f32, tag="pav", name="ps_av")
                for s in range(8):
                    nc.tensor.matmul(pav[:DH + 1, 0:QB], g.vvE[s][:, h, :],
                                     aA[:, s, 0:QB], start=(s == 0),
                                     stop=(s == 7))
                for s in range(8):
                    nc.tensor.matmul(pav[:DH + 1, QB:512], g.vv[s][:, h, :],
                                     aA[:, s, QB:512], start=(s == 0),
                                     stop=False)
                for s in range(8, 16):
                    nc.tensor.matmul(pav[:DH + 1, QB:512], g.vv[s][:, h, :],
                                     aB[:, s - 8, :], start=False,
                                     stop=(s == 15))
                den = g.stats.tile([1, 512], f32, tag="den", name="den")
                nc.vector.tensor_copy(den[:], pav[DH:DH + 1, :512])
                rr = g.stats.tile([1, 512], bt16, tag="rr", name="rr")
                with nc.allow_low_precision(reason="softmax denom in bf16"):
                    nc.vector.reciprocal(rr[:], den[:])
                prb = pvp.tile([P, 512], f32, tag="pav", name="ps_r")
                nc.tensor.matmul(prb[:DH, :512], g.ones64[:], rr[:],
                                 start=True, stop=True)
                rbc = rcp.tile([DH, 512], bt16, tag="rbc", name="rbc")
                with nc.allow_low_precision(reason="softmax denom in bf16"):
                    nc.vector.tensor_copy(rbc[:], prb[:DH, :512])
                nc.vector.tensor_mul(out=g.hcat[pair][hs, :],
                                     in0=pav[:DH, :512], in1=rbc[:])

        # Wo projection + residual (uses hcat/x1f/wos before they die)
        for m in range(CC):
            pa = pvp.tile([P, 512], f32, tag="pav", name="ps_o")
            for c in range(CC):
                nc.tensor.matmul(pa[:, :512], wos[:, c, m * P:(m + 1) * P],
                                 g.hcat[c][:, :], start=(c == 0),
                                 stop=(c == CC - 1))
            with nc.allow_low_precision(reason="x2 residual in bf16"):
                nc.vector.scalar_tensor_tensor(
                    out=g.x2T[m][:, :], in0=pa[:, :512],
                    scalar=g.sb_vec["bo"][:, m:m + 1], in1=g.x1f[m][:, :],
                    op0=OP.add, op1=OP.add)


def _phase_d(g):
    """LN2 + FFN (fp8 DoubleRow) + output.

    FFN weights are pre-scaled by 64 on the host so fp8e4 quantization stays
    in the normal range; the 1/64 rides the gelu's scale operand (FFN1) and
    the epilogue's scalar multiply (FFN2). b2 is pre-folded into the f32
    residual copy of ln2(x) (bias be2f = beta2 + b2)."""
    nc, tc = g.nc, g.tc
    FSC = 1.0 / 64
    x2T = g.x2T
    with tc.tile_pool(name="cp", bufs=3) as cp, \
         tc.tile_pool(name="psB", bufs=2, space="PSUM") as psB, \
         tc.tile_pool(name="x3p", bufs=1) as x3p:
        g.psB = psB
        # x3T8[kp]: ln2(x) in fp8, DoubleRow-interleaved over feature pairs;
        # x3L8: the x8-scaled fp8 quantization residual (split precision)
        x3T8 = [x3p.tile([P, 2, 512], f8, tag=f"x3T8{c}", name=f"x3T8{c}")
                for c in range(CC // 2)]
        x3L8 = [x3p.tile([P, 2, 512], f8, tag=f"x3L8{c}", name=f"x3L8{c}")
                for c in range(CC // 2)]
        x3Tf = [x3p.tile([P, 512], f32, tag=f"x3Tf{c}", name=f"x3Tf{c}")
                for c in range(CC)]

        def ln2_in(rt):
            x2r = cp.tile([P, C], f32, tag="x2r", name="x2r")
            for c in range(CC):
                pt = g.psB.tile([P, 512], bt16, tag="psb", name="ps_t2")
                nc.tensor.transpose(pt[:P, :P], x2T[c][:, rt * P:(rt + 1) * P],
                                    g.identb[:])
                nc.scalar.copy(out=x2r[:, c * P:(c + 1) * P], in_=pt[:P, :P])
            _layernorm_pre(g, x2r)
            return x2r

        def ln2_out(x2r, rt):
            def wr2(c, pt):
                with nc.allow_low_precision(reason="ffn input quantize fp8"):
                    nc.scalar.activation(
                        out=x3T8[c // 2][:, c % 2, rt * P:(rt + 1) * P],
                        in_=pt, func=AF.Identity, bias=g.be2s[:, c:c + 1],
                        scale=g.g2s[:, c:c + 1])
                nc.scalar.activation(
                    out=x3Tf[c][:, rt * P:(rt + 1) * P], in_=pt,
                    func=AF.Identity, bias=g.sb_vec["be2f"][:, c:c + 1],
                    scale=g.g2s[:, c:c + 1])
            _layernorm_post(g, x2r, wr2)

        # software-pipelined: PE transposes of tile rt+1 overlap Act/DVE of rt
        x2rs = [ln2_in(0), ln2_in(1), ln2_in(2)]
        ln2_out(x2rs[0], 0)
        x2rs.append(ln2_in(3))
        ln2_out(x2rs[1], 1)
        ln2_out(x2rs[2], 2)
        ln2_out(x2rs[3], 3)

        # x3 fp8 residual: x3lo = fp8(8 * (x3 - b2 - dequant(x3hi)))
        for c in range(CC):
            dt_ = cp.tile([P, 512], bt16, tag="dres", name="dres")
            with nc.allow_low_precision(reason="fp8 split residual"):
                nc.vector.scalar_tensor_tensor(
                    out=dt_[:], in0=x3Tf[c][:, :],
                    scalar=g.sb_vec["b2"][:, c:c + 1],
                    in1=x3T8[c // 2][:, c % 2, :],
                    op0=OP.subtract, op1=OP.subtract)
                nc.vector.tensor_scalar(
                    out=x3L8[c // 2][:, c % 2, :], in0=dt_[:], scalar1=8.0,
                    scalar2=None, op0=OP.mult)

        # FFN in fp8 DoubleRow (256-row contraction per matmul); weights
        # stream through dedicated deep-prefetch pools so transfers overlap
        # the Act-bound LN2 stretch above.
        with tc.tile_pool(name="dp", bufs=1) as dp, \
             tc.tile_pool(name="w1p", bufs=10) as w1p, \
             tc.tile_pool(name="w2p", bufs=4) as w2p:
            h1p = [dp.tile([P, 2, 512], f8, tag=f"h1_{m}", name=f"h1_{m}")
                   for m in range(MM // 2)]
            for m in range(MM):
                w1m = w1p.tile([P, 3, 4, 2, P], f8, tag="w1m", name="w1m")
                nc.gpsimd.dma_start(w1m[:], g.w1[m])
                p1 = g.ps.tile([P, 512], f32, tag="ps", name="ps_f1")
                for kp in range(4):
                    nc.tensor.matmul(p1[:], w1m[:, 0, kp], x3T8[kp][:],
                                     perf_mode=mybir.MatmulPerfMode.DoubleRow,
                                     start=(kp == 0), stop=False)
                    nc.tensor.matmul(p1[:], w1m[:, 1, kp], x3T8[kp][:],
                                     perf_mode=mybir.MatmulPerfMode.DoubleRow,
                                     start=False, stop=False)
                    nc.tensor.matmul(p1[:], w1m[:, 2, kp], x3L8[kp][:],
                                     perf_mode=mybir.MatmulPerfMode.DoubleRow,
                                     start=False, stop=(kp == 3))
                with nc.allow_low_precision(reason="ffn hidden fp8"):
                    nc.scalar.activation(
                        out=h1p[m // 2][:, m % 2, :], in_=p1[:], func=AF.Gelu,
                        bias=g.sb_vec["b1"][:, m:m + 1], scale=FSC)
            for oc in range(CC):
                w2m = w2p.tile([P, 16, 2, P], f8, tag="w2m", name="w2m")
                nc.gpsimd.dma_start(w2m[:], g.w2[oc])
                p2 = g.ps.tile([P, 512], f32, tag="ps", name="ps_f2")
                for kp in range(16):
                    nc.tensor.matmul(p2[:], w2m[:, kp], h1p[kp][:],
                                     perf_mode=mybir.MatmulPerfMode.DoubleRow,
                                     start=(kp == 0), stop=(kp == 15))
                ot = cp.tile([P, R], f32, tag="otile", name="otile")
                nc.vector.scalar_tensor_tensor(
                    out=ot[:], in0=p2[:], scalar=FSC,
                    in1=x3Tf[oc][:, :], op0=OP.mult, op1=OP.add)
                nc.sync.dma_start(g.out[oc], ot[:])


def build_kernel():
    nc = bass.Bass("TRN2", target_bir_lowering=False, num_devices=8)
    g = Ctx()
    g.nc = nc

    # Weight layouts match the SBUF tile layout exactly so every DMA is
    # contiguous per partition (>=512B elements run at full DMA rate).
    g.xc = nc.dram_tensor("xc", [RT, C], f32, kind="ExternalInput").ap()
    g.wq = nc.dram_tensor("wq", [CC, P, CC, P], bt16, kind="ExternalInput").ap()
    g.wk = nc.dram_tensor("wk", [CC, P, CC, P], bt16, kind="ExternalInput").ap()
    g.wv = nc.dram_tensor("wv", [P, CC, C], bt16, kind="ExternalInput").ap()
    g.wo = nc.dram_tensor("wo", [P, CC, C], bt16, kind="ExternalInput").ap()
    # w1 packs three fp8 copies (hi*64, residual*64, hi*8) for the 3-term
    # split-precision FFN1: x3hi@W1hi + x3hi@W1lo + x3lo@W1hi8, all scale 64.
    g.w1 = nc.dram_tensor("w1", [MM, P, 3, 4, 2, P], f8,
                          kind="ExternalInput").ap()
    g.w2 = nc.dram_tensor("w2", [CC, P, 16, 2, P], f8,
                          kind="ExternalInput").ap()
    g.masks = nc.dram_tensor("masks", [P, 4, QB], bt16,
                             kind="ExternalInput").ap()
    # All per-feature vectors packed into one tensor (single DMA); svl/sve
    # are per-core per-slot causal-relevance bits for the V-zeroing scheme.
    VEC_SPECS = [("bq", CC), ("bk", CC), ("bo", CC), ("b1", MM),
                 ("b2", CC), ("g1", CC), ("be1", CC), ("g2", CC), ("be2", CC),
                 ("be2f", CC), ("svl", 16), ("sve", 8)]
    NVEC = sum(n for _, n in VEC_SPECS)
    g.vecs = nc.dram_tensor("vecs", [P, NVEC], f32, kind="ExternalInput").ap()
    g.out = nc.dram_tensor("out", [CC, P, R], f32, kind="ExternalOutput").ap()

    with tile.TileContext(nc) as tc:
        g.tc = tc
        with tc.tile_pool(name="setup", bufs=1) as setup, \
             tc.tile_pool(name="stats", bufs=4) as stats, \
             tc.tile_pool(name="wstr", bufs=4) as wstr, \
             tc.tile_pool(name="x1f_p", bufs=1) as x1f_p:
            g.stats, g.wstr = stats, wstr

            ident = setup.tile([P, P], f32, tag="ident", name="ident")
            make_identity(nc, ident[:])
            g.ident = ident
            identb = setup.tile([P, P], bt16, tag="identb", name="identb")
            with nc.allow_low_precision(reason="identity matrix cast"):
                nc.vector.tensor_copy(identb[:], ident[:])
            g.identb = identb
            g.ones64 = setup.tile([1, DH], bt16, tag="ones64", name="ones64")
            nc.vector.memset(g.ones64[:], 1.0)
            g.eps_sb = setup.tile([P, 1], f32, tag="eps", name="eps")
            nc.vector.memset(g.eps_sb[:], EPS)
            vt = setup.tile([P, NVEC], f32, tag="vec_all", name="vec_all")
            nc.sync.dma_start(vt[:], g.vecs)
            g.sb_vec = {}
            off = 0
            for nm, n in VEC_SPECS:
                g.sb_vec[nm] = vt[:, off:off + n]
                off += n
            g.g1s, g.be1s = g.sb_vec["g1"], g.sb_vec["be1"]
            g.g2s, g.be2s = g.sb_vec["g2"], g.sb_vec["be2"]

            g.x1f = [x1f_p.tile([P, 512], bt16, tag=f"x1f{c}", name=f"x1f{c}")
                     for c in range(CC)]

            with tc.tile_pool(name="x2p", bufs=1) as x2p:
                g.x2T = [x2p.tile([P, 512], bt16, tag=f"x2T{m}",
                                  name=f"x2T{m}") for m in range(CC)]
                with tc.tile_pool(name="kvp", bufs=1) as kvp:
                    g.kT = [[kvp.tile([P, 512], bt16, tag=f"kT{m}_{rb}",
                                      name=f"kT{m}_{rb}") for rb in range(4)]
                            for m in range(CC)]
                    g.vv = [kvp.tile([P, H, DH + 1], bt16, tag=f"vv{kt}",
                                     name=f"vv{kt}") for kt in range(T // P)]
                    g.vvE = [kvp.tile([P, H, DH + 1], bt16, tag=f"vvE{kt}",
                                      name=f"vvE{kt}") for kt in range(8)]
                    g.qT = [kvp.tile([P, 512], bt16, tag=f"qT{m}",
                                     name=f"qT{m}") for m in range(CC)]
                    with tc.tile_pool(name="psA", bufs=8, space="PSUM") as psA:
                        g.ps = psA
                        _phase_a(g)
                    _phase_b(g)
                with tc.tile_pool(name="psC", bufs=6, space="PSUM") as psC:
                    g.ps = psC
                    _phase_d(g)
    _split_sync_waits(nc)
    return nc


_NC_CACHE = None


def _get_nc():
    global _NC_CACHE
    if _NC_CACHE is None:
        _NC_CACHE = build_kernel()
    return _NC_CACHE


def _prep_shared(inputs):
    scale = DH ** -0.5
    Wq = np.asarray(inputs["Wq"], np.float32)
    Wk = np.asarray(inputs["Wk"], np.float32)
    Wv = np.asarray(inputs["Wv"], np.float32)
    Wo = np.asarray(inputs["Wo"], np.float32)
    W1 = np.asarray(inputs["W1"], np.float32)
    W2 = np.asarray(inputs["W2"], np.float32)
    bv_c = np.asarray(inputs["bv"], np.float32).reshape(C)

    def kt_layout(w):
        # [C(in), C(out)] -> [m, p, c, 128]: SBUF tile order for wkm/wqm
        cin, cout = w.shape
        return np.ascontiguousarray(
            w.reshape(cin // P, P, cout // P, P).transpose(2, 1, 0, 3)
        ).astype(bf16)

    def dr_layout(w, s=64.0):
        # [K, F] -> [m, p, kp, i, 128]: fp8 DoubleRow-interleaved, x`s` scaled
        k, fdim = w.shape
        return np.ascontiguousarray(
            (w * s).reshape(k // 256, 2, P, fdim // P, P)
            .transpose(3, 2, 0, 1, 4)).astype(fp8)

    def dr3_layout(w):
        # Three stacked fp8 DoubleRow copies: hi*64, residual*64, hi*8
        hi_mat = (w * 64.0).astype(fp8).astype(np.float32) / 64.0
        return np.ascontiguousarray(np.stack(
            [dr_layout(w, 64.0), dr_layout(w - hi_mat, 64.0),
             dr_layout(w, 8.0)], axis=2))

    def row_layout(w):
        # [C(in), F] -> [p, c, F]: SBUF tile order for wvs/wos
        cin = w.shape[0]
        return np.ascontiguousarray(
            w.reshape(cin // P, P, -1).transpose(1, 0, 2)).astype(bf16)

    def vec_layout(v):
        # [n*P] -> [P, n] (transposed into partition-major SBUF layout)
        n = v.size // P
        return np.ascontiguousarray(v.reshape(n, P).T).copy()

    shared = {
        "wq": kt_layout(Wq.transpose(1, 0, 2).reshape(C, C) * scale),
        "wk": kt_layout(Wk.transpose(1, 0, 2).reshape(C, C)),
        "wv": row_layout(Wv.transpose(1, 0, 2).reshape(C, C)),
        "wo": row_layout(Wo),
        "w1": dr3_layout(W1),
        "w2": dr_layout(W2),
        "vecs_base": np.concatenate([
            vec_layout(np.asarray(inputs["bq"], np.float32).reshape(C)
                       * scale),
            vec_layout(np.asarray(inputs["bk"], np.float32).reshape(C)),
            vec_layout(np.asarray(inputs["bo"], np.float32) + bv_c @ Wo),
            vec_layout(np.asarray(inputs["b1"], np.float32)),
            vec_layout(np.asarray(inputs["b2"], np.float32)),
            vec_layout(np.asarray(inputs["gamma1"], np.float32)),
            vec_layout(np.asarray(inputs["beta1"], np.float32)),
            vec_layout(np.asarray(inputs["gamma2"], np.float32)),
            vec_layout(np.asarray(inputs["beta2"], np.float32)),
            vec_layout(np.asarray(inputs["beta2"], np.float32)
                       + np.asarray(inputs["b2"], np.float32)),
        ], axis=1),
    }
    return shared


def _core_perm(j):
    """Slot -> key-chunk permutation: diagonal (causally partial) chunks of
    the early block at slots {0,1}, of the late block at slots {8,9}."""
    fixed = {0: 2 * j, 1: 2 * j + 1, 8: 14 - 2 * j, 9: 15 - 2 * j}
    rest = [c for c in range(16) if c not in fixed.values()]
    perm = []
    for s in range(16):
        perm.append(fixed[s] if s in fixed else rest.pop(0))
    return perm


def _core_masks(j, perm):
    """[128, 4, 256] bf16 0/1 masks for the diagonal slots: entries 0,1 mask
    slots 0,1 against the early block's columns; entries 2,3 mask slots 8,9
    against the late block's columns."""
    out = np.zeros((P, 4, QB), np.float32)
    key_p = np.arange(P)[:, None]
    col = np.arange(QB)[None, :]
    for i in range(2):
        out[:, i, :] = (perm[i] * P + key_p <= j * QB + col)
        out[:, 2 + i, :] = (perm[8 + i] * P + key_p <= (7 - j) * QB + col)
    return out.astype(bf16)


def _core_sv(j, perm):
    """Per-slot relevance bits. svl[s]: late half of slot s is causally live
    (diagonal late slots 8,9 use masks instead -> 1). sve[s]: early half of
    slot s is fully live (diagonal early slots 0,1 use masks -> 1)."""
    svl = np.zeros(16, np.float32)
    sve = np.zeros(8, np.float32)
    for s in range(16):
        if s in (8, 9) or perm[s] < 14 - 2 * j:
            svl[s] = 1.0
    for s in range(8):
        if s in (0, 1) or perm[s] < 2 * j:
            sve[s] = 1.0
    return svl, sve


def _make_in_maps(inputs):
    x = np.asarray(inputs["x"], np.float32)
    shared = _prep_shared(inputs)
    vecs_base = shared.pop("vecs_base")
    in_maps = []
    for c in range(8):
        gg, j = c // 4, c % 4
        perm = _core_perm(j)
        xb = x[gg]
        xbp = np.concatenate([xb[p * P:(p + 1) * P] for p in perm], 0)
        xq = np.concatenate([xb[j * QB:(j + 1) * QB],
                             xb[(7 - j) * QB:(8 - j) * QB]], 0)
        svl, sve = _core_sv(j, perm)
        m = dict(shared)
        m["xc"] = np.ascontiguousarray(np.concatenate([xbp, xq], 0))
        m["masks"] = _core_masks(j, perm)
        m["vecs"] = np.ascontiguousarray(np.concatenate([
            vecs_base,
            np.broadcast_to(svl[None, :], (P, 16)),
            np.broadcast_to(sve[None, :], (P, 8)),
        ], axis=1))
        in_maps.append(m)
    return in_maps


def _assemble(results):
    out = np.zeros((B, T, C), np.float32)
    for c in range(8):
        gg, j = c // 4, c % 4
        o = results[c]["out"].reshape(C, R).T  # [512, C] rows = 2 blocks
        out[gg, j * QB:(j + 1) * QB] = o[:QB]
        out[gg, (7 - j) * QB:(8 - j) * QB] = o[QB:]
    return out


def kernel(**inputs):
    in_maps = _make_in_maps(inputs)
    nc = _get_nc()
    res = run_bass_kernel_spmd(nc, in_maps, core_ids=list(range(8)))
    return _assemble(res.results)



# revision 84
# speedup vs baseline: 1.7619x; 1.0538x over previous
"""Trainium2 Bass kernel for a dense transformer block (B=2, T=2048, C=1024,
H=16, DFF=4096), distributed over 8 NeuronCores.

Sharding: 2 batch groups x 4-way query-block sharding. Core c handles batch
g=c//4 and query blocks {j, 7-j} (j=c%4) of 8 blocks of 256 rows. K/V are
computed per-core for the full batch (replicated; no collectives). Causality
is exploited statically: key-chunks 0-7 are needed by both query blocks
(masked only on block-0's columns), chunks 8-15 only by the late block.
The data-dependent causal boundary is applied with per-core 0/1 masks so one
NEFF serves all 8 cores (SPMD).

Activations are kept feature-major ("xT") so every matmul chains without
transposes; layernorm runs row-major with PE transposes between domains.
Softmax denominators ride the AV matmul as an extra ones-column of V.
"""
import numpy as np
import ml_dtypes

import concourse.bass as bass
import concourse.mybir as mybir
import concourse.tile as tile
from concourse.vector_clock import ScopedClock
from concourse.bass_utils import run_bass_kernel_spmd
from concourse.masks import make_identity

bf16 = ml_dtypes.bfloat16
fp8 = ml_dtypes.float8_e4m3
f32 = mybir.dt.float32
bt16 = mybir.dt.bfloat16
f8 = mybir.dt.float8e4
AF = mybir.ActivationFunctionType
OP = mybir.AluOpType

B, T, C, H, DH, DFF = 2, 2048, 1024, 16, 64, 4096
P = 128
QB = 256            # rows per query block
R = 512             # own query rows per core
RT = T + R          # ln1 rows per core (full batch + own q rows)
CC = C // P         # 8 feature chunks
MM = DFF // P       # 32 ffn chunks
EPS = 1e-5


# ---------------------------------------------------------------------------
# The walrus build in this container rejects instructions with >1 sync wait.
# Tile's sem assignment can emit several on one instruction; split the excess
# onto same-engine NoOps placed immediately before.
def _patched_drain_and_barrier(self, tick_clock, wait_clock):
    nc = self.nc
    probe = nc.sync.nop(nofuse=True, hint="tail_wait_probe")
    wait_clock.add_sem_waits(probe.ins, ScopedClock({None: tick_clock.global_clock}))
    si = probe.ins.sync_info
    waits = list(si.on_wait) if si is not None else []
    if si is not None:
        si.on_wait = waits[:1]
    for w in waits[1:]:
        n2 = nc.sync.nop(nofuse=True, hint="tail_wait_split")
        n2.ins.sync_info = mybir.SyncInfo(on_wait=[w], on_update=[])
    nc.sync.drain()
    nc.all_engine_barrier()
    assert self.sems is not None
    popped = nc._tile_sem_poison_stack.pop()
    assert popped is self._sem_poison
    nc.clear_and_free_semaphores(list(self.sems.allocated().values()))
    nc.all_engine_barrier()


tile.TileContext._drain_and_barrier = _patched_drain_and_barrier

_MAX_WAITS = 1
_split_counter = [0]


def _split_sync_waits(nc):
    for fn in nc.m.functions:
        for bb in fn.blocks:
            new_insts = []
            for inst in bb.instructions:
                si = getattr(inst, "sync_info", None)
                lim = _MAX_WAITS
                if si is not None and si.on_wait and len(si.on_wait) > lim:
                    waits = list(si.on_wait)
                    keep = waits[-lim:]
                    excess = waits[:-lim]
                    for i in range(0, len(excess), _MAX_WAITS):
                        _split_counter[0] += 1
                        nop = mybir.InstNoOp(
                            name=f"I-wsplit-{_split_counter[0]}", ins=[], outs=[])
                        nop.engine = inst.engine
                        nop.sync_info = mybir.SyncInfo(
                            on_wait=excess[i:i + _MAX_WAITS], on_update=[])
                        new_insts.append(nop)
                    si.on_wait = keep
                new_insts.append(inst)
            bb.instructions = new_insts
# ---------------------------------------------------------------------------


class Ctx:
    pass


def _layernorm_pre(g, xt, xb):
    """Row-major LN of xt [128, C] into the bf16 tile xb (DVE + tiny Act
    sqrt). bf16 output makes the downstream PE transposes 2x cheaper."""
    nc = g.nc
    st = g.stats.tile([P, 2, 6], f32, tag="bnst", name="bnst")
    xv = xt.rearrange("p (s d) -> p s d", s=2)
    for sg in range(2):
        nc.vector.bn_stats(out=st[:, sg, :], in_=xv[:, sg, :])
    mv = g.stats.tile([P, 2], f32, tag="bnmv", name="bnmv")
    nc.vector.bn_aggr(out=mv[:], in_=st[:])
    sq = g.stats.tile([P, 1], f32, tag="bnsq", name="bnsq")
    nc.scalar.activation(out=sq[:], in_=mv[:, 1:2], func=AF.Sqrt,
                         bias=g.eps_sb[:], scale=float(C) / (C - 1))
    rstd = g.stats.tile([P, 1], f32, tag="bnrstd", name="bnrstd")
    nc.vector.reciprocal(rstd[:], sq[:])
    with nc.allow_low_precision(reason="normalized x in bf16"):
        nc.vector.tensor_scalar(out=xb[:], in0=xt[:], scalar1=mv[:, 0:1],
                                scalar2=rstd[:], op0=OP.subtract, op1=OP.mult)


def _layernorm_post(g, xb, out_writes):
    """Transpose normalized bf16 xb per feature chunk; out_writes(c, psum)."""
    nc = g.nc
    for c in range(CC):
        pt = g.psb.tile([P, 512], bt16, tag="psb", name="ps_t")
        nc.tensor.transpose(pt[:P, :P], xb[:, c * P:(c + 1) * P], g.identb[:])
        out_writes(c, pt[:P, :P])


def _layernorm_tile(g, xt, xb, out_writes):
    _layernorm_pre(g, xt, xb)
    _layernorm_post(g, xb, out_writes)


def _ln1_pre(g, rt):
    """DMA + row-major LN of tile rt; returns the normalized bf16 tile."""
    nc = g.nc
    xt = g.xio.tile([P, C], f32, tag="xin", name="xin")
    nc.sync.dma_start(xt[:], g.xc[rt * P:(rt + 1) * P, :])
    xb = g.xio.tile([P, C], bt16, tag="xbn", name="xbn")
    _layernorm_pre(g, xt, xb)
    return xb


def _ln1_post(g, x1T, xt, rt):
    nc = g.nc
    rb, r0 = rt // 4, (rt % 4) * P

    def wr1(c, pt):
        nc.scalar.activation(
            out=x1T[rb][:, c, r0:r0 + P], in_=pt, func=AF.Identity,
            bias=g.be1s[:, c:c + 1], scale=g.g1s[:, c:c + 1])
        if rt >= T // P:
            q0 = (rt - T // P) * P
            nc.scalar.activation(
                out=g.x1f[c][:, q0:q0 + P], in_=pt, func=AF.Identity,
                bias=g.be1s[:, c:c + 1], scale=g.g1s[:, c:c + 1])
    _layernorm_post(g, xt, wr1)


def _wk_fetch(g, m):
    wkm = g.wstr.tile([P, CC, P], bt16, tag="wstr", name="wkm")
    g.nc.gpsimd.dma_start(wkm[:, :, :], g.wk[m])
    return wkm


def _kproj_chunk(g, x1T, rb, m, wkm=None):
    """K^T projection of row-block rb, output chunk m."""
    nc = g.nc
    if wkm is None:
        wkm = _wk_fetch(g, m)
    pk = g.ps.tile([P, 512], f32, tag="ps", name="ps_k")
    for c in range(CC):
        nc.tensor.matmul(pk[:], wkm[:, c, :], x1T[rb][:, c, :],
                         start=(c == 0), stop=(c == CC - 1))
    nc.vector.tensor_scalar(
        out=g.kT[m][rb][:, :], in0=pk[:],
        scalar1=g.sb_vec["bk"][:, m:m + 1], scalar2=None, op0=OP.add)


def _vproj_kt(g, x1T, wvs, kt):
    """V projection (row-major, both 512-col halves) for key-tile kt.
    Writes vv[kt] scaled by the per-core late-relevance bit svl[kt], and (for
    kt < 8) vvE[kt] scaled by the early-relevance bit sve[kt]."""
    nc = g.nc
    rb, r0 = kt // 4, (kt % 4) * P
    for half in range(2):
        pv = g.ps.tile([P, 512], f32, tag="ps", name="ps_v")
        for c in range(CC):
            nc.tensor.matmul(pv[:], x1T[rb][:, c, r0:r0 + P],
                             wvs[:, c, half * 512:(half + 1) * 512],
                             start=(c == 0), stop=(c == CC - 1))
        pvh = pv.rearrange("p (h d) -> p h d", h=8)
        nc.vector.tensor_scalar(
            out=g.vv[kt][:, half * 8:(half + 1) * 8, 0:DH], in0=pvh,
            scalar1=g.sb_vec["svl"][:, kt:kt + 1], scalar2=None, op0=OP.mult)
        if kt < 8:
            # second (early-block) copy on the Act engine to offload DVE
            nc.scalar.activation(
                out=g.vvE[kt][:, half * 8:(half + 1) * 8, 0:DH], in_=pvh,
                func=AF.Identity, scale=g.sb_vec["sve"][:, kt:kt + 1])


def _phase_a(g):
    """LN1 + transpose + Q/K/V projections, software-pipelined at tile
    granularity: LN stats (DVE) run ahead; each LN transpose batch is
    staggered between projection matmul chunks of the previous row-block so
    the in-order PE stream never waits long."""
    nc, tc = g.nc, g.tc
    with tc.tile_pool(name="x1p", bufs=1) as x1p, \
         tc.tile_pool(name="xio", bufs=3) as xio, \
         tc.tile_pool(name="wvp", bufs=1) as wvp:
        g.xio = xio
        # x1T split per 512-row block (rb 0-3 = batch, rb 4 = own q rows)
        x1T = [x1p.tile([P, CC, 512], bt16, tag=f"x1T{rb}", name=f"x1T{rb}")
               for rb in range(5)]
        wvs = wvp.tile([P, CC, C], bt16, tag="wvs", name="wvs")
        # Prologue: LN of row-block 0 (no projections to overlap with yet)
        xts = [_ln1_pre(g, rt) for rt in range(3)]
        _ln1_post(g, x1T, xts[0], 0)
        xts.append(_ln1_pre(g, 3))
        _ln1_post(g, x1T, xts[1], 1)
        _ln1_post(g, x1T, xts[2], 2)
        _ln1_post(g, x1T, xts[3], 3)

        # ones columns (softmax denominator), scaled by per-slot relevance
        for kt in range(T // P):
            nc.vector.memset(g.vv[kt][:, :, DH:DH + 1], 1.0)
            nc.vector.tensor_scalar(
                out=g.vv[kt][:, :, DH:DH + 1], in0=g.vv[kt][:, :, DH:DH + 1],
                scalar1=g.sb_vec["svl"][:, kt:kt + 1], scalar2=None,
                op0=OP.mult)
        for kt in range(8):
            nc.vector.memset(g.vvE[kt][:, :, DH:DH + 1], 1.0)
            nc.vector.tensor_scalar(
                out=g.vvE[kt][:, :, DH:DH + 1],
                in0=g.vvE[kt][:, :, DH:DH + 1],
                scalar1=g.sb_vec["sve"][:, kt:kt + 1], scalar2=None,
                op0=OP.mult)

        # Steady state: projections of rb overlap LN of rb+1.
        # PE emission order per rb: K(m0) T(t0) K(m1) T(t1) K(m2) T(t2)
        # K(m3) T(t3) K(m4..7) V(kt*4); LN-pre(t_i) is emitted just before
        # K(m_i) so DVE stats run one matmul-chunk ahead of the transpose.
        # The first two K-weight chunks of rb+1 are prefetched before the
        # V block so the next iteration's PE stream starts without a DMA
        # stall; wvs (V weights) is fetched late enough not to block the
        # startup x-tile loads.
        wk_pre = [_wk_fetch(g, 0), _wk_fetch(g, 1)]
        for rb in range(4):
            for m in range(CC):
                if rb == 0 and m in (3, 5):
                    h0 = (m - 3) // 2 * 4
                    nc.gpsimd.dma_start(wvs[:, h0:h0 + 4, :],
                                        g.wv[:, h0:h0 + 4, :])
                if m < 4:
                    xt = _ln1_pre(g, (rb + 1) * 4 + m)
                _kproj_chunk(g, x1T, rb, m,
                             wkm=wk_pre[m] if m < 2 else None)
                if m < 4:
                    _ln1_post(g, x1T, xt, (rb + 1) * 4 + m)
            if rb < 3:
                wk_pre = [_wk_fetch(g, 0), _wk_fetch(g, 1)]
            for i in range(4):
                _vproj_kt(g, x1T, wvs, rb * 4 + i)

        # Q^T projection of the own-query rows (x1T[4])
        for m in range(CC):
            wqm = g.wstr.tile([P, CC, P], bt16, tag="wstr", name="wqm")
            nc.gpsimd.dma_start(wqm[:, :, :], g.wq[m])
            pq = g.ps.tile([P, 512], f32, tag="ps", name="ps_q")
            for c in range(CC):
                nc.tensor.matmul(pq[:], wqm[:, c, :], x1T[4][:, c, :],
                                 start=(c == 0), stop=(c == CC - 1))
            nc.vector.tensor_scalar(
                out=g.qT[m][:, :], in0=pq[:],
                scalar1=g.sb_vec["bq"][:, m:m + 1], scalar2=None, op0=OP.add)


def _phase_b(g):
    """Attention, both query blocks fused on the free dim (cols 0:256 = early
    block, 256:512 = late block). Key chunks arrive permuted per core so the
    causally-partial (diagonal) chunks sit at slots {0,1} (early) / {8,9}
    (late); all other slots need no elementwise mask because the V copies
    (vvE for the early half, vv for the late half) are zeroed per-slot when
    that chunk is causally irrelevant, nulling both numerator and the
    ones-column denominator. exp runs on 2-PSUM-bank batches; the softmax
    denominator reciprocal is broadcast on the idle GpSimd engine."""
    nc, tc = g.nc, g.tc
    with tc.tile_pool(name="mp", bufs=1) as mp, \
         tc.tile_pool(name="apl", bufs=2) as apl, \
         tc.tile_pool(name="rcp", bufs=2) as rcp, \
         tc.tile_pool(name="hp", bufs=1) as hp, \
         tc.tile_pool(name="wop", bufs=1) as wop, \
         tc.tile_pool(name="scp", bufs=3, space="PSUM") as scp, \
         tc.tile_pool(name="pvp", bufs=2, space="PSUM") as pvp:
        g.hcat = [hp.tile([P, 512], bt16, tag=f"hcat{c}", name=f"hcat{c}")
                  for c in range(CC)]
        mq = mp.tile([P, 4, QB], bt16, tag="mask", name="mask")
        nc.sync.dma_start(mq[:], g.masks)
        # prefetch the Wo weights during attention (DMA engine is idle here)
        wos = wop.tile([P, CC, C], bt16, tag="wos", name="wos")
        nc.gpsimd.dma_start(wos[:], g.wo)
        for pair in range(CC):
            for hl in range(2):
                h = 2 * pair + hl
                hs = slice(hl * DH, (hl + 1) * DH)
                aA = apl.tile([P, 8, 512], bt16, tag="aA", name="aA")
                aB = apl.tile([P, 8, QB], bt16, tag="aB", name="aB")
                # scores slots 0..7 (512 wide): 2 slots per 2-bank psum tile
                for t4 in range(4):
                    psc = scp.tile([P, 1024], f32, tag="sc", name="ps_s")
                    for k in range(2):
                        s = 2 * t4 + k
                        rb, k0 = s // 4, (s % 4) * P
                        nc.tensor.matmul(
                            psc[:, k * 512:(k + 1) * 512],
                            g.kT[pair][rb][hs, k0:k0 + P],
                            g.qT[pair][hs, 0:512], start=True, stop=True,
                            tile_position=(hl * DH, 0))
                    nc.scalar.activation(out=aA[:, 2 * t4:2 * t4 + 2, :],
                                         in_=psc[:], func=AF.Exp)
                # slots 8..15 (late block only, 256 wide): 4 per psum tile
                for t4 in range(2):
                    psc = scp.tile([P, 1024], f32, tag="sc", name="ps_sB")
                    for k in range(4):
                        s = 8 + 4 * t4 + k
                        rb, k0 = s // 4, (s % 4) * P
                        nc.tensor.matmul(
                            psc[:, k * QB:(k + 1) * QB],
                            g.kT[pair][rb][hs, k0:k0 + P],
                            g.qT[pair][hs, QB:512], start=True, stop=True,
                            tile_position=(hl * DH, 0))
                    nc.scalar.activation(out=aB[:, 4 * t4:4 * t4 + 4, :],
                                         in_=psc[:], func=AF.Exp)
                # elementwise causal masks only on the diagonal slots
                nc.vector.tensor_mul(aA[:, 0:2, 0:QB], aA[:, 0:2, 0:QB],
                                     mq[:, 0:2, :])
                nc.vector.tensor_mul(aB[:, 0:2, :], aB[:, 0:2, :],
                                     mq[:, 2:4, :])
                pav = pvp.tile([P, 512], f32, tag="pav", name="ps_av")
                for s in range(8):
                    nc.tensor.matmul(pav[:DH + 1, 0:QB], g.vvE[s][:, h, :],
                                     aA[:, s, 0:QB], start=(s == 0),
                                     stop=(s == 7))
                for s in range(8):
                    nc.tensor.matmul(pav[:DH + 1, QB:512], g.vv[s][:, h, :],
                                     aA[:, s, QB:512], start=(s == 0),
                                     stop=False)
                for s in range(8, 16):
                    nc.tensor.matmul(pav[:DH + 1, QB:512], g.vv[s][:, h, :],
                                     aB[:, s - 8, :], start=False,
                                     stop=(s == 15))
                den = g.stats.tile([1, 512], f32, tag="den", name="den")
                nc.vector.tensor_copy(den[:], pav[DH:DH + 1, :512])
                rr = g.stats.tile([1, 512], bt16, tag="rr", name="rr")
                with nc.allow_low_precision(reason="softmax denom in bf16"):
                    nc.vector.reciprocal(rr[:], den[:])
                prb = pvp.tile([P, 512], f32, tag="pav", name="ps_r")
                nc.tensor.matmul(prb[:DH, :512], g.ones64[:], rr[:],
                                 start=True, stop=True)
                rbc = rcp.tile([DH, 512], bt16, tag="rbc", name="rbc")
                with nc.allow_low_precision(reason="softmax denom in bf16"):
                    nc.vector.tensor_copy(rbc[:], prb[:DH, :512])
                nc.vector.tensor_mul(out=g.hcat[pair][hs, :],
                                     in0=pav[:DH, :512], in1=rbc[:])

        # Wo projection + residual (uses hcat/x1f/wos before they die)
        for m in range(CC):
            pa = pvp.tile([P, 512], f32, tag="pav", name="ps_o")
            for c in range(CC):
                nc.tensor.matmul(pa[:, :512], wos[:, c, m * P:(m + 1) * P],
                                 g.hcat[c][:, :], start=(c == 0),
                                 stop=(c == CC - 1))
            with nc.allow_low_precision(reason="x2 residual in bf16"):
                nc.vector.scalar_tensor_tensor(
                    out=g.x2T[m][:, :], in0=pa[:, :512],
                    scalar=g.sb_vec["bo"][:, m:m + 1], in1=g.x1f[m][:, :],
                    op0=OP.add, op1=OP.add)


def _phase_d(g):
    """LN2 + FFN (fp8 DoubleRow) + output.

    FFN weights are pre-scaled by 64 on the host so fp8e4 quantization stays
    in the normal range; the 1/64 rides the gelu's scale operand (FFN1) and
    the epilogue's scalar multiply (FFN2). b2 is pre-folded into the f32
    residual copy of ln2(x) (bias be2f = beta2 + b2)."""
    nc, tc = g.nc, g.tc
    FSC = 1.0 / 64
    x2T = g.x2T
    with tc.tile_pool(name="cp", bufs=3) as cp, \
         tc.tile_pool(name="psB", bufs=2, space="PSUM") as psB, \
         tc.tile_pool(name="x3p", bufs=1) as x3p:
        g.psb = psB
        # x3T8[kp]: ln2(x) in fp8, DoubleRow-interleaved over feature pairs;
        # x3L8: the x8-scaled fp8 quantization residual (split precision)
        x3T8 = [x3p.tile([P, 2, 512], f8, tag=f"x3T8{c}", name=f"x3T8{c}")
                for c in range(CC // 2)]
        x3L8 = [x3p.tile([P, 2, 512], f8, tag=f"x3L8{c}", name=f"x3L8{c}")
                for c in range(CC // 2)]
        x3Tf = [x3p.tile([P, 512], f32, tag=f"x3Tf{c}", name=f"x3Tf{c}")
                for c in range(CC)]

        def ln2_in(rt):
            x2r = cp.tile([P, C], f32, tag="x2r", name="x2r")
            for c in range(CC):
                pt = g.psb.tile([P, 512], bt16, tag="psb", name="ps_t2")
                nc.tensor.transpose(pt[:P, :P], x2T[c][:, rt * P:(rt + 1) * P],
                                    g.identb[:])
                nc.scalar.copy(out=x2r[:, c * P:(c + 1) * P], in_=pt[:P, :P])
            x2b = cp.tile([P, C], bt16, tag="x2b", name="x2b")
            _layernorm_pre(g, x2r, x2b)
            return x2b

        def ln2_out(x2r, rt):
            def wr2(c, pt):
                with nc.allow_low_precision(reason="ffn input quantize fp8"):
                    nc.scalar.activation(
                        out=x3T8[c // 2][:, c % 2, rt * P:(rt + 1) * P],
                        in_=pt, func=AF.Identity, bias=g.be2s[:, c:c + 1],
                        scale=g.g2s[:, c:c + 1])
                nc.scalar.activation(
                    out=x3Tf[c][:, rt * P:(rt + 1) * P], in_=pt,
                    func=AF.Identity, bias=g.sb_vec["be2f"][:, c:c + 1],
                    scale=g.g2s[:, c:c + 1])
            _layernorm_post(g, x2r, wr2)

        # software-pipelined: PE transposes of tile rt+1 overlap Act/DVE of rt
        x2rs = [ln2_in(0), ln2_in(1), ln2_in(2)]
        ln2_out(x2rs[0], 0)
        x2rs.append(ln2_in(3))
        ln2_out(x2rs[1], 1)
        ln2_out(x2rs[2], 2)
        ln2_out(x2rs[3], 3)

        # x3 fp8 residual: x3lo = fp8(8 * (x3 - b2 - dequant(x3hi)))
        for c in range(CC):
            dt_ = cp.tile([P, 512], bt16, tag="dres", name="dres")
            with nc.allow_low_precision(reason="fp8 split residual"):
                nc.vector.scalar_tensor_tensor(
                    out=dt_[:], in0=x3Tf[c][:, :],
                    scalar=g.sb_vec["b2"][:, c:c + 1],
                    in1=x3T8[c // 2][:, c % 2, :],
                    op0=OP.subtract, op1=OP.subtract)
                nc.vector.tensor_scalar(
                    out=x3L8[c // 2][:, c % 2, :], in0=dt_[:], scalar1=8.0,
                    scalar2=None, op0=OP.mult)

        # FFN in fp8 DoubleRow (256-row contraction per matmul); weights
        # stream through dedicated deep-prefetch pools so transfers overlap
        # the Act-bound LN2 stretch above.
        with tc.tile_pool(name="dp", bufs=1) as dp, \
             tc.tile_pool(name="w1p", bufs=10) as w1p, \
             tc.tile_pool(name="w2p", bufs=4) as w2p:
            h1p = [dp.tile([P, 2, 512], f8, tag=f"h1_{m}", name=f"h1_{m}")
                   for m in range(MM // 2)]
            for m in range(MM):
                w1m = w1p.tile([P, 3, 4, 2, P], f8, tag="w1m", name="w1m")
                nc.gpsimd.dma_start(w1m[:], g.w1[m])
                p1 = g.ps.tile([P, 512], f32, tag="ps", name="ps_f1")
                for kp in range(4):
                    nc.tensor.matmul(p1[:], w1m[:, 0, kp], x3T8[kp][:],
                                     perf_mode=mybir.MatmulPerfMode.DoubleRow,
                                     start=(kp == 0), stop=False)
                    nc.tensor.matmul(p1[:], w1m[:, 1, kp], x3T8[kp][:],
                                     perf_mode=mybir.MatmulPerfMode.DoubleRow,
                                     start=False, stop=False)
                    nc.tensor.matmul(p1[:], w1m[:, 2, kp], x3L8[kp][:],
                                     perf_mode=mybir.MatmulPerfMode.DoubleRow,
                                     start=False, stop=(kp == 3))
                with nc.allow_low_precision(reason="ffn hidden fp8"):
                    nc.scalar.activation(
                        out=h1p[m // 2][:, m % 2, :], in_=p1[:], func=AF.Gelu,
                        bias=g.sb_vec["b1"][:, m:m + 1], scale=FSC)
            for oc in range(CC):
                w2m = w2p.tile([P, 16, 2, P], f8, tag="w2m", name="w2m")
                nc.gpsimd.dma_start(w2m[:], g.w2[oc])
                p2 = g.ps.tile([P, 512], f32, tag="ps", name="ps_f2")
                for kp in range(16):
                    nc.tensor.matmul(p2[:], w2m[:, kp], h1p[kp][:],
                                     perf_mode=mybir.MatmulPerfMode.DoubleRow,
                                     start=(kp == 0), stop=(kp == 15))
                ot = cp.tile([P, R], f32, tag="otile", name="otile")
                nc.vector.scalar_tensor_tensor(
                    out=ot[:], in0=p2[:], scalar=FSC,
                    in1=x3Tf[oc][:, :], op0=OP.mult, op1=OP.add)
                nc.sync.dma_start(g.out[oc], ot[:])


def build_kernel():
    nc = bass.Bass("TRN2", target_bir_lowering=False, num_devices=8)
    g = Ctx()
    g.nc = nc

    # Weight layouts match the SBUF tile layout exactly so every DMA is
    # contiguous per partition (>=512B elements run at full DMA rate).
    g.xc = nc.dram_tensor("xc", [RT, C], f32, kind="ExternalInput").ap()
    g.wq = nc.dram_tensor("wq", [CC, P, CC, P], bt16, kind="ExternalInput").ap()
    g.wk = nc.dram_tensor("wk", [CC, P, CC, P], bt16, kind="ExternalInput").ap()
    g.wv = nc.dram_tensor("wv", [P, CC, C], bt16, kind="ExternalInput").ap()
    g.wo = nc.dram_tensor("wo", [P, CC, C], bt16, kind="ExternalInput").ap()
    # w1 packs three fp8 copies (hi*64, residual*64, hi*8) for the 3-term
    # split-precision FFN1: x3hi@W1hi + x3hi@W1lo + x3lo@W1hi8, all scale 64.
    g.w1 = nc.dram_tensor("w1", [MM, P, 3, 4, 2, P], f8,
                          kind="ExternalInput").ap()
    g.w2 = nc.dram_tensor("w2", [CC, P, 16, 2, P], f8,
                          kind="ExternalInput").ap()
    g.masks = nc.dram_tensor("masks", [P, 4, QB], bt16,
                             kind="ExternalInput").ap()
    # All per-feature vectors packed into one tensor (single DMA); svl/sve
    # are per-core per-slot causal-relevance bits for the V-zeroing scheme.
    VEC_SPECS = [("bq", CC), ("bk", CC), ("bo", CC), ("b1", MM),
                 ("b2", CC), ("g1", CC), ("be1", CC), ("g2", CC), ("be2", CC),
                 ("be2f", CC), ("svl", 16), ("sve", 8)]
    NVEC = sum(n for _, n in VEC_SPECS)
    g.vecs = nc.dram_tensor("vecs", [P, NVEC], f32, kind="ExternalInput").ap()
    g.out = nc.dram_tensor("out", [CC, P, R], f32, kind="ExternalOutput").ap()

    with tile.TileContext(nc) as tc:
        g.tc = tc
        with tc.tile_pool(name="setup", bufs=1) as setup, \
             tc.tile_pool(name="stats", bufs=4) as stats, \
             tc.tile_pool(name="wstr", bufs=4) as wstr, \
             tc.tile_pool(name="x1f_p", bufs=1) as x1f_p:
            g.stats, g.wstr = stats, wstr

            ident = setup.tile([P, P], f32, tag="ident", name="ident")
            make_identity(nc, ident[:])
            g.ident = ident
            identb = setup.tile([P, P], bt16, tag="identb", name="identb")
            with nc.allow_low_precision(reason="identity matrix cast"):
                nc.vector.tensor_copy(identb[:], ident[:])
            g.identb = identb
            g.ones64 = setup.tile([1, DH], bt16, tag="ones64", name="ones64")
            nc.vector.memset(g.ones64[:], 1.0)
            g.eps_sb = setup.tile([P, 1], f32, tag="eps", name="eps")
            nc.vector.memset(g.eps_sb[:], EPS)
            vt = setup.tile([P, NVEC], f32, tag="vec_all", name="vec_all")
            nc.sync.dma_start(vt[:], g.vecs)
            g.sb_vec = {}
            off = 0
            for nm, n in VEC_SPECS:
                g.sb_vec[nm] = vt[:, off:off + n]
                off += n
            g.g1s, g.be1s = g.sb_vec["g1"], g.sb_vec["be1"]
            g.g2s, g.be2s = g.sb_vec["g2"], g.sb_vec["be2"]

            g.x1f = [x1f_p.tile([P, 512], bt16, tag=f"x1f{c}", name=f"x1f{c}")
                     for c in range(CC)]

            with tc.tile_pool(name="x2p", bufs=1) as x2p:
                g.x2T = [x2p.tile([P, 512], bt16, tag=f"x2T{m}",
                                  name=f"x2T{m}") for m in range(CC)]
                with tc.tile_pool(name="kvp", bufs=1) as kvp:
                    g.kT = [[kvp.tile([P, 512], bt16, tag=f"kT{m}_{rb}",
                                      name=f"kT{m}_{rb}") for rb in range(4)]
                            for m in range(CC)]
                    g.vv = [kvp.tile([P, H, DH + 1], bt16, tag=f"vv{kt}",
                                     name=f"vv{kt}") for kt in range(T // P)]
                    g.vvE = [kvp.tile([P, H, DH + 1], bt16, tag=f"vvE{kt}",
                                      name=f"vvE{kt}") for kt in range(8)]
                    g.qT = [kvp.tile([P, 512], bt16, tag=f"qT{m}",
                                     name=f"qT{m}") for m in range(CC)]
                    with tc.tile_pool(name="psA", bufs=6,
                                      space="PSUM") as psA, \
                         tc.tile_pool(name="psAb", bufs=2,
                                      space="PSUM") as psAb:
                        g.ps, g.psb = psA, psAb
                        _phase_a(g)
                    _phase_b(g)
                with tc.tile_pool(name="psC", bufs=6, space="PSUM") as psC:
                    g.ps = psC
                    _phase_d(g)
    _split_sync_waits(nc)
    return nc


_NC_CACHE = None


def _get_nc():
    global _NC_CACHE
    if _NC_CACHE is None:
        _NC_CACHE = build_kernel()
    return _NC_CACHE


def _prep_shared(inputs):
    scale = DH ** -0.5
    Wq = np.asarray(inputs["Wq"], np.float32)
    Wk = np.asarray(inputs["Wk"], np.float32)
    Wv = np.asarray(inputs["Wv"], np.float32)
    Wo = np.asarray(inputs["Wo"], np.float32)
    W1 = np.asarray(inputs["W1"], np.float32)
    W2 = np.asarray(inputs["W2"], np.float32)
    bv_c = np.asarray(inputs["bv"], np.float32).reshape(C)

    def kt_layout(w):
        # [C(in), C(out)] -> [m, p, c, 128]: SBUF tile order for wkm/wqm
        cin, cout = w.shape
        return np.ascontiguousarray(
            w.reshape(cin // P, P, cout // P, P).transpose(2, 1, 0, 3)
        ).astype(bf16)

    def dr_layout(w, s=64.0):
        # [K, F] -> [m, p, kp, i, 128]: fp8 DoubleRow-interleaved, x`s` scaled
        k, fdim = w.shape
        return np.ascontiguousarray(
            (w * s).reshape(k // 256, 2, P, fdim // P, P)
            .transpose(3, 2, 0, 1, 4)).astype(fp8)

    def dr3_layout(w):
        # Three stacked fp8 DoubleRow copies: hi*64, residual*64, hi*8
        hi_mat = (w * 64.0).astype(fp8).astype(np.float32) / 64.0
        return np.ascontiguousarray(np.stack(
            [dr_layout(w, 64.0), dr_layout(w - hi_mat, 64.0),
             dr_layout(w, 8.0)], axis=2))

    def row_layout(w):
        # [C(in), F] -> [p, c, F]: SBUF tile order for wvs/wos
        cin = w.shape[0]
        return np.ascontiguousarray(
            w.reshape(cin // P, P, -1).transpose(1, 0, 2)).astype(bf16)

    def vec_layout(v):
        # [n*P] -> [P, n] (transposed into partition-major SBUF layout)
        n = v.size // P
        return np.ascontiguousarray(v.reshape(n, P).T).copy()

    shared = {
        "wq": kt_layout(Wq.transpose(1, 0, 2).reshape(C, C) * scale),
        "wk": kt_layout(Wk.transpose(1, 0, 2).reshape(C, C)),
        "wv": row_layout(Wv.transpose(1, 0, 2).reshape(C, C)),
        "wo": row_layout(Wo),
        "w1": dr3_layout(W1),
        "w2": dr_layout(W2),
        "vecs_base": np.concatenate([
            vec_layout(np.asarray(inputs["bq"], np.float32).reshape(C)
                       * scale),
            vec_layout(np.asarray(inputs["bk"], np.float32).reshape(C)),
            vec_layout(np.asarray(inputs["bo"], np.float32) + bv_c @ Wo),
            vec_layout(np.asarray(inputs["b1"], np.float32)),
            vec_layout(np.asarray(inputs["b2"], np.float32)),
            vec_layout(np.asarray(inputs["gamma1"], np.float32)),
            vec_layout(np.asarray(inputs["beta1"], np.float32)),
            vec_layout(np.asarray(inputs["gamma2"], np.float32)),
            vec_layout(np.asarray(inputs["beta2"], np.float32)),
            vec_layout(np.asarray(inputs["beta2"], np.float32)
                       + np.asarray(inputs["b2"], np.float32)),
        ], axis=1),
    }
    return shared


def _core_perm(j):
    """Slot -> key-chunk permutation: diagonal (causally partial) chunks of
    the early block at slots {0,1}, of the late block at slots {8,9}."""
    fixed = {0: 2 * j, 1: 2 * j + 1, 8: 14 - 2 * j, 9: 15 - 2 * j}
    rest = [c for c in range(16) if c not in fixed.values()]
    perm = []
    for s in range(16):
        perm.append(fixed[s] if s in fixed else rest.pop(0))
    return perm


def _core_masks(j, perm):
    """[128, 4, 256] bf16 0/1 masks for the diagonal slots: entries 0,1 mask
    slots 0,1 against the early block's columns; entries 2,3 mask slots 8,9
    against the late block's columns."""
    out = np.zeros((P, 4, QB), np.float32)
    key_p = np.arange(P)[:, None]
    col = np.arange(QB)[None, :]
    for i in range(2):
        out[:, i, :] = (perm[i] * P + key_p <= j * QB + col)
        out[:, 2 + i, :] = (perm[8 + i] * P + key_p <= (7 - j) * QB + col)
    return out.astype(bf16)


def _core_sv(j, perm):
    """Per-slot relevance bits. svl[s]: late half of slot s is causally live
    (diagonal late slots 8,9 use masks instead -> 1). sve[s]: early half of
    slot s is fully live (diagonal early slots 0,1 use masks -> 1)."""
    svl = np.zeros(16, np.float32)
    sve = np.zeros(8, np.float32)
    for s in range(16):
        if s in (8, 9) or perm[s] < 14 - 2 * j:
            svl[s] = 1.0
    for s in range(8):
        if s in (0, 1) or perm[s] < 2 * j:
            sve[s] = 1.0
    return svl, sve


def _make_in_maps(inputs):
    x = np.asarray(inputs["x"], np.float32)
    shared = _prep_shared(inputs)
    vecs_base = shared.pop("vecs_base")
    in_maps = []
    for c in range(8):
        gg, j = c // 4, c % 4
        perm = _core_perm(j)
        xb = x[gg]
        xbp = np.concatenate([xb[p * P:(p + 1) * P] for p in perm], 0)
        xq = np.concatenate([xb[j * QB:(j + 1) * QB],
                             xb[(7 - j) * QB:(8 - j) * QB]], 0)
        svl, sve = _core_sv(j, perm)
        m = dict(shared)
        m["xc"] = np.ascontiguousarray(np.concatenate([xbp, xq], 0))
        m["masks"] = _core_masks(j, perm)
        m["vecs"] = np.ascontiguousarray(np.concatenate([
            vecs_base,
            np.broadcast_to(svl[None, :], (P, 16)),
            np.broadcast_to(sve[None, :], (P, 8)),
        ], axis=1))
        in_maps.append(m)
    return in_maps


def _assemble(results):
    out = np.zeros((B, T, C), np.float32)
    for c in range(8):
        gg, j = c // 4, c % 4
        o = results[c]["out"].reshape(C, R).T  # [512, C] rows = 2 blocks
        out[gg, j * QB:(j + 1) * QB] = o[:QB]
        out[gg, (7 - j) * QB:(8 - j) * QB] = o[QB:]
    return out


def kernel(**inputs):
    in_maps = _make_in_maps(inputs)
    nc = _get_nc()
    res = run_bass_kernel_spmd(nc, in_maps, core_ids=list(range(8)))
    return _assemble(res.results)



# revision 97
# speedup vs baseline: 1.8685x; 1.0605x over previous
"""Trainium2 Bass kernel for a dense transformer block (B=2, T=2048, C=1024,
H=16, DFF=4096), distributed over 8 NeuronCores.

Sharding: 2 batch groups x 4-way query-block sharding. Core c handles batch
g=c//4 and query blocks {j, 7-j} (j=c%4) of 8 blocks of 256 rows. K/V are
computed per-core for the full batch (replicated; no collectives). Causality
is exploited statically: key-chunks 0-7 are needed by both query blocks
(masked only on block-0's columns), chunks 8-15 only by the late block.
The data-dependent causal boundary is applied with per-core 0/1 masks so one
NEFF serves all 8 cores (SPMD).

Activations are kept feature-major ("xT") so every matmul chains without
transposes; layernorm runs row-major with PE transposes between domains.
Softmax denominators ride the AV matmul as an extra ones-column of V.
"""
import numpy as np
import ml_dtypes

import concourse.bass as bass
import concourse.mybir as mybir
import concourse.tile as tile
from concourse.vector_clock import ScopedClock
from concourse.bass_utils import run_bass_kernel_spmd
from concourse.masks import make_identity

bf16 = ml_dtypes.bfloat16
fp8 = ml_dtypes.float8_e4m3
f32 = mybir.dt.float32
bt16 = mybir.dt.bfloat16
f8 = mybir.dt.float8e4
AF = mybir.ActivationFunctionType
OP = mybir.AluOpType

B, T, C, H, DH, DFF = 2, 2048, 1024, 16, 64, 4096
P = 128
QB = 256            # rows per query block
R = 512             # own query rows per core
RT = T + R          # ln1 rows per core (full batch + own q rows)
CC = C // P         # 8 feature chunks
MM = DFF // P       # 32 ffn chunks
EPS = 1e-5


# ---------------------------------------------------------------------------
# The walrus build in this container rejects instructions with >1 sync wait.
# Tile's sem assignment can emit several on one instruction; split the excess
# onto same-engine NoOps placed immediately before.
def _patched_drain_and_barrier(self, tick_clock, wait_clock):
    nc = self.nc
    probe = nc.sync.nop(nofuse=True, hint="tail_wait_probe")
    wait_clock.add_sem_waits(probe.ins, ScopedClock({None: tick_clock.global_clock}))
    si = probe.ins.sync_info
    waits = list(si.on_wait) if si is not None else []
    if si is not None:
        si.on_wait = waits[:1]
    for w in waits[1:]:
        n2 = nc.sync.nop(nofuse=True, hint="tail_wait_split")
        n2.ins.sync_info = mybir.SyncInfo(on_wait=[w], on_update=[])
    nc.sync.drain()
    nc.all_engine_barrier()
    assert self.sems is not None
    popped = nc._tile_sem_poison_stack.pop()
    assert popped is self._sem_poison
    nc.clear_and_free_semaphores(list(self.sems.allocated().values()))
    nc.all_engine_barrier()


tile.TileContext._drain_and_barrier = _patched_drain_and_barrier

_MAX_WAITS = 1
_split_counter = [0]


def _split_sync_waits(nc):
    for fn in nc.m.functions:
        for bb in fn.blocks:
            new_insts = []
            for inst in bb.instructions:
                si = getattr(inst, "sync_info", None)
                lim = _MAX_WAITS
                if si is not None and si.on_wait and len(si.on_wait) > lim:
                    waits = list(si.on_wait)
                    keep = waits[-lim:]
                    excess = waits[:-lim]
                    for i in range(0, len(excess), _MAX_WAITS):
                        _split_counter[0] += 1
                        nop = mybir.InstNoOp(
                            name=f"I-wsplit-{_split_counter[0]}", ins=[], outs=[])
                        nop.engine = inst.engine
                        nop.sync_info = mybir.SyncInfo(
                            on_wait=excess[i:i + _MAX_WAITS], on_update=[])
                        new_insts.append(nop)
                    si.on_wait = keep
                new_insts.append(inst)
            bb.instructions = new_insts
# ---------------------------------------------------------------------------


class Ctx:
    pass


def _layernorm_pre(g, xt, xb):
    """Row-major LN of xt [128, C] into the bf16 tile xb (DVE + tiny Act
    sqrt). bf16 output makes the downstream PE transposes 2x cheaper."""
    nc = g.nc
    st = g.stats.tile([P, 2, 6], f32, tag="bnst", name="bnst")
    xv = xt.rearrange("p (s d) -> p s d", s=2)
    for sg in range(2):
        nc.vector.bn_stats(out=st[:, sg, :], in_=xv[:, sg, :])
    mv = g.stats.tile([P, 2], f32, tag="bnmv", name="bnmv")
    nc.vector.bn_aggr(out=mv[:], in_=st[:])
    sq = g.stats.tile([P, 1], f32, tag="bnsq", name="bnsq")
    nc.scalar.activation(out=sq[:], in_=mv[:, 1:2], func=AF.Sqrt,
                         bias=g.eps_sb[:], scale=float(C) / (C - 1))
    rstd = g.stats.tile([P, 1], f32, tag="bnrstd", name="bnrstd")
    nc.vector.reciprocal(rstd[:], sq[:])
    with nc.allow_low_precision(reason="normalized x in bf16"):
        nc.vector.tensor_scalar(out=xb[:], in0=xt[:], scalar1=mv[:, 0:1],
                                scalar2=rstd[:], op0=OP.subtract, op1=OP.mult)


def _layernorm_post(g, xb, out_writes):
    """Transpose normalized bf16 xb per feature chunk; out_writes(c, psum)."""
    nc = g.nc
    for c in range(CC):
        pt = g.psb.tile([P, 512], bt16, tag="psb", name="ps_t")
        nc.tensor.transpose(pt[:P, :P], xb[:, c * P:(c + 1) * P], g.identb[:])
        out_writes(c, pt[:P, :P])


def _layernorm_tile(g, xt, xb, out_writes):
    _layernorm_pre(g, xt, xb)
    _layernorm_post(g, xb, out_writes)


def _ln1_pre(g, rt):
    """DMA + row-major LN of tile rt; returns the normalized bf16 tile."""
    nc = g.nc
    xt = g.xio.tile([P, C], bt16, tag="xin", name="xin")
    nc.sync.dma_start(xt[:], g.xc[rt * P:(rt + 1) * P, :])
    xb = g.xio.tile([P, C], bt16, tag="xbn", name="xbn")
    _layernorm_pre(g, xt, xb)
    return xb


def _ln1_post(g, x1T, xt, rt):
    nc = g.nc
    rb, r0 = rt // 4, (rt % 4) * P

    def wr1(c, pt):
        nc.scalar.activation(
            out=x1T[rb][:, c, r0:r0 + P], in_=pt, func=AF.Identity,
            bias=g.be1s[:, c:c + 1], scale=g.g1s[:, c:c + 1])
        if rt >= T // P:
            q0 = (rt - T // P) * P
            nc.scalar.activation(
                out=g.x1f[c][:, q0:q0 + P], in_=pt, func=AF.Identity,
                bias=g.be1s[:, c:c + 1], scale=g.g1s[:, c:c + 1])
        else:
            # fp8 copy of ln1(x) feeding the DoubleRow V projection
            with nc.allow_low_precision(reason="x1 fp8 for V matmul"):
                nc.scalar.activation(
                    out=g.x1T8[rb][:, c // 2, c % 2, r0:r0 + P], in_=pt,
                    func=AF.Identity, bias=g.be1s[:, c:c + 1],
                    scale=g.g1s[:, c:c + 1])
    _layernorm_post(g, xt, wr1)


def _wk_fetch(g, m):
    wkm = g.wstr.tile([P, CC, P], bt16, tag="wstr", name="wkm")
    g.nc.gpsimd.dma_start(wkm[:, :, :], g.wk[m])
    return wkm


def _kproj_chunk(g, x1T, rb, m, wkm=None):
    """K^T projection of row-block rb, output chunk m."""
    nc = g.nc
    if wkm is None:
        wkm = _wk_fetch(g, m)
    pk = g.ps.tile([P, 512], f32, tag="ps", name="ps_k")
    for c in range(CC):
        nc.tensor.matmul(pk[:], wkm[:, c, :], x1T[rb][:, c, :],
                         start=(c == 0), stop=(c == CC - 1))
    nc.vector.tensor_scalar(
        out=g.kT[m][rb][:, :], in0=pk[:],
        scalar1=g.sb_vec["bk"][:, m:m + 1], scalar2=None, op0=OP.add)


def _vproj_kt(g, x1T, wvs, kt):
    """V projection (fp8 DoubleRow, both 512-col halves) for key-tile kt.
    Writes vv[kt] scaled by the per-core late-relevance bit svl[kt] (which
    also carries the 1/64 fp8 weight descale - it cancels in the softmax
    division), and (for kt < 8) vvE[kt] scaled by sve[kt]."""
    nc = g.nc
    rb, r0 = kt // 4, (kt % 4) * P
    for half in range(2):
        pv = g.ps.tile([P, 512], f32, tag="ps", name="ps_v")
        for kp in range(4):
            nc.tensor.matmul(pv[:], g.x1T8[rb][:, kp, :, r0:r0 + P],
                             wvs[:, kp, :, half * 512:(half + 1) * 512],
                             perf_mode=mybir.MatmulPerfMode.DoubleRow,
                             start=(kp == 0), stop=(kp == 3))
        pvh = pv.rearrange("p (h d) -> p h d", h=8)
        nc.vector.tensor_scalar(
            out=g.vv[kt][:, half * 8:(half + 1) * 8, 0:DH], in0=pvh,
            scalar1=g.sb_vec["sv64l"][:, kt:kt + 1], scalar2=None,
            op0=OP.mult)
        if kt < 8:
            # second (early-block) copy on the Act engine to offload DVE
            nc.scalar.activation(
                out=g.vvE[kt][:, half * 8:(half + 1) * 8, 0:DH], in_=pvh,
                func=AF.Identity, scale=g.sb_vec["sv64e"][:, kt:kt + 1])


def _phase_a(g):
    """LN1 + transpose + Q/K/V projections, software-pipelined at tile
    granularity: LN stats (DVE) run ahead; each LN transpose batch is
    staggered between projection matmul chunks of the previous row-block so
    the in-order PE stream never waits long."""
    nc, tc = g.nc, g.tc
    with tc.tile_pool(name="x1p", bufs=1) as x1p, \
         tc.tile_pool(name="xio", bufs=3) as xio, \
         tc.tile_pool(name="wvp", bufs=1) as wvp:
        g.xio = xio
        # x1T split per 512-row block (rb 0-3 = batch, rb 4 = own q rows)
        x1T = [x1p.tile([P, CC, 512], bt16, tag=f"x1T{rb}", name=f"x1T{rb}")
               for rb in range(5)]
        g.x1T8 = [x1p.tile([P, 4, 2, 512], f8, tag=f"x1T8{rb}",
                           name=f"x1T8{rb}") for rb in range(4)]
        wvs = wvp.tile([P, 4, 2, C], f8, tag="wvs", name="wvs")
        # Prologue: LN of row-block 0 (no projections to overlap with yet)
        xts = [_ln1_pre(g, rt) for rt in range(3)]
        _ln1_post(g, x1T, xts[0], 0)
        xts.append(_ln1_pre(g, 3))
        _ln1_post(g, x1T, xts[1], 1)
        _ln1_post(g, x1T, xts[2], 2)
        _ln1_post(g, x1T, xts[3], 3)

        # ones columns (softmax denominator), scaled by per-slot relevance
        for kt in range(T // P):
            nc.vector.memset(g.vv[kt][:, :, DH:DH + 1], 1.0)
            nc.vector.tensor_scalar(
                out=g.vv[kt][:, :, DH:DH + 1], in0=g.vv[kt][:, :, DH:DH + 1],
                scalar1=g.sb_vec["svl"][:, kt:kt + 1], scalar2=None,
                op0=OP.mult)
        for kt in range(8):
            nc.vector.memset(g.vvE[kt][:, :, DH:DH + 1], 1.0)
            nc.vector.tensor_scalar(
                out=g.vvE[kt][:, :, DH:DH + 1],
                in0=g.vvE[kt][:, :, DH:DH + 1],
                scalar1=g.sb_vec["sve"][:, kt:kt + 1], scalar2=None,
                op0=OP.mult)

        # Steady state: projections of rb overlap LN of rb+1.
        # PE emission order per rb: K(m0) T(t0) K(m1) T(t1) K(m2) T(t2)
        # K(m3) T(t3) K(m4..7) V(kt*4); LN-pre(t_i) is emitted just before
        # K(m_i) so DVE stats run one matmul-chunk ahead of the transpose.
        # The first two K-weight chunks of rb+1 are prefetched before the
        # V block so the next iteration's PE stream starts without a DMA
        # stall; wvs (V weights) is fetched late enough not to block the
        # startup x-tile loads.
        wk_pre = [_wk_fetch(g, 0), _wk_fetch(g, 1)]
        for rb in range(4):
            for m in range(CC):
                if rb == 0 and m in (3, 5):
                    h0 = (m - 3) // 2 * 512
                    nc.gpsimd.dma_start(wvs[:, :, :, h0:h0 + 512],
                                        g.wv[:, :, :, h0:h0 + 512])
                if m < 4:
                    xt = _ln1_pre(g, (rb + 1) * 4 + m)
                _kproj_chunk(g, x1T, rb, m,
                             wkm=wk_pre[m] if m < 2 else None)
                if m < 4:
                    _ln1_post(g, x1T, xt, (rb + 1) * 4 + m)
            if rb < 3:
                wk_pre = [_wk_fetch(g, 0), _wk_fetch(g, 1)]
            for i in range(4):
                _vproj_kt(g, x1T, wvs, rb * 4 + i)

        # Q^T projection of the own-query rows (x1T[4])
        for m in range(CC):
            wqm = g.wstr.tile([P, CC, P], bt16, tag="wstr", name="wqm")
            nc.gpsimd.dma_start(wqm[:, :, :], g.wq[m])
            pq = g.ps.tile([P, 512], f32, tag="ps", name="ps_q")
            for c in range(CC):
                nc.tensor.matmul(pq[:], wqm[:, c, :], x1T[4][:, c, :],
                                 start=(c == 0), stop=(c == CC - 1))
            nc.vector.tensor_scalar(
                out=g.qT[m][:, :], in0=pq[:],
                scalar1=g.sb_vec["bq"][:, m:m + 1], scalar2=None, op0=OP.add)


def _phase_b(g):
    """Attention, both query blocks fused on the free dim (cols 0:256 = early
    block, 256:512 = late block). Key chunks arrive permuted per core so the
    causally-partial (diagonal) chunks sit at slots {0,1} (early) / {8,9}
    (late); all other slots need no elementwise mask because the V copies
    (vvE for the early half, vv for the late half) are zeroed per-slot when
    that chunk is causally irrelevant, nulling both numerator and the
    ones-column denominator. exp runs on 2-PSUM-bank batches; the softmax
    denominator reciprocal is broadcast on the idle GpSimd engine."""
    nc, tc = g.nc, g.tc
    with tc.tile_pool(name="mp", bufs=1) as mp, \
         tc.tile_pool(name="apl", bufs=2) as apl, \
         tc.tile_pool(name="rcp", bufs=2) as rcp, \
         tc.tile_pool(name="hp", bufs=1) as hp, \
         tc.tile_pool(name="wop", bufs=1) as wop, \
         tc.tile_pool(name="scp", bufs=3, space="PSUM") as scp, \
         tc.tile_pool(name="pvp", bufs=2, space="PSUM") as pvp:
        g.hcat = [hp.tile([P, 512], bt16, tag=f"hcat{c}", name=f"hcat{c}")
                  for c in range(CC)]
        mq = mp.tile([P, 4, QB], bt16, tag="mask", name="mask")
        nc.sync.dma_start(mq[:], g.masks)
        # prefetch the Wo weights during attention (DMA engine is idle here)
        wos = wop.tile([P, CC, C], bt16, tag="wos", name="wos")
        nc.gpsimd.dma_start(wos[:], g.wo)
        for pair in range(CC):
            for hl in range(2):
                h = 2 * pair + hl
                hs = slice(hl * DH, (hl + 1) * DH)
                aA = apl.tile([P, 8, 512], bt16, tag="aA", name="aA")
                aB = apl.tile([P, 8, QB], bt16, tag="aB", name="aB")
                # scores slots 0..7 (512 wide): 2 slots per 2-bank psum tile
                for t4 in range(4):
                    psc = scp.tile([P, 1024], f32, tag="sc", name="ps_s")
                    for k in range(2):
                        s = 2 * t4 + k
                        rb, k0 = s // 4, (s % 4) * P
                        nc.tensor.matmul(
                            psc[:, k * 512:(k + 1) * 512],
                            g.kT[pair][rb][hs, k0:k0 + P],
                            g.qT[pair][hs, 0:512], start=True, stop=True,
                            tile_position=(hl * DH, 0))
                    nc.scalar.activation(out=aA[:, 2 * t4:2 * t4 + 2, :],
                                         in_=psc[:], func=AF.Exp)
                # slots 8..15 (late block only, 256 wide): 4 per psum tile
                for t4 in range(2):
                    psc = scp.tile([P, 1024], f32, tag="sc", name="ps_sB")
                    for k in range(4):
                        s = 8 + 4 * t4 + k
                        rb, k0 = s // 4, (s % 4) * P
                        nc.tensor.matmul(
                            psc[:, k * QB:(k + 1) * QB],
                            g.kT[pair][rb][hs, k0:k0 + P],
                            g.qT[pair][hs, QB:512], start=True, stop=True,
                            tile_position=(hl * DH, 0))
                    nc.scalar.activation(out=aB[:, 4 * t4:4 * t4 + 4, :],
                                         in_=psc[:], func=AF.Exp)
                # elementwise causal masks only on the diagonal slots
                nc.vector.tensor_mul(aA[:, 0:2, 0:QB], aA[:, 0:2, 0:QB],
                                     mq[:, 0:2, :])
                nc.vector.tensor_mul(aB[:, 0:2, :], aB[:, 0:2, :],
                                     mq[:, 2:4, :])
                pav = pvp.tile([P, 512], f32, tag="pav", name="ps_av")
                for s in range(8):
                    nc.tensor.matmul(pav[:DH + 1, 0:QB], g.vvE[s][:, h, :],
                                     aA[:, s, 0:QB], start=(s == 0),
                                     stop=(s == 7))
                for s in range(8):
                    nc.tensor.matmul(pav[:DH + 1, QB:512], g.vv[s][:, h, :],
                                     aA[:, s, QB:512], start=(s == 0),
                                     stop=False)
                for s in range(8, 16):
                    nc.tensor.matmul(pav[:DH + 1, QB:512], g.vv[s][:, h, :],
                                     aB[:, s - 8, :], start=False,
                                     stop=(s == 15))
                den = g.stats.tile([1, 512], f32, tag="den", name="den")
                nc.vector.tensor_copy(den[:], pav[DH:DH + 1, :512])
                rr = g.stats.tile([1, 512], bt16, tag="rr", name="rr")
                with nc.allow_low_precision(reason="softmax denom in bf16"):
                    nc.vector.reciprocal(rr[:], den[:])
                prb = pvp.tile([P, 512], f32, tag="pav", name="ps_r")
                nc.tensor.matmul(prb[:DH, :512], g.ones64[:], rr[:],
                                 start=True, stop=True)
                rbc = rcp.tile([DH, 512], bt16, tag="rbc", name="rbc")
                with nc.allow_low_precision(reason="softmax denom in bf16"):
                    nc.vector.tensor_copy(rbc[:], prb[:DH, :512])
                nc.vector.tensor_mul(out=g.hcat[pair][hs, :],
                                     in0=pav[:DH, :512], in1=rbc[:])

        # Wo projection + residual (uses hcat/x1f/wos before they die)
        for m in range(CC):
            pa = pvp.tile([P, 512], f32, tag="pav", name="ps_o")
            for c in range(CC):
                nc.tensor.matmul(pa[:, :512], wos[:, c, m * P:(m + 1) * P],
                                 g.hcat[c][:, :], start=(c == 0),
                                 stop=(c == CC - 1))
            with nc.allow_low_precision(reason="x2 residual in bf16"):
                nc.vector.scalar_tensor_tensor(
                    out=g.x2T[m][:, :], in0=pa[:, :512],
                    scalar=g.sb_vec["bo"][:, m:m + 1], in1=g.x1f[m][:, :],
                    op0=OP.add, op1=OP.add)


def _phase_d(g):
    """LN2 + FFN (fp8 DoubleRow) + output.

    FFN weights are pre-scaled by 64 on the host so fp8e4 quantization stays
    in the normal range; the 1/64 rides the gelu's scale operand (FFN1) and
    the epilogue's scalar multiply (FFN2). b2 is pre-folded into the f32
    residual copy of ln2(x) (bias be2f = beta2 + b2)."""
    nc, tc = g.nc, g.tc
    FSC = 1.0 / 64
    x2T = g.x2T
    with tc.tile_pool(name="cp", bufs=3) as cp, \
         tc.tile_pool(name="psB", bufs=2, space="PSUM") as psB, \
         tc.tile_pool(name="x3p", bufs=1) as x3p:
        g.psb = psB
        # x3T8[kp]: ln2(x) in fp8, DoubleRow-interleaved over feature pairs;
        # x3L8: the x8-scaled fp8 quantization residual (split precision)
        x3T8 = [x3p.tile([P, 2, 512], f8, tag=f"x3T8{c}", name=f"x3T8{c}")
                for c in range(CC // 2)]
        x3L8 = [x3p.tile([P, 2, 512], f8, tag=f"x3L8{c}", name=f"x3L8{c}")
                for c in range(CC // 2)]
        x3Tf = [x3p.tile([P, 512], f32, tag=f"x3Tf{c}", name=f"x3Tf{c}")
                for c in range(CC)]

        def ln2_in(rt):
            x2r = cp.tile([P, C], f32, tag="x2r", name="x2r")
            for c in range(CC):
                pt = g.psb.tile([P, 512], bt16, tag="psb", name="ps_t2")
                nc.tensor.transpose(pt[:P, :P], x2T[c][:, rt * P:(rt + 1) * P],
                                    g.identb[:])
                nc.scalar.copy(out=x2r[:, c * P:(c + 1) * P], in_=pt[:P, :P])
            x2b = cp.tile([P, C], bt16, tag="x2b", name="x2b")
            _layernorm_pre(g, x2r, x2b)
            return x2b

        def ln2_out(x2r, rt):
            def wr2(c, pt):
                with nc.allow_low_precision(reason="ffn input quantize fp8"):
                    nc.scalar.activation(
                        out=x3T8[c // 2][:, c % 2, rt * P:(rt + 1) * P],
                        in_=pt, func=AF.Identity, bias=g.be2s[:, c:c + 1],
                        scale=g.g2s[:, c:c + 1])
                nc.scalar.activation(
                    out=x3Tf[c][:, rt * P:(rt + 1) * P], in_=pt,
                    func=AF.Identity, bias=g.sb_vec["be2f"][:, c:c + 1],
                    scale=g.g2s[:, c:c + 1])
            _layernorm_post(g, x2r, wr2)

        # software-pipelined: PE transposes of tile rt+1 overlap Act/DVE of rt
        x2rs = [ln2_in(0), ln2_in(1), ln2_in(2)]
        ln2_out(x2rs[0], 0)
        x2rs.append(ln2_in(3))
        ln2_out(x2rs[1], 1)
        ln2_out(x2rs[2], 2)
        ln2_out(x2rs[3], 3)

        # x3 fp8 residual: x3lo = fp8(8 * (x3 - b2 - dequant(x3hi)))
        for c in range(CC):
            dt_ = cp.tile([P, 512], bt16, tag="dres", name="dres")
            with nc.allow_low_precision(reason="fp8 split residual"):
                nc.vector.scalar_tensor_tensor(
                    out=dt_[:], in0=x3Tf[c][:, :],
                    scalar=g.sb_vec["b2"][:, c:c + 1],
                    in1=x3T8[c // 2][:, c % 2, :],
                    op0=OP.subtract, op1=OP.subtract)
                nc.vector.tensor_scalar(
                    out=x3L8[c // 2][:, c % 2, :], in0=dt_[:], scalar1=8.0,
                    scalar2=None, op0=OP.mult)

        # FFN in fp8 DoubleRow (256-row contraction per matmul); weights
        # stream through dedicated deep-prefetch pools so transfers overlap
        # the Act-bound LN2 stretch above.
        with tc.tile_pool(name="dp", bufs=1) as dp, \
             tc.tile_pool(name="w1p", bufs=10) as w1p, \
             tc.tile_pool(name="w2p", bufs=4) as w2p:
            h1p = [dp.tile([P, 2, 512], f8, tag=f"h1_{m}", name=f"h1_{m}")
                   for m in range(MM // 2)]
            for m in range(MM):
                w1m = w1p.tile([P, 3, 4, 2, P], f8, tag="w1m", name="w1m")
                nc.gpsimd.dma_start(w1m[:], g.w1[m])
                p1 = g.ps.tile([P, 512], f32, tag="ps", name="ps_f1")
                for kp in range(4):
                    nc.tensor.matmul(p1[:], w1m[:, 0, kp], x3T8[kp][:],
                                     perf_mode=mybir.MatmulPerfMode.DoubleRow,
                                     start=(kp == 0), stop=False)
                    nc.tensor.matmul(p1[:], w1m[:, 1, kp], x3T8[kp][:],
                                     perf_mode=mybir.MatmulPerfMode.DoubleRow,
                                     start=False, stop=False)
                    nc.tensor.matmul(p1[:], w1m[:, 2, kp], x3L8[kp][:],
                                     perf_mode=mybir.MatmulPerfMode.DoubleRow,
                                     start=False, stop=(kp == 3))
                with nc.allow_low_precision(reason="ffn hidden fp8"):
                    nc.scalar.activation(
                        out=h1p[m // 2][:, m % 2, :], in_=p1[:], func=AF.Gelu,
                        bias=g.sb_vec["b1"][:, m:m + 1], scale=FSC)
            for oc in range(CC):
                w2m = w2p.tile([P, 16, 2, P], f8, tag="w2m", name="w2m")
                nc.gpsimd.dma_start(w2m[:], g.w2[oc])
                p2 = g.ps.tile([P, 512], f32, tag="ps", name="ps_f2")
                for kp in range(16):
                    nc.tensor.matmul(p2[:], w2m[:, kp], h1p[kp][:],
                                     perf_mode=mybir.MatmulPerfMode.DoubleRow,
                                     start=(kp == 0), stop=(kp == 15))
                ot = cp.tile([P, R], f32, tag="otile", name="otile")
                nc.vector.scalar_tensor_tensor(
                    out=ot[:], in0=p2[:], scalar=FSC,
                    in1=x3Tf[oc][:, :], op0=OP.mult, op1=OP.add)
                nc.sync.dma_start(g.out[oc], ot[:])


def build_kernel():
    nc = bass.Bass("TRN2", target_bir_lowering=False, num_devices=8)
    g = Ctx()
    g.nc = nc

    # Weight layouts match the SBUF tile layout exactly so every DMA is
    # contiguous per partition (>=512B elements run at full DMA rate).
    g.xc = nc.dram_tensor("xc", [RT, C], bt16, kind="ExternalInput").ap()
    g.wq = nc.dram_tensor("wq", [CC, P, CC, P], bt16, kind="ExternalInput").ap()
    g.wk = nc.dram_tensor("wk", [CC, P, CC, P], bt16, kind="ExternalInput").ap()
    g.wv = nc.dram_tensor("wv", [P, 4, 2, C], f8, kind="ExternalInput").ap()
    g.wo = nc.dram_tensor("wo", [P, CC, C], bt16, kind="ExternalInput").ap()
    # w1 packs three fp8 copies (hi*64, residual*64, hi*8) for the 3-term
    # split-precision FFN1: x3hi@W1hi + x3hi@W1lo + x3lo@W1hi8, all scale 64.
    g.w1 = nc.dram_tensor("w1", [MM, P, 3, 4, 2, P], f8,
                          kind="ExternalInput").ap()
    g.w2 = nc.dram_tensor("w2", [CC, P, 16, 2, P], f8,
                          kind="ExternalInput").ap()
    g.masks = nc.dram_tensor("masks", [P, 4, QB], bt16,
                             kind="ExternalInput").ap()
    # All per-feature vectors packed into one tensor (single DMA); svl/sve
    # are per-core per-slot causal-relevance bits for the V-zeroing scheme.
    VEC_SPECS = [("bq", CC), ("bk", CC), ("bo", CC), ("b1", MM),
                 ("b2", CC), ("g1", CC), ("be1", CC), ("g2", CC), ("be2", CC),
                 ("be2f", CC), ("svl", 16), ("sve", 8),
                 ("sv64l", 16), ("sv64e", 8)]
    NVEC = sum(n for _, n in VEC_SPECS)
    g.vecs = nc.dram_tensor("vecs", [P, NVEC], f32, kind="ExternalInput").ap()
    g.out = nc.dram_tensor("out", [CC, P, R], f32, kind="ExternalOutput").ap()

    with tile.TileContext(nc) as tc:
        g.tc = tc
        with tc.tile_pool(name="setup", bufs=1) as setup, \
             tc.tile_pool(name="stats", bufs=4) as stats, \
             tc.tile_pool(name="wstr", bufs=4) as wstr, \
             tc.tile_pool(name="x1f_p", bufs=1) as x1f_p:
            g.stats, g.wstr = stats, wstr

            ident = setup.tile([P, P], f32, tag="ident", name="ident")
            make_identity(nc, ident[:])
            g.ident = ident
            identb = setup.tile([P, P], bt16, tag="identb", name="identb")
            with nc.allow_low_precision(reason="identity matrix cast"):
                nc.vector.tensor_copy(identb[:], ident[:])
            g.identb = identb
            g.ones64 = setup.tile([1, DH], bt16, tag="ones64", name="ones64")
            nc.vector.memset(g.ones64[:], 1.0)
            g.eps_sb = setup.tile([P, 1], f32, tag="eps", name="eps")
            nc.vector.memset(g.eps_sb[:], EPS)
            vt = setup.tile([P, NVEC], f32, tag="vec_all", name="vec_all")
            nc.sync.dma_start(vt[:], g.vecs)
            g.sb_vec = {}
            off = 0
            for nm, n in VEC_SPECS:
                g.sb_vec[nm] = vt[:, off:off + n]
                off += n
            g.g1s, g.be1s = g.sb_vec["g1"], g.sb_vec["be1"]
            g.g2s, g.be2s = g.sb_vec["g2"], g.sb_vec["be2"]

            g.x1f = [x1f_p.tile([P, 512], bt16, tag=f"x1f{c}", name=f"x1f{c}")
                     for c in range(CC)]

            with tc.tile_pool(name="x2p", bufs=1) as x2p:
                g.x2T = [x2p.tile([P, 512], bt16, tag=f"x2T{m}",
                                  name=f"x2T{m}") for m in range(CC)]
                with tc.tile_pool(name="kvp", bufs=1) as kvp:
                    g.kT = [[kvp.tile([P, 512], bt16, tag=f"kT{m}_{rb}",
                                      name=f"kT{m}_{rb}") for rb in range(4)]
                            for m in range(CC)]
                    g.vv = [kvp.tile([P, H, DH + 1], bt16, tag=f"vv{kt}",
                                     name=f"vv{kt}") for kt in range(T // P)]
                    g.vvE = [kvp.tile([P, H, DH + 1], bt16, tag=f"vvE{kt}",
                                      name=f"vvE{kt}") for kt in range(8)]
                    g.qT = [kvp.tile([P, 512], bt16, tag=f"qT{m}",
                                     name=f"qT{m}") for m in range(CC)]
                    with tc.tile_pool(name="psA", bufs=6,
                                      space="PSUM") as psA, \
                         tc.tile_pool(name="psAb", bufs=2,
                                      space="PSUM") as psAb:
                        g.ps, g.psb = psA, psAb
                        _phase_a(g)
                    _phase_b(g)
                with tc.tile_pool(name="psC", bufs=6, space="PSUM") as psC:
                    g.ps = psC
                    _phase_d(g)
    _split_sync_waits(nc)
    return nc


_NC_CACHE = None


def _get_nc():
    global _NC_CACHE
    if _NC_CACHE is None:
        _NC_CACHE = build_kernel()
    return _NC_CACHE


def _prep_shared(inputs):
    scale = DH ** -0.5
    Wq = np.asarray(inputs["Wq"], np.float32)
    Wk = np.asarray(inputs["Wk"], np.float32)
    Wv = np.asarray(inputs["Wv"], np.float32)
    Wo = np.asarray(inputs["Wo"], np.float32)
    W1 = np.asarray(inputs["W1"], np.float32)
    W2 = np.asarray(inputs["W2"], np.float32)
    bv_c = np.asarray(inputs["bv"], np.float32).reshape(C)

    def kt_layout(w):
        # [C(in), C(out)] -> [m, p, c, 128]: SBUF tile order for wkm/wqm
        cin, cout = w.shape
        return np.ascontiguousarray(
            w.reshape(cin // P, P, cout // P, P).transpose(2, 1, 0, 3)
        ).astype(bf16)

    def dr_layout(w, s=64.0):
        # [K, F] -> [m, p, kp, i, 128]: fp8 DoubleRow-interleaved, x`s` scaled
        k, fdim = w.shape
        return np.ascontiguousarray(
            (w * s).reshape(k // 256, 2, P, fdim // P, P)
            .transpose(3, 2, 0, 1, 4)).astype(fp8)

    def dr3_layout(w):
        # Three stacked fp8 DoubleRow copies: hi*64, residual*64, hi*8
        hi_mat = (w * 64.0).astype(fp8).astype(np.float32) / 64.0
        return np.ascontiguousarray(np.stack(
            [dr_layout(w, 64.0), dr_layout(w - hi_mat, 64.0),
             dr_layout(w, 8.0)], axis=2))

    def row_layout(w):
        # [C(in), F] -> [p, c, F]: SBUF tile order for wvs/wos
        cin = w.shape[0]
        return np.ascontiguousarray(
            w.reshape(cin // P, P, -1).transpose(1, 0, 2)).astype(bf16)

    def vec_layout(v):
        # [n*P] -> [P, n] (transposed into partition-major SBUF layout)
        n = v.size // P
        return np.ascontiguousarray(v.reshape(n, P).T).copy()

    shared = {
        "wq": kt_layout(Wq.transpose(1, 0, 2).reshape(C, C) * scale),
        "wk": kt_layout(Wk.transpose(1, 0, 2).reshape(C, C)),
        # fp8 DoubleRow layout [p, kp, i, F], x64 scaled (descaled via svl)
        "wv": np.ascontiguousarray(
            (Wv.transpose(1, 0, 2).reshape(C, C) * 64.0)
            .reshape(4, 2, P, C).transpose(2, 0, 1, 3)).astype(fp8),
        "wo": row_layout(Wo),
        "w1": dr3_layout(W1),
        "w2": dr_layout(W2),
        "vecs_base": np.concatenate([
            vec_layout(np.asarray(inputs["bq"], np.float32).reshape(C)
                       * scale),
            vec_layout(np.asarray(inputs["bk"], np.float32).reshape(C)),
            vec_layout(np.asarray(inputs["bo"], np.float32) + bv_c @ Wo),
            vec_layout(np.asarray(inputs["b1"], np.float32)),
            vec_layout(np.asarray(inputs["b2"], np.float32)),
            vec_layout(np.asarray(inputs["gamma1"], np.float32)),
            vec_layout(np.asarray(inputs["beta1"], np.float32)),
            vec_layout(np.asarray(inputs["gamma2"], np.float32)),
            vec_layout(np.asarray(inputs["beta2"], np.float32)),
            vec_layout(np.asarray(inputs["beta2"], np.float32)
                       + np.asarray(inputs["b2"], np.float32)),
        ], axis=1),
    }
    return shared


def _core_perm(j):
    """Slot -> key-chunk permutation: diagonal (causally partial) chunks of
    the early block at slots {0,1}, of the late block at slots {8,9}."""
    fixed = {0: 2 * j, 1: 2 * j + 1, 8: 14 - 2 * j, 9: 15 - 2 * j}
    rest = [c for c in range(16) if c not in fixed.values()]
    perm = []
    for s in range(16):
        perm.append(fixed[s] if s in fixed else rest.pop(0))
    return perm


def _core_masks(j, perm):
    """[128, 4, 256] bf16 0/1 masks for the diagonal slots: entries 0,1 mask
    slots 0,1 against the early block's columns; entries 2,3 mask slots 8,9
    against the late block's columns."""
    out = np.zeros((P, 4, QB), np.float32)
    key_p = np.arange(P)[:, None]
    col = np.arange(QB)[None, :]
    for i in range(2):
        out[:, i, :] = (perm[i] * P + key_p <= j * QB + col)
        out[:, 2 + i, :] = (perm[8 + i] * P + key_p <= (7 - j) * QB + col)
    return out.astype(bf16)


def _core_sv(j, perm):
    """Per-slot relevance bits. svl[s]: late half of slot s is causally live
    (diagonal late slots 8,9 use masks instead -> 1). sve[s]: early half of
    slot s is fully live (diagonal early slots 0,1 use masks -> 1)."""
    svl = np.zeros(16, np.float32)
    sve = np.zeros(8, np.float32)
    for s in range(16):
        if s in (8, 9) or perm[s] < 14 - 2 * j:
            svl[s] = 1.0
    for s in range(8):
        if s in (0, 1) or perm[s] < 2 * j:
            sve[s] = 1.0
    return svl, sve


def _make_in_maps(inputs):
    x = np.asarray(inputs["x"], np.float32)
    shared = _prep_shared(inputs)
    vecs_base = shared.pop("vecs_base")
    in_maps = []
    for c in range(8):
        gg, j = c // 4, c % 4
        perm = _core_perm(j)
        xb = x[gg]
        xbp = np.concatenate([xb[p * P:(p + 1) * P] for p in perm], 0)
        xq = np.concatenate([xb[j * QB:(j + 1) * QB],
                             xb[(7 - j) * QB:(8 - j) * QB]], 0)
        svl, sve = _core_sv(j, perm)
        m = dict(shared)
        m["xc"] = np.ascontiguousarray(
            np.concatenate([xbp, xq], 0)).astype(bf16)
        m["masks"] = _core_masks(j, perm)
        m["vecs"] = np.ascontiguousarray(np.concatenate([
            vecs_base,
            np.broadcast_to(svl[None, :], (P, 16)),
            np.broadcast_to(sve[None, :], (P, 8)),
            np.broadcast_to(svl[None, :] / 64, (P, 16)),
            np.broadcast_to(sve[None, :] / 64, (P, 8)),
        ], axis=1))
        in_maps.append(m)
    return in_maps


def _assemble(results):
    out = np.zeros((B, T, C), np.float32)
    for c in range(8):
        gg, j = c // 4, c % 4
        o = results[c]["out"].reshape(C, R).T  # [512, C] rows = 2 blocks
        out[gg, j * QB:(j + 1) * QB] = o[:QB]
        out[gg, (7 - j) * QB:(8 - j) * QB] = o[QB:]
    return out


def kernel(**inputs):
    in_maps = _make_in_maps(inputs)
    nc = _get_nc()
    res = run_bass_kernel_spmd(nc, in_maps, core_ids=list(range(8)))
    return _assemble(res.results)



# revision 99
# speedup vs baseline: 1.9488x; 1.0430x over previous
"""Trainium2 Bass kernel for a dense transformer block (B=2, T=2048, C=1024,
H=16, DFF=4096), distributed over 8 NeuronCores.

Sharding: 2 batch groups x 4-way query-block sharding. Core c handles batch
g=c//4 and query blocks {j, 7-j} (j=c%4) of 8 blocks of 256 rows. K/V are
computed per-core for the full batch (replicated; no collectives). Causality
is exploited statically: key-chunks 0-7 are needed by both query blocks
(masked only on block-0's columns), chunks 8-15 only by the late block.
The data-dependent causal boundary is applied with per-core 0/1 masks so one
NEFF serves all 8 cores (SPMD).

Activations are kept feature-major ("xT") so every matmul chains without
transposes; layernorm runs row-major with PE transposes between domains.
Softmax denominators ride the AV matmul as an extra ones-column of V.
"""
import numpy as np
import ml_dtypes

import concourse.bass as bass
import concourse.mybir as mybir
import concourse.tile as tile
from concourse.vector_clock import ScopedClock
from concourse.bass_utils import run_bass_kernel_spmd
from concourse.masks import make_identity

bf16 = ml_dtypes.bfloat16
fp8 = ml_dtypes.float8_e4m3
f32 = mybir.dt.float32
bt16 = mybir.dt.bfloat16
f8 = mybir.dt.float8e4
AF = mybir.ActivationFunctionType
OP = mybir.AluOpType

B, T, C, H, DH, DFF = 2, 2048, 1024, 16, 64, 4096
P = 128
QB = 256            # rows per query block
R = 512             # own query rows per core
RT = T + R          # ln1 rows per core (full batch + own q rows)
CC = C // P         # 8 feature chunks
MM = DFF // P       # 32 ffn chunks
EPS = 1e-5


# ---------------------------------------------------------------------------
# The walrus build in this container rejects instructions with >1 sync wait.
# Tile's sem assignment can emit several on one instruction; split the excess
# onto same-engine NoOps placed immediately before.
def _patched_drain_and_barrier(self, tick_clock, wait_clock):
    nc = self.nc
    probe = nc.sync.nop(nofuse=True, hint="tail_wait_probe")
    wait_clock.add_sem_waits(probe.ins, ScopedClock({None: tick_clock.global_clock}))
    si = probe.ins.sync_info
    waits = list(si.on_wait) if si is not None else []
    if si is not None:
        si.on_wait = waits[:1]
    for w in waits[1:]:
        n2 = nc.sync.nop(nofuse=True, hint="tail_wait_split")
        n2.ins.sync_info = mybir.SyncInfo(on_wait=[w], on_update=[])
    nc.sync.drain()
    nc.all_engine_barrier()
    assert self.sems is not None
    popped = nc._tile_sem_poison_stack.pop()
    assert popped is self._sem_poison
    nc.clear_and_free_semaphores(list(self.sems.allocated().values()))
    nc.all_engine_barrier()


tile.TileContext._drain_and_barrier = _patched_drain_and_barrier

_MAX_WAITS = 1
_split_counter = [0]


def _split_sync_waits(nc):
    for fn in nc.m.functions:
        for bb in fn.blocks:
            new_insts = []
            for inst in bb.instructions:
                si = getattr(inst, "sync_info", None)
                lim = _MAX_WAITS
                if si is not None and si.on_wait and len(si.on_wait) > lim:
                    waits = list(si.on_wait)
                    keep = waits[-lim:]
                    excess = waits[:-lim]
                    for i in range(0, len(excess), _MAX_WAITS):
                        _split_counter[0] += 1
                        nop = mybir.InstNoOp(
                            name=f"I-wsplit-{_split_counter[0]}", ins=[], outs=[])
                        nop.engine = inst.engine
                        nop.sync_info = mybir.SyncInfo(
                            on_wait=excess[i:i + _MAX_WAITS], on_update=[])
                        new_insts.append(nop)
                    si.on_wait = keep
                new_insts.append(inst)
            bb.instructions = new_insts
# ---------------------------------------------------------------------------


class Ctx:
    pass


def _layernorm_pre(g, xt, xb):
    """Row-major LN of xt [128, C] into the bf16 tile xb (DVE + tiny Act
    sqrt). bf16 output makes the downstream PE transposes 2x cheaper."""
    nc = g.nc
    st = g.stats.tile([P, 2, 6], f32, tag="bnst", name="bnst")
    xv = xt.rearrange("p (s d) -> p s d", s=2)
    for sg in range(2):
        nc.vector.bn_stats(out=st[:, sg, :], in_=xv[:, sg, :])
    mv = g.stats.tile([P, 2], f32, tag="bnmv", name="bnmv")
    nc.vector.bn_aggr(out=mv[:], in_=st[:])
    sq = g.stats.tile([P, 1], f32, tag="bnsq", name="bnsq")
    nc.scalar.activation(out=sq[:], in_=mv[:, 1:2], func=AF.Sqrt,
                         bias=g.eps_sb[:], scale=float(C) / (C - 1))
    rstd = g.stats.tile([P, 1], f32, tag="bnrstd", name="bnrstd")
    nc.vector.reciprocal(rstd[:], sq[:])
    with nc.allow_low_precision(reason="normalized x in bf16"):
        nc.vector.tensor_scalar(out=xb[:], in0=xt[:], scalar1=mv[:, 0:1],
                                scalar2=rstd[:], op0=OP.subtract, op1=OP.mult)


def _layernorm_post(g, xb, out_writes):
    """Transpose normalized bf16 xb per feature chunk; out_writes(c, psum)."""
    nc = g.nc
    for c in range(CC):
        pt = g.psb.tile([P, 512], bt16, tag="psb", name="ps_t")
        nc.tensor.transpose(pt[:P, :P], xb[:, c * P:(c + 1) * P], g.identb[:])
        out_writes(c, pt[:P, :P])


def _layernorm_tile(g, xt, xb, out_writes):
    _layernorm_pre(g, xt, xb)
    _layernorm_post(g, xb, out_writes)


def _ln1_pre(g, rt):
    """DMA + row-major LN of tile rt; returns the normalized bf16 tile."""
    nc = g.nc
    xt = g.xio.tile([P, C], bt16, tag="xin", name="xin")
    nc.sync.dma_start(xt[:], g.xc[rt * P:(rt + 1) * P, :])
    xb = g.xio.tile([P, C], bt16, tag="xbn", name="xbn")
    _layernorm_pre(g, xt, xb)
    return xb


def _ln1_post(g, x1T, xt, rt):
    nc = g.nc
    rb, r0 = rt // 4, (rt % 4) * P

    def wr1(c, pt):
        nc.scalar.activation(
            out=x1T[rb][:, c, r0:r0 + P], in_=pt, func=AF.Identity,
            bias=g.be1s[:, c:c + 1], scale=g.g1s[:, c:c + 1])
        if rt >= T // P:
            q0 = (rt - T // P) * P
            nc.scalar.activation(
                out=g.x1f[c][:, q0:q0 + P], in_=pt, func=AF.Identity,
                bias=g.be1s[:, c:c + 1], scale=g.g1s[:, c:c + 1])
        elif c % 2 == 1:
            # fp8 copy of ln1(x) feeding the DoubleRow V projection; odd
            # chunks on Act here, even chunks bulk-copied on DVE later
            with nc.allow_low_precision(reason="x1 fp8 for V matmul"):
                nc.scalar.activation(
                    out=g.x1T8[rb][:, c // 2, c % 2, r0:r0 + P], in_=pt,
                    func=AF.Identity, bias=g.be1s[:, c:c + 1],
                    scale=g.g1s[:, c:c + 1])
    _layernorm_post(g, xt, wr1)


def _wk_fetch(g, m):
    wkm = g.wstr.tile([P, CC, P], bt16, tag="wstr", name="wkm")
    g.nc.gpsimd.dma_start(wkm[:, :, :], g.wk[m])
    return wkm


def _kproj_chunk(g, x1T, rb, m, wkm=None):
    """K^T projection of row-block rb, output chunk m."""
    nc = g.nc
    if wkm is None:
        wkm = _wk_fetch(g, m)
    pk = g.ps.tile([P, 512], f32, tag="ps", name="ps_k")
    for c in range(CC):
        nc.tensor.matmul(pk[:], wkm[:, c, :], x1T[rb][:, c, :],
                         start=(c == 0), stop=(c == CC - 1))
    nc.vector.tensor_scalar(
        out=g.kT[m][rb][:, :], in0=pk[:],
        scalar1=g.sb_vec["bk"][:, m:m + 1], scalar2=None, op0=OP.add)


def _vproj_kt(g, x1T, wvs, kt):
    """V projection (fp8 DoubleRow, both 512-col halves) for key-tile kt.
    Writes vv[kt] scaled by the per-core late-relevance bit svl[kt] (which
    also carries the 1/64 fp8 weight descale - it cancels in the softmax
    division), and (for kt < 8) vvE[kt] scaled by sve[kt]."""
    nc = g.nc
    rb, r0 = kt // 4, (kt % 4) * P
    for half in range(2):
        pv = g.ps.tile([P, 512], f32, tag="ps", name="ps_v")
        for kp in range(4):
            nc.tensor.matmul(pv[:], g.x1T8[rb][:, kp, :, r0:r0 + P],
                             wvs[:, kp, :, half * 512:(half + 1) * 512],
                             perf_mode=mybir.MatmulPerfMode.DoubleRow,
                             start=(kp == 0), stop=(kp == 3))
        pvh = pv.rearrange("p (h d) -> p h d", h=8)
        nc.vector.tensor_scalar(
            out=g.vv[kt][:, half * 8:(half + 1) * 8, 0:DH], in0=pvh,
            scalar1=g.sb_vec["sv64l"][:, kt:kt + 1], scalar2=None,
            op0=OP.mult)
        if kt < 8:
            # second (early-block) copy on the Act engine to offload DVE
            nc.scalar.activation(
                out=g.vvE[kt][:, half * 8:(half + 1) * 8, 0:DH], in_=pvh,
                func=AF.Identity, scale=g.sb_vec["sv64e"][:, kt:kt + 1])


def _phase_a(g):
    """LN1 + transpose + Q/K/V projections, software-pipelined at tile
    granularity: LN stats (DVE) run ahead; each LN transpose batch is
    staggered between projection matmul chunks of the previous row-block so
    the in-order PE stream never waits long."""
    nc, tc = g.nc, g.tc
    with tc.tile_pool(name="x1p", bufs=1) as x1p, \
         tc.tile_pool(name="xio", bufs=3) as xio, \
         tc.tile_pool(name="wvp", bufs=1) as wvp:
        g.xio = xio
        # x1T split per 512-row block (rb 0-3 = batch, rb 4 = own q rows)
        x1T = [x1p.tile([P, CC, 512], bt16, tag=f"x1T{rb}", name=f"x1T{rb}")
               for rb in range(5)]
        g.x1T8 = [x1p.tile([P, 4, 2, 512], f8, tag=f"x1T8{rb}",
                           name=f"x1T8{rb}") for rb in range(4)]
        wvs = wvp.tile([P, 4, 2, C], f8, tag="wvs", name="wvs")
        # Prologue: LN of row-block 0 (no projections to overlap with yet)
        xts = [_ln1_pre(g, rt) for rt in range(3)]
        _ln1_post(g, x1T, xts[0], 0)
        xts.append(_ln1_pre(g, 3))
        _ln1_post(g, x1T, xts[1], 1)
        _ln1_post(g, x1T, xts[2], 2)
        _ln1_post(g, x1T, xts[3], 3)

        # ones columns (softmax denominator), scaled by per-slot relevance
        for kt in range(T // P):
            nc.vector.memset(g.vv[kt][:, :, DH:DH + 1], 1.0)
            nc.vector.tensor_scalar(
                out=g.vv[kt][:, :, DH:DH + 1], in0=g.vv[kt][:, :, DH:DH + 1],
                scalar1=g.sb_vec["svl"][:, kt:kt + 1], scalar2=None,
                op0=OP.mult)
        for kt in range(8):
            nc.vector.memset(g.vvE[kt][:, :, DH:DH + 1], 1.0)
            nc.vector.tensor_scalar(
                out=g.vvE[kt][:, :, DH:DH + 1],
                in0=g.vvE[kt][:, :, DH:DH + 1],
                scalar1=g.sb_vec["sve"][:, kt:kt + 1], scalar2=None,
                op0=OP.mult)

        # Steady state: projections of rb overlap LN of rb+1.
        # PE emission order per rb: K(m0) T(t0) K(m1) T(t1) K(m2) T(t2)
        # K(m3) T(t3) K(m4..7) V(kt*4); LN-pre(t_i) is emitted just before
        # K(m_i) so DVE stats run one matmul-chunk ahead of the transpose.
        # The first two K-weight chunks of rb+1 are prefetched before the
        # V block so the next iteration's PE stream starts without a DMA
        # stall; wvs (V weights) is fetched late enough not to block the
        # startup x-tile loads.
        wk_pre = [_wk_fetch(g, 0), _wk_fetch(g, 1)]
        for rb in range(4):
            for m in range(CC):
                if rb == 0 and m in (3, 5):
                    h0 = (m - 3) // 2 * 512
                    nc.gpsimd.dma_start(wvs[:, :, :, h0:h0 + 512],
                                        g.wv[:, :, :, h0:h0 + 512])
                if m < 4:
                    xt = _ln1_pre(g, (rb + 1) * 4 + m)
                _kproj_chunk(g, x1T, rb, m,
                             wkm=wk_pre[m] if m < 2 else None)
                if m < 4:
                    _ln1_post(g, x1T, xt, (rb + 1) * 4 + m)
            if rb < 3:
                wk_pre = [_wk_fetch(g, 0), _wk_fetch(g, 1)]
            # even chunks of the fp8 x1 copy, bulk on DVE (Act does odd)
            for c in range(0, CC, 2):
                with nc.allow_low_precision(reason="x1 fp8 for V matmul"):
                    nc.vector.tensor_copy(
                        out=g.x1T8[rb][:, c // 2, 0, :],
                        in_=x1T[rb][:, c, :])
            for i in range(4):
                _vproj_kt(g, x1T, wvs, rb * 4 + i)

        # Q^T projection of the own-query rows (x1T[4])
        for m in range(CC):
            wqm = g.wstr.tile([P, CC, P], bt16, tag="wstr", name="wqm")
            nc.gpsimd.dma_start(wqm[:, :, :], g.wq[m])
            pq = g.ps.tile([P, 512], f32, tag="ps", name="ps_q")
            for c in range(CC):
                nc.tensor.matmul(pq[:], wqm[:, c, :], x1T[4][:, c, :],
                                 start=(c == 0), stop=(c == CC - 1))
            nc.vector.tensor_scalar(
                out=g.qT[m][:, :], in0=pq[:],
                scalar1=g.sb_vec["bq"][:, m:m + 1], scalar2=None, op0=OP.add)


def _phase_b(g):
    """Attention, both query blocks fused on the free dim (cols 0:256 = early
    block, 256:512 = late block). Key chunks arrive permuted per core so the
    causally-partial (diagonal) chunks sit at slots {0,1} (early) / {8,9}
    (late); all other slots need no elementwise mask because the V copies
    (vvE for the early half, vv for the late half) are zeroed per-slot when
    that chunk is causally irrelevant, nulling both numerator and the
    ones-column denominator. exp runs on 2-PSUM-bank batches; the softmax
    denominator reciprocal is broadcast on the idle GpSimd engine."""
    nc, tc = g.nc, g.tc
    with tc.tile_pool(name="mp", bufs=1) as mp, \
         tc.tile_pool(name="apl", bufs=2) as apl, \
         tc.tile_pool(name="rcp", bufs=2) as rcp, \
         tc.tile_pool(name="hp", bufs=1) as hp, \
         tc.tile_pool(name="wop", bufs=1) as wop, \
         tc.tile_pool(name="scp", bufs=3, space="PSUM") as scp, \
         tc.tile_pool(name="pvp", bufs=2, space="PSUM") as pvp:
        g.hcat = [hp.tile([P, 512], bt16, tag=f"hcat{c}", name=f"hcat{c}")
                  for c in range(CC)]
        mq = mp.tile([P, 4, QB], bt16, tag="mask", name="mask")
        nc.sync.dma_start(mq[:], g.masks)
        # prefetch the Wo weights during attention (DMA engine is idle here)
        wos = wop.tile([P, CC, C], bt16, tag="wos", name="wos")
        nc.gpsimd.dma_start(wos[:], g.wo)
        for pair in range(CC):
            for hl in range(2):
                h = 2 * pair + hl
                hs = slice(hl * DH, (hl + 1) * DH)
                aA = apl.tile([P, 8, 512], bt16, tag="aA", name="aA")
                aB = apl.tile([P, 8, QB], bt16, tag="aB", name="aB")
                # scores slots 0..7 (512 wide): 2 slots per 2-bank psum tile
                for t4 in range(4):
                    psc = scp.tile([P, 1024], f32, tag="sc", name="ps_s")
                    for k in range(2):
                        s = 2 * t4 + k
                        rb, k0 = s // 4, (s % 4) * P
                        nc.tensor.matmul(
                            psc[:, k * 512:(k + 1) * 512],
                            g.kT[pair][rb][hs, k0:k0 + P],
                            g.qT[pair][hs, 0:512], start=True, stop=True,
                            tile_position=(hl * DH, 0))
                    nc.scalar.activation(out=aA[:, 2 * t4:2 * t4 + 2, :],
                                         in_=psc[:], func=AF.Exp)
                # slots 8..15 (late block only, 256 wide): 4 per psum tile
                for t4 in range(2):
                    psc = scp.tile([P, 1024], f32, tag="sc", name="ps_sB")
                    for k in range(4):
                        s = 8 + 4 * t4 + k
                        rb, k0 = s // 4, (s % 4) * P
                        nc.tensor.matmul(
                            psc[:, k * QB:(k + 1) * QB],
                            g.kT[pair][rb][hs, k0:k0 + P],
                            g.qT[pair][hs, QB:512], start=True, stop=True,
                            tile_position=(hl * DH, 0))
                    nc.scalar.activation(out=aB[:, 4 * t4:4 * t4 + 4, :],
                                         in_=psc[:], func=AF.Exp)
                # elementwise causal masks only on the diagonal slots
                nc.vector.tensor_mul(aA[:, 0:2, 0:QB], aA[:, 0:2, 0:QB],
                                     mq[:, 0:2, :])
                nc.vector.tensor_mul(aB[:, 0:2, :], aB[:, 0:2, :],
                                     mq[:, 2:4, :])
                pav = pvp.tile([P, 512], f32, tag="pav", name="ps_av")
                for s in range(8):
                    nc.tensor.matmul(pav[:DH + 1, 0:QB], g.vvE[s][:, h, :],
                                     aA[:, s, 0:QB], start=(s == 0),
                                     stop=(s == 7))
                for s in range(8):
                    nc.tensor.matmul(pav[:DH + 1, QB:512], g.vv[s][:, h, :],
                                     aA[:, s, QB:512], start=(s == 0),
                                     stop=False)
                for s in range(8, 16):
                    nc.tensor.matmul(pav[:DH + 1, QB:512], g.vv[s][:, h, :],
                                     aB[:, s - 8, :], start=False,
                                     stop=(s == 15))
                den = g.stats.tile([1, 512], f32, tag="den", name="den")
                nc.vector.tensor_copy(den[:], pav[DH:DH + 1, :512])
                rr = g.stats.tile([1, 512], bt16, tag="rr", name="rr")
                with nc.allow_low_precision(reason="softmax denom in bf16"):
                    nc.vector.reciprocal(rr[:], den[:])
                prb = pvp.tile([P, 512], f32, tag="pav", name="ps_r")
                nc.tensor.matmul(prb[:DH, :512], g.ones64[:], rr[:],
                                 start=True, stop=True)
                rbc = rcp.tile([DH, 512], bt16, tag="rbc", name="rbc")
                with nc.allow_low_precision(reason="softmax denom in bf16"):
                    nc.vector.tensor_copy(rbc[:], prb[:DH, :512])
                nc.vector.tensor_mul(out=g.hcat[pair][hs, :],
                                     in0=pav[:DH, :512], in1=rbc[:])

        # Wo projection + residual (uses hcat/x1f/wos before they die)
        for m in range(CC):
            pa = pvp.tile([P, 512], f32, tag="pav", name="ps_o")
            for c in range(CC):
                nc.tensor.matmul(pa[:, :512], wos[:, c, m * P:(m + 1) * P],
                                 g.hcat[c][:, :], start=(c == 0),
                                 stop=(c == CC - 1))
            with nc.allow_low_precision(reason="x2 residual in bf16"):
                nc.vector.scalar_tensor_tensor(
                    out=g.x2T[m][:, :], in0=pa[:, :512],
                    scalar=g.sb_vec["bo"][:, m:m + 1], in1=g.x1f[m][:, :],
                    op0=OP.add, op1=OP.add)


def _phase_d(g):
    """LN2 + FFN (fp8 DoubleRow) + output.

    FFN weights are pre-scaled by 64 on the host so fp8e4 quantization stays
    in the normal range; the 1/64 rides the gelu's scale operand (FFN1) and
    the epilogue's scalar multiply (FFN2). b2 is pre-folded into the f32
    residual copy of ln2(x) (bias be2f = beta2 + b2)."""
    nc, tc = g.nc, g.tc
    FSC = 1.0 / 64
    x2T = g.x2T
    with tc.tile_pool(name="cp", bufs=3) as cp, \
         tc.tile_pool(name="psB", bufs=2, space="PSUM") as psB, \
         tc.tile_pool(name="x3p", bufs=1) as x3p:
        g.psb = psB
        # x3T8[kp]: ln2(x) in fp8, DoubleRow-interleaved over feature pairs;
        # x3L8: the x8-scaled fp8 quantization residual (split precision)
        x3T8 = [x3p.tile([P, 2, 512], f8, tag=f"x3T8{c}", name=f"x3T8{c}")
                for c in range(CC // 2)]
        x3L8 = [x3p.tile([P, 2, 512], f8, tag=f"x3L8{c}", name=f"x3L8{c}")
                for c in range(CC // 2)]
        x3Tf = [x3p.tile([P, 512], f32, tag=f"x3Tf{c}", name=f"x3Tf{c}")
                for c in range(CC)]

        def ln2_in(rt):
            x2r = cp.tile([P, C], f32, tag="x2r", name="x2r")
            for c in range(CC):
                pt = g.psb.tile([P, 512], bt16, tag="psb", name="ps_t2")
                nc.tensor.transpose(pt[:P, :P], x2T[c][:, rt * P:(rt + 1) * P],
                                    g.identb[:])
                nc.scalar.copy(out=x2r[:, c * P:(c + 1) * P], in_=pt[:P, :P])
            x2b = cp.tile([P, C], bt16, tag="x2b", name="x2b")
            _layernorm_pre(g, x2r, x2b)
            return x2b

        def ln2_out(x2r, rt):
            def wr2(c, pt):
                with nc.allow_low_precision(reason="ffn input quantize fp8"):
                    nc.scalar.activation(
                        out=x3T8[c // 2][:, c % 2, rt * P:(rt + 1) * P],
                        in_=pt, func=AF.Identity, bias=g.be2s[:, c:c + 1],
                        scale=g.g2s[:, c:c + 1])
                nc.scalar.activation(
                    out=x3Tf[c][:, rt * P:(rt + 1) * P], in_=pt,
                    func=AF.Identity, bias=g.sb_vec["be2f"][:, c:c + 1],
                    scale=g.g2s[:, c:c + 1])
            _layernorm_post(g, x2r, wr2)

        # software-pipelined: PE transposes of tile rt+1 overlap Act/DVE of rt
        x2rs = [ln2_in(0), ln2_in(1), ln2_in(2)]
        ln2_out(x2rs[0], 0)
        x2rs.append(ln2_in(3))
        ln2_out(x2rs[1], 1)
        ln2_out(x2rs[2], 2)
        ln2_out(x2rs[3], 3)

        # x3 fp8 residual: x3lo = fp8(8 * (x3 - b2 - dequant(x3hi)))
        for c in range(CC):
            dt_ = cp.tile([P, 512], bt16, tag="dres", name="dres")
            with nc.allow_low_precision(reason="fp8 split residual"):
                nc.vector.scalar_tensor_tensor(
                    out=dt_[:], in0=x3Tf[c][:, :],
                    scalar=g.sb_vec["b2"][:, c:c + 1],
                    in1=x3T8[c // 2][:, c % 2, :],
                    op0=OP.subtract, op1=OP.subtract)
                nc.vector.tensor_scalar(
                    out=x3L8[c // 2][:, c % 2, :], in0=dt_[:], scalar1=8.0,
                    scalar2=None, op0=OP.mult)

        # FFN in fp8 DoubleRow (256-row contraction per matmul); weights
        # stream through dedicated deep-prefetch pools so transfers overlap
        # the Act-bound LN2 stretch above.
        with tc.tile_pool(name="dp", bufs=1) as dp, \
             tc.tile_pool(name="w1p", bufs=10) as w1p, \
             tc.tile_pool(name="w2p", bufs=4) as w2p:
            h1p = [dp.tile([P, 2, 512], f8, tag=f"h1_{m}", name=f"h1_{m}")
                   for m in range(MM // 2)]
            for m in range(MM):
                w1m = w1p.tile([P, 3, 4, 2, P], f8, tag="w1m", name="w1m")
                nc.gpsimd.dma_start(w1m[:], g.w1[m])
                p1 = g.ps.tile([P, 512], f32, tag="ps", name="ps_f1")
                for kp in range(4):
                    nc.tensor.matmul(p1[:], w1m[:, 0, kp], x3T8[kp][:],
                                     perf_mode=mybir.MatmulPerfMode.DoubleRow,
                                     start=(kp == 0), stop=False)
                    nc.tensor.matmul(p1[:], w1m[:, 1, kp], x3T8[kp][:],
                                     perf_mode=mybir.MatmulPerfMode.DoubleRow,
                                     start=False, stop=False)
                    nc.tensor.matmul(p1[:], w1m[:, 2, kp], x3L8[kp][:],
                                     perf_mode=mybir.MatmulPerfMode.DoubleRow,
                                     start=False, stop=(kp == 3))
                with nc.allow_low_precision(reason="ffn hidden fp8"):
                    nc.scalar.activation(
                        out=h1p[m // 2][:, m % 2, :], in_=p1[:], func=AF.Gelu,
                        bias=g.sb_vec["b1"][:, m:m + 1], scale=FSC)
            for oc in range(CC):
                w2m = w2p.tile([P, 16, 2, P], f8, tag="w2m", name="w2m")
                nc.gpsimd.dma_start(w2m[:], g.w2[oc])
                p2 = g.ps.tile([P, 512], f32, tag="ps", name="ps_f2")
                for kp in range(16):
                    nc.tensor.matmul(p2[:], w2m[:, kp], h1p[kp][:],
                                     perf_mode=mybir.MatmulPerfMode.DoubleRow,
                                     start=(kp == 0), stop=(kp == 15))
                ot = cp.tile([P, R], f32, tag="otile", name="otile")
                nc.vector.scalar_tensor_tensor(
                    out=ot[:], in0=p2[:], scalar=FSC,
                    in1=x3Tf[oc][:, :], op0=OP.mult, op1=OP.add)
                nc.sync.dma_start(g.out[oc], ot[:])


def build_kernel():
    nc = bass.Bass("TRN2", target_bir_lowering=False, num_devices=8)
    g = Ctx()
    g.nc = nc

    # Weight layouts match the SBUF tile layout exactly so every DMA is
    # contiguous per partition (>=512B elements run at full DMA rate).
    g.xc = nc.dram_tensor("xc", [RT, C], bt16, kind="ExternalInput").ap()
    g.wq = nc.dram_tensor("wq", [CC, P, CC, P], bt16, kind="ExternalInput").ap()
    g.wk = nc.dram_tensor("wk", [CC, P, CC, P], bt16, kind="ExternalInput").ap()
    g.wv = nc.dram_tensor("wv", [P, 4, 2, C], f8, kind="ExternalInput").ap()
    g.wo = nc.dram_tensor("wo", [P, CC, C], bt16, kind="ExternalInput").ap()
    # w1 packs three fp8 copies (hi*64, residual*64, hi*8) for the 3-term
    # split-precision FFN1: x3hi@W1hi + x3hi@W1lo + x3lo@W1hi8, all scale 64.
    g.w1 = nc.dram_tensor("w1", [MM, P, 3, 4, 2, P], f8,
                          kind="ExternalInput").ap()
    g.w2 = nc.dram_tensor("w2", [CC, P, 16, 2, P], f8,
                          kind="ExternalInput").ap()
    g.masks = nc.dram_tensor("masks", [P, 4, QB], bt16,
                             kind="ExternalInput").ap()
    # All per-feature vectors packed into one tensor (single DMA); svl/sve
    # are per-core per-slot causal-relevance bits for the V-zeroing scheme.
    VEC_SPECS = [("bq", CC), ("bk", CC), ("bo", CC), ("b1", MM),
                 ("b2", CC), ("g1", CC), ("be1", CC), ("g2", CC), ("be2", CC),
                 ("be2f", CC), ("svl", 16), ("sve", 8),
                 ("sv64l", 16), ("sv64e", 8)]
    NVEC = sum(n for _, n in VEC_SPECS)
    g.vecs = nc.dram_tensor("vecs", [P, NVEC], f32, kind="ExternalInput").ap()
    g.out = nc.dram_tensor("out", [CC, P, R], f32, kind="ExternalOutput").ap()

    with tile.TileContext(nc) as tc:
        g.tc = tc
        with tc.tile_pool(name="setup", bufs=1) as setup, \
             tc.tile_pool(name="stats", bufs=4) as stats, \
             tc.tile_pool(name="wstr", bufs=4) as wstr, \
             tc.tile_pool(name="x1f_p", bufs=1) as x1f_p:
            g.stats, g.wstr = stats, wstr

            ident = setup.tile([P, P], f32, tag="ident", name="ident")
            make_identity(nc, ident[:])
            g.ident = ident
            identb = setup.tile([P, P], bt16, tag="identb", name="identb")
            with nc.allow_low_precision(reason="identity matrix cast"):
                nc.vector.tensor_copy(identb[:], ident[:])
            g.identb = identb
            g.ones64 = setup.tile([1, DH], bt16, tag="ones64", name="ones64")
            nc.vector.memset(g.ones64[:], 1.0)
            g.eps_sb = setup.tile([P, 1], f32, tag="eps", name="eps")
            nc.vector.memset(g.eps_sb[:], EPS)
            vt = setup.tile([P, NVEC], f32, tag="vec_all", name="vec_all")
            nc.sync.dma_start(vt[:], g.vecs)
            g.sb_vec = {}
            off = 0
            for nm, n in VEC_SPECS:
                g.sb_vec[nm] = vt[:, off:off + n]
                off += n
            g.g1s, g.be1s = g.sb_vec["g1"], g.sb_vec["be1"]
            g.g2s, g.be2s = g.sb_vec["g2"], g.sb_vec["be2"]

            g.x1f = [x1f_p.tile([P, 512], bt16, tag=f"x1f{c}", name=f"x1f{c}")
                     for c in range(CC)]

            with tc.tile_pool(name="x2p", bufs=1) as x2p:
                g.x2T = [x2p.tile([P, 512], bt16, tag=f"x2T{m}",
                                  name=f"x2T{m}") for m in range(CC)]
                with tc.tile_pool(name="kvp", bufs=1) as kvp:
                    g.kT = [[kvp.tile([P, 512], bt16, tag=f"kT{m}_{rb}",
                                      name=f"kT{m}_{rb}") for rb in range(4)]
                            for m in range(CC)]
                    g.vv = [kvp.tile([P, H, DH + 1], bt16, tag=f"vv{kt}",
                                     name=f"vv{kt}") for kt in range(T // P)]
                    g.vvE = [kvp.tile([P, H, DH + 1], bt16, tag=f"vvE{kt}",
                                      name=f"vvE{kt}") for kt in range(8)]
                    g.qT = [kvp.tile([P, 512], bt16, tag=f"qT{m}",
                                     name=f"qT{m}") for m in range(CC)]
                    with tc.tile_pool(name="psA", bufs=6,
                                      space="PSUM") as psA, \
                         tc.tile_pool(name="psAb", bufs=2,
                                      space="PSUM") as psAb:
                        g.ps, g.psb = psA, psAb
                        _phase_a(g)
                    _phase_b(g)
                with tc.tile_pool(name="psC", bufs=6, space="PSUM") as psC:
                    g.ps = psC
                    _phase_d(g)
    _split_sync_waits(nc)
    return nc


_NC_CACHE = None


def _get_nc():
    global _NC_CACHE
    if _NC_CACHE is None:
        _NC_CACHE = build_kernel()
    return _NC_CACHE


def _prep_shared(inputs):
    scale = DH ** -0.5
    Wq = np.asarray(inputs["Wq"], np.float32)
    Wk = np.asarray(inputs["Wk"], np.float32)
    Wv = np.asarray(inputs["Wv"], np.float32)
    Wo = np.asarray(inputs["Wo"], np.float32)
    W1 = np.asarray(inputs["W1"], np.float32)
    W2 = np.asarray(inputs["W2"], np.float32)
    bv_c = np.asarray(inputs["bv"], np.float32).reshape(C)

    def kt_layout(w):
        # [C(in), C(out)] -> [m, p, c, 128]: SBUF tile order for wkm/wqm
        cin, cout = w.shape
        return np.ascontiguousarray(
            w.reshape(cin // P, P, cout // P, P).transpose(2, 1, 0, 3)
        ).astype(bf16)

    def dr_layout(w, s=64.0):
        # [K, F] -> [m, p, kp, i, 128]: fp8 DoubleRow-interleaved, x`s` scaled
        k, fdim = w.shape
        return np.ascontiguousarray(
            (w * s).reshape(k // 256, 2, P, fdim // P, P)
            .transpose(3, 2, 0, 1, 4)).astype(fp8)

    def dr3_layout(w):
        # Three stacked fp8 DoubleRow copies: hi*64, residual*64, hi*8
        hi_mat = (w * 64.0).astype(fp8).astype(np.float32) / 64.0
        return np.ascontiguousarray(np.stack(
            [dr_layout(w, 64.0), dr_layout(w - hi_mat, 64.0),
             dr_layout(w, 8.0)], axis=2))

    def row_layout(w):
        # [C(in), F] -> [p, c, F]: SBUF tile order for wvs/wos
        cin = w.shape[0]
        return np.ascontiguousarray(
            w.reshape(cin // P, P, -1).transpose(1, 0, 2)).astype(bf16)

    def vec_layout(v):
        # [n*P] -> [P, n] (transposed into partition-major SBUF layout)
        n = v.size // P
        return np.ascontiguousarray(v.reshape(n, P).T).copy()

    shared = {
        "wq": kt_layout(Wq.transpose(1, 0, 2).reshape(C, C) * scale),
        "wk": kt_layout(Wk.transpose(1, 0, 2).reshape(C, C)),
        # fp8 DoubleRow layout [p, kp, i, F], x64 scaled (descaled via svl)
        "wv": np.ascontiguousarray(
            (Wv.transpose(1, 0, 2).reshape(C, C) * 64.0)
            .reshape(4, 2, P, C).transpose(2, 0, 1, 3)).astype(fp8),
        "wo": row_layout(Wo),
        "w1": dr3_layout(W1),
        "w2": dr_layout(W2),
        "vecs_base": np.concatenate([
            vec_layout(np.asarray(inputs["bq"], np.float32).reshape(C)
                       * scale),
            vec_layout(np.asarray(inputs["bk"], np.float32).reshape(C)),
            vec_layout(np.asarray(inputs["bo"], np.float32) + bv_c @ Wo),
            vec_layout(np.asarray(inputs["b1"], np.float32)),
            vec_layout(np.asarray(inputs["b2"], np.float32)),
            vec_layout(np.asarray(inputs["gamma1"], np.float32)),
            vec_layout(np.asarray(inputs["beta1"], np.float32)),
            vec_layout(np.asarray(inputs["gamma2"], np.float32)),
            vec_layout(np.asarray(inputs["beta2"], np.float32)),
            vec_layout(np.asarray(inputs["beta2"], np.float32)
                       + np.asarray(inputs["b2"], np.float32)),
        ], axis=1),
    }
    return shared


def _core_perm(j):
    """Slot -> key-chunk permutation: diagonal (causally partial) chunks of
    the early block at slots {0,1}, of the late block at slots {8,9}."""
    fixed = {0: 2 * j, 1: 2 * j + 1, 8: 14 - 2 * j, 9: 15 - 2 * j}
    rest = [c for c in range(16) if c not in fixed.values()]
    perm = []
    for s in range(16):
        perm.append(fixed[s] if s in fixed else rest.pop(0))
    return perm


def _core_masks(j, perm):
    """[128, 4, 256] bf16 0/1 masks for the diagonal slots: entries 0,1 mask
    slots 0,1 against the early block's columns; entries 2,3 mask slots 8,9
    against the late block's columns."""
    out = np.zeros((P, 4, QB), np.float32)
    key_p = np.arange(P)[:, None]
    col = np.arange(QB)[None, :]
    for i in range(2):
        out[:, i, :] = (perm[i] * P + key_p <= j * QB + col)
        out[:, 2 + i, :] = (perm[8 + i] * P + key_p <= (7 - j) * QB + col)
    return out.astype(bf16)


def _core_sv(j, perm):
    """Per-slot relevance bits. svl[s]: late half of slot s is causally live
    (diagonal late slots 8,9 use masks instead -> 1). sve[s]: early half of
    slot s is fully live (diagonal early slots 0,1 use masks -> 1)."""
    svl = np.zeros(16, np.float32)
    sve = np.zeros(8, np.float32)
    for s in range(16):
        if s in (8, 9) or perm[s] < 14 - 2 * j:
            svl[s] = 1.0
    for s in range(8):
        if s in (0, 1) or perm[s] < 2 * j:
            sve[s] = 1.0
    return svl, sve


def _make_in_maps(inputs):
    x = np.asarray(inputs["x"], np.float32)
    shared = _prep_shared(inputs)
    vecs_base = shared.pop("vecs_base")
    in_maps = []
    for c in range(8):
        gg, j = c // 4, c % 4
        perm = _core_perm(j)
        xb = x[gg]
        xbp = np.concatenate([xb[p * P:(p + 1) * P] for p in perm], 0)
        xq = np.concatenate([xb[j * QB:(j + 1) * QB],
                             xb[(7 - j) * QB:(8 - j) * QB]], 0)
        svl, sve = _core_sv(j, perm)
        m = dict(shared)
        m["xc"] = np.ascontiguousarray(
            np.concatenate([xbp, xq], 0)).astype(bf16)
        m["masks"] = _core_masks(j, perm)
        m["vecs"] = np.ascontiguousarray(np.concatenate([
            vecs_base,
            np.broadcast_to(svl[None, :], (P, 16)),
            np.broadcast_to(sve[None, :], (P, 8)),
            np.broadcast_to(svl[None, :] / 64, (P, 16)),
            np.broadcast_to(sve[None, :] / 64, (P, 8)),
        ], axis=1))
        in_maps.append(m)
    return in_maps


def _assemble(results):
    out = np.zeros((B, T, C), np.float32)
    for c in range(8):
        gg, j = c // 4, c % 4
        o = results[c]["out"].reshape(C, R).T  # [512, C] rows = 2 blocks
        out[gg, j * QB:(j + 1) * QB] = o[:QB]
        out[gg, (7 - j) * QB:(8 - j) * QB] = o[QB:]
    return out


def kernel(**inputs):
    in_maps = _make_in_maps(inputs)
    nc = _get_nc()
    res = run_bass_kernel_spmd(nc, in_maps, core_ids=list(range(8)))
    return _assemble(res.results)



# revision 103
# speedup vs baseline: 2.0208x; 1.0370x over previous
"""Trainium2 Bass kernel for a dense transformer block (B=2, T=2048, C=1024,
H=16, DFF=4096), distributed over 8 NeuronCores.

Sharding: 2 batch groups x 4-way query-block sharding. Core c handles batch
g=c//4 and query blocks {j, 7-j} (j=c%4) of 8 blocks of 256 rows. K/V are
computed per-core for the full batch (replicated; no collectives). Causality
is exploited statically: key-chunks 0-7 are needed by both query blocks
(masked only on block-0's columns), chunks 8-15 only by the late block.
The data-dependent causal boundary is applied with per-core 0/1 masks so one
NEFF serves all 8 cores (SPMD).

Activations are kept feature-major ("xT") so every matmul chains without
transposes; layernorm runs row-major with PE transposes between domains.
Softmax denominators ride the AV matmul as an extra ones-column of V.
"""
import numpy as np
import ml_dtypes

import concourse.bass as bass
import concourse.mybir as mybir
import concourse.tile as tile
from concourse.vector_clock import ScopedClock
from concourse.bass_utils import run_bass_kernel_spmd
from concourse.masks import make_identity

bf16 = ml_dtypes.bfloat16
fp8 = ml_dtypes.float8_e4m3
f32 = mybir.dt.float32
bt16 = mybir.dt.bfloat16
f8 = mybir.dt.float8e4
AF = mybir.ActivationFunctionType
OP = mybir.AluOpType

B, T, C, H, DH, DFF = 2, 2048, 1024, 16, 64, 4096
P = 128
QB = 256            # rows per query block
R = 512             # own query rows per core
RT = T + R          # ln1 rows per core (full batch + own q rows)
CC = C // P         # 8 feature chunks
MM = DFF // P       # 32 ffn chunks
EPS = 1e-5


# ---------------------------------------------------------------------------
# The walrus build in this container rejects instructions with >1 sync wait.
# Tile's sem assignment can emit several on one instruction; split the excess
# onto same-engine NoOps placed immediately before.
def _patched_drain_and_barrier(self, tick_clock, wait_clock):
    nc = self.nc
    probe = nc.sync.nop(nofuse=True, hint="tail_wait_probe")
    wait_clock.add_sem_waits(probe.ins, ScopedClock({None: tick_clock.global_clock}))
    si = probe.ins.sync_info
    waits = list(si.on_wait) if si is not None else []
    if si is not None:
        si.on_wait = waits[:1]
    for w in waits[1:]:
        n2 = nc.sync.nop(nofuse=True, hint="tail_wait_split")
        n2.ins.sync_info = mybir.SyncInfo(on_wait=[w], on_update=[])
    nc.sync.drain()
    nc.all_engine_barrier()
    assert self.sems is not None
    popped = nc._tile_sem_poison_stack.pop()
    assert popped is self._sem_poison
    nc.clear_and_free_semaphores(list(self.sems.allocated().values()))
    nc.all_engine_barrier()


tile.TileContext._drain_and_barrier = _patched_drain_and_barrier

_MAX_WAITS = 1
_split_counter = [0]


def _split_sync_waits(nc):
    for fn in nc.m.functions:
        for bb in fn.blocks:
            new_insts = []
            for inst in bb.instructions:
                si = getattr(inst, "sync_info", None)
                lim = _MAX_WAITS
                if si is not None and si.on_wait and len(si.on_wait) > lim:
                    waits = list(si.on_wait)
                    keep = waits[-lim:]
                    excess = waits[:-lim]
                    for i in range(0, len(excess), _MAX_WAITS):
                        _split_counter[0] += 1
                        nop = mybir.InstNoOp(
                            name=f"I-wsplit-{_split_counter[0]}", ins=[], outs=[])
                        nop.engine = inst.engine
                        nop.sync_info = mybir.SyncInfo(
                            on_wait=excess[i:i + _MAX_WAITS], on_update=[])
                        new_insts.append(nop)
                    si.on_wait = keep
                new_insts.append(inst)
            bb.instructions = new_insts
# ---------------------------------------------------------------------------


class Ctx:
    pass


def _layernorm_pre(g, xt, xb):
    """Row-major LN of xt [128, C] into the bf16 tile xb (DVE + tiny Act
    sqrt). bf16 output makes the downstream PE transposes 2x cheaper."""
    nc = g.nc
    st = g.stats.tile([P, 2, 6], f32, tag="bnst", name="bnst")
    xv = xt.rearrange("p (s d) -> p s d", s=2)
    for sg in range(2):
        nc.vector.bn_stats(out=st[:, sg, :], in_=xv[:, sg, :])
    mv = g.stats.tile([P, 2], f32, tag="bnmv", name="bnmv")
    nc.vector.bn_aggr(out=mv[:], in_=st[:])
    sq = g.stats.tile([P, 1], f32, tag="bnsq", name="bnsq")
    nc.scalar.activation(out=sq[:], in_=mv[:, 1:2], func=AF.Sqrt,
                         bias=g.eps_sb[:], scale=float(C) / (C - 1))
    rstd = g.stats.tile([P, 1], f32, tag="bnrstd", name="bnrstd")
    nc.vector.reciprocal(rstd[:], sq[:])
    with nc.allow_low_precision(reason="normalized x in bf16"):
        nc.vector.tensor_scalar(out=xb[:], in0=xt[:], scalar1=mv[:, 0:1],
                                scalar2=rstd[:], op0=OP.subtract, op1=OP.mult)


def _layernorm_post(g, xb, out_writes):
    """Transpose normalized bf16 xb per feature chunk (4 chunks batched per
    PSUM tile); out_writes(c, psum_slice)."""
    nc = g.nc
    for half in range(2):
        pt = g.psb.tile([P, 512], bt16, tag="psb", name="ps_t")
        for i in range(4):
            c = half * 4 + i
            nc.tensor.transpose(pt[:P, i * P:(i + 1) * P],
                                xb[:, c * P:(c + 1) * P], g.identb[:])
        for i in range(4):
            out_writes(half * 4 + i, pt[:P, i * P:(i + 1) * P])


def _layernorm_tile(g, xt, xb, out_writes):
    _layernorm_pre(g, xt, xb)
    _layernorm_post(g, xb, out_writes)


def _ln1_pre(g, rt):
    """DMA + row-major LN of tile rt; returns the normalized bf16 tile."""
    nc = g.nc
    xt = g.xio.tile([P, C], bt16, tag="xin", name="xin")
    nc.sync.dma_start(xt[:], g.xc[rt * P:(rt + 1) * P, :])
    xb = g.xio.tile([P, C], bt16, tag="xbn", name="xbn")
    _layernorm_pre(g, xt, xb)
    return xb


def _ln1_post(g, x1T, xt, rt):
    nc = g.nc
    rb, r0 = rt // 4, (rt % 4) * P

    def wr1(c, pt):
        nc.scalar.activation(
            out=x1T[rb][:, c, r0:r0 + P], in_=pt, func=AF.Identity,
            bias=g.be1s[:, c:c + 1], scale=g.g1s[:, c:c + 1])
        if rt >= T // P:
            q0 = (rt - T // P) * P
            nc.scalar.activation(
                out=g.x1f[c][:, q0:q0 + P], in_=pt, func=AF.Identity,
                bias=g.be1s[:, c:c + 1], scale=g.g1s[:, c:c + 1])
        elif c % 2 == 1:
            # fp8 copy of ln1(x) feeding the DoubleRow V projection; odd
            # chunks on Act here, even chunks bulk-copied on DVE later
            with nc.allow_low_precision(reason="x1 fp8 for V matmul"):
                nc.scalar.activation(
                    out=g.x1T8[rb][:, c // 2, c % 2, r0:r0 + P], in_=pt,
                    func=AF.Identity, bias=g.be1s[:, c:c + 1],
                    scale=g.g1s[:, c:c + 1])
    _layernorm_post(g, xt, wr1)


def _wk_fetch(g, m):
    wkm = g.wstr.tile([P, CC, P], bt16, tag="wstr", name="wkm")
    g.nc.gpsimd.dma_start(wkm[:, :, :], g.wk[m])
    return wkm


def _kproj_chunk(g, x1T, rb, m, wkm=None):
    """K^T projection of row-block rb, output chunk m."""
    nc = g.nc
    if wkm is None:
        wkm = _wk_fetch(g, m)
    pk = g.ps.tile([P, 512], f32, tag="ps", name="ps_k")
    for c in range(CC):
        nc.tensor.matmul(pk[:], wkm[:, c, :], x1T[rb][:, c, :],
                         start=(c == 0), stop=(c == CC - 1))
    nc.vector.tensor_scalar(
        out=g.kT[m][rb][:, :], in0=pk[:],
        scalar1=g.sb_vec["bk"][:, m:m + 1], scalar2=None, op0=OP.add)


def _vproj_kt(g, x1T, wvs, kt):
    """V projection (fp8 DoubleRow, both 512-col halves) for key-tile kt.
    Writes vv[kt] scaled by the per-core late-relevance bit svl[kt] (which
    also carries the 1/64 fp8 weight descale - it cancels in the softmax
    division), and (for kt < 8) vvE[kt] scaled by sve[kt]."""
    nc = g.nc
    rb, r0 = kt // 4, (kt % 4) * P
    for half in range(2):
        pv = g.ps.tile([P, 512], f32, tag="ps", name="ps_v")
        for kp in range(4):
            nc.tensor.matmul(pv[:], g.x1T8[rb][:, kp, :, r0:r0 + P],
                             wvs[:, kp, :, half * 512:(half + 1) * 512],
                             perf_mode=mybir.MatmulPerfMode.DoubleRow,
                             start=(kp == 0), stop=(kp == 3))
        pvh = pv.rearrange("p (h d) -> p h d", h=8)
        nc.vector.tensor_scalar(
            out=g.vv[kt][:, half * 8:(half + 1) * 8, 0:DH], in0=pvh,
            scalar1=g.sb_vec["sv64l"][:, kt:kt + 1], scalar2=None,
            op0=OP.mult)
        if kt < 8:
            # second (early-block) copy on the Act engine to offload DVE
            nc.scalar.activation(
                out=g.vvE[kt][:, half * 8:(half + 1) * 8, 0:DH], in_=pvh,
                func=AF.Identity, scale=g.sb_vec["sv64e"][:, kt:kt + 1])


def _phase_a(g):
    """LN1 + transpose + Q/K/V projections, software-pipelined at tile
    granularity: LN stats (DVE) run ahead; each LN transpose batch is
    staggered between projection matmul chunks of the previous row-block so
    the in-order PE stream never waits long."""
    nc, tc = g.nc, g.tc
    with tc.tile_pool(name="x1p", bufs=1) as x1p, \
         tc.tile_pool(name="xio", bufs=3) as xio, \
         tc.tile_pool(name="wvp", bufs=1) as wvp:
        g.xio = xio
        # x1T split per 512-row block (rb 0-3 = batch, rb 4 = own q rows)
        x1T = [x1p.tile([P, CC, 512], bt16, tag=f"x1T{rb}", name=f"x1T{rb}")
               for rb in range(5)]
        g.x1T8 = [x1p.tile([P, 4, 2, 512], f8, tag=f"x1T8{rb}",
                           name=f"x1T8{rb}") for rb in range(4)]
        wvs = wvp.tile([P, 4, 2, C], f8, tag="wvs", name="wvs")
        # Prologue: LN of row-block 0 (no projections to overlap with yet)
        xts = [_ln1_pre(g, rt) for rt in range(3)]
        _ln1_post(g, x1T, xts[0], 0)
        xts.append(_ln1_pre(g, 3))
        _ln1_post(g, x1T, xts[1], 1)
        _ln1_post(g, x1T, xts[2], 2)
        _ln1_post(g, x1T, xts[3], 3)

        # ones columns (softmax denominator), scaled by per-slot relevance
        for kt in range(T // P):
            nc.vector.memset(g.vv[kt][:, :, DH:DH + 1], 1.0)
            nc.vector.tensor_scalar(
                out=g.vv[kt][:, :, DH:DH + 1], in0=g.vv[kt][:, :, DH:DH + 1],
                scalar1=g.sb_vec["svl"][:, kt:kt + 1], scalar2=None,
                op0=OP.mult)
        for kt in range(8):
            nc.vector.memset(g.vvE[kt][:, :, DH:DH + 1], 1.0)
            nc.vector.tensor_scalar(
                out=g.vvE[kt][:, :, DH:DH + 1],
                in0=g.vvE[kt][:, :, DH:DH + 1],
                scalar1=g.sb_vec["sve"][:, kt:kt + 1], scalar2=None,
                op0=OP.mult)

        # Steady state: projections of rb overlap LN of rb+1.
        # PE emission order per rb: K(m0) T(t0) K(m1) T(t1) K(m2) T(t2)
        # K(m3) T(t3) K(m4..7) V(kt*4); LN-pre(t_i) is emitted just before
        # K(m_i) so DVE stats run one matmul-chunk ahead of the transpose.
        # The first two K-weight chunks of rb+1 are prefetched before the
        # V block so the next iteration's PE stream starts without a DMA
        # stall; wvs (V weights) is fetched late enough not to block the
        # startup x-tile loads.
        wk_pre = [_wk_fetch(g, 0), _wk_fetch(g, 1)]
        for rb in range(4):
            for m in range(CC):
                if rb == 0 and m in (3, 5):
                    h0 = (m - 3) // 2 * 512
                    nc.gpsimd.dma_start(wvs[:, :, :, h0:h0 + 512],
                                        g.wv[:, :, :, h0:h0 + 512])
                if m < 4:
                    xt = _ln1_pre(g, (rb + 1) * 4 + m)
                _kproj_chunk(g, x1T, rb, m,
                             wkm=wk_pre[m] if m < 2 else None)
                if m < 4:
                    _ln1_post(g, x1T, xt, (rb + 1) * 4 + m)
            if rb < 3:
                wk_pre = [_wk_fetch(g, 0), _wk_fetch(g, 1)]
            # even chunks of the fp8 x1 copy, bulk on DVE (Act does odd)
            for c in range(0, CC, 2):
                with nc.allow_low_precision(reason="x1 fp8 for V matmul"):
                    nc.vector.tensor_copy(
                        out=g.x1T8[rb][:, c // 2, 0, :],
                        in_=x1T[rb][:, c, :])
            for i in range(4):
                _vproj_kt(g, x1T, wvs, rb * 4 + i)

        # Q^T projection of the own-query rows (x1T[4])
        for m in range(CC):
            wqm = g.wstr.tile([P, CC, P], bt16, tag="wstr", name="wqm")
            nc.gpsimd.dma_start(wqm[:, :, :], g.wq[m])
            pq = g.ps.tile([P, 512], f32, tag="ps", name="ps_q")
            for c in range(CC):
                nc.tensor.matmul(pq[:], wqm[:, c, :], x1T[4][:, c, :],
                                 start=(c == 0), stop=(c == CC - 1))
            nc.vector.tensor_scalar(
                out=g.qT[m][:, :], in0=pq[:],
                scalar1=g.sb_vec["bq"][:, m:m + 1], scalar2=None, op0=OP.add)


def _phase_b(g):
    """Attention, both query blocks fused on the free dim (cols 0:256 = early
    block, 256:512 = late block). Key chunks arrive permuted per core so the
    causally-partial (diagonal) chunks sit at slots {0,1} (early) / {8,9}
    (late); all other slots need no elementwise mask because the V copies
    (vvE for the early half, vv for the late half) are zeroed per-slot when
    that chunk is causally irrelevant, nulling both numerator and the
    ones-column denominator. exp runs on 2-PSUM-bank batches; the softmax
    denominator reciprocal is broadcast on the idle GpSimd engine."""
    nc, tc = g.nc, g.tc
    with tc.tile_pool(name="mp", bufs=1) as mp, \
         tc.tile_pool(name="apl", bufs=2) as apl, \
         tc.tile_pool(name="rcp", bufs=2) as rcp, \
         tc.tile_pool(name="hp", bufs=1) as hp, \
         tc.tile_pool(name="wop", bufs=1) as wop, \
         tc.tile_pool(name="scp", bufs=3, space="PSUM") as scp, \
         tc.tile_pool(name="pvp", bufs=2, space="PSUM") as pvp:
        g.hcat = [hp.tile([P, 512], bt16, tag=f"hcat{c}", name=f"hcat{c}")
                  for c in range(CC)]
        mq = mp.tile([P, 4, QB], bt16, tag="mask", name="mask")
        nc.sync.dma_start(mq[:], g.masks)
        # prefetch the Wo weights during attention (DMA engine is idle here)
        wos = wop.tile([P, CC, C], bt16, tag="wos", name="wos")
        nc.gpsimd.dma_start(wos[:], g.wo)
        for pair in range(CC):
            for hl in range(2):
                h = 2 * pair + hl
                hs = slice(hl * DH, (hl + 1) * DH)
                aA = apl.tile([P, 8, 512], bt16, tag="aA", name="aA")
                aB = apl.tile([P, 8, QB], bt16, tag="aB", name="aB")
                # scores slots 0..7 (512 wide): 2 slots per 2-bank psum tile
                for t4 in range(4):
                    psc = scp.tile([P, 1024], f32, tag="sc", name="ps_s")
                    for k in range(2):
                        s = 2 * t4 + k
                        rb, k0 = s // 4, (s % 4) * P
                        nc.tensor.matmul(
                            psc[:, k * 512:(k + 1) * 512],
                            g.kT[pair][rb][hs, k0:k0 + P],
                            g.qT[pair][hs, 0:512], start=True, stop=True,
                            tile_position=(hl * DH, 0))
                    nc.scalar.activation(out=aA[:, 2 * t4:2 * t4 + 2, :],
                                         in_=psc[:], func=AF.Exp)
                # slots 8..15 (late block only, 256 wide): 4 per psum tile
                for t4 in range(2):
                    psc = scp.tile([P, 1024], f32, tag="sc", name="ps_sB")
                    for k in range(4):
                        s = 8 + 4 * t4 + k
                        rb, k0 = s // 4, (s % 4) * P
                        nc.tensor.matmul(
                            psc[:, k * QB:(k + 1) * QB],
                            g.kT[pair][rb][hs, k0:k0 + P],
                            g.qT[pair][hs, QB:512], start=True, stop=True,
                            tile_position=(hl * DH, 0))
                    nc.scalar.activation(out=aB[:, 4 * t4:4 * t4 + 4, :],
                                         in_=psc[:], func=AF.Exp)
                # elementwise causal masks only on the diagonal slots
                nc.vector.tensor_mul(aA[:, 0:2, 0:QB], aA[:, 0:2, 0:QB],
                                     mq[:, 0:2, :])
                nc.vector.tensor_mul(aB[:, 0:2, :], aB[:, 0:2, :],
                                     mq[:, 2:4, :])
                pav = pvp.tile([P, 512], f32, tag="pav", name="ps_av")
                for s in range(8):
                    nc.tensor.matmul(pav[:DH + 1, 0:QB], g.vvE[s][:, h, :],
                                     aA[:, s, 0:QB], start=(s == 0),
                                     stop=(s == 7))
                for s in range(8):
                    nc.tensor.matmul(pav[:DH + 1, QB:512], g.vv[s][:, h, :],
                                     aA[:, s, QB:512], start=(s == 0),
                                     stop=False)
                for s in range(8, 16):
                    nc.tensor.matmul(pav[:DH + 1, QB:512], g.vv[s][:, h, :],
                                     aB[:, s - 8, :], start=False,
                                     stop=(s == 15))
                den = g.stats.tile([1, 512], f32, tag="den", name="den")
                nc.vector.tensor_copy(den[:], pav[DH:DH + 1, :512])
                rr = g.stats.tile([1, 512], bt16, tag="rr", name="rr")
                with nc.allow_low_precision(reason="softmax denom in bf16"):
                    nc.vector.reciprocal(rr[:], den[:])
                prb = pvp.tile([P, 512], f32, tag="pav", name="ps_r")
                nc.tensor.matmul(prb[:DH, :512], g.ones64[:], rr[:],
                                 start=True, stop=True)
                rbc = rcp.tile([DH, 512], bt16, tag="rbc", name="rbc")
                with nc.allow_low_precision(reason="softmax denom in bf16"):
                    nc.vector.tensor_copy(rbc[:], prb[:DH, :512])
                nc.vector.tensor_mul(out=g.hcat[pair][hs, :],
                                     in0=pav[:DH, :512], in1=rbc[:])

        # Wo projection + residual (uses hcat/x1f/wos before they die)
        for m in range(CC):
            pa = pvp.tile([P, 512], f32, tag="pav", name="ps_o")
            for c in range(CC):
                nc.tensor.matmul(pa[:, :512], wos[:, c, m * P:(m + 1) * P],
                                 g.hcat[c][:, :], start=(c == 0),
                                 stop=(c == CC - 1))
            with nc.allow_low_precision(reason="x2 residual in bf16"):
                nc.vector.scalar_tensor_tensor(
                    out=g.x2T[m][:, :], in0=pa[:, :512],
                    scalar=g.sb_vec["bo"][:, m:m + 1], in1=g.x1f[m][:, :],
                    op0=OP.add, op1=OP.add)


def _phase_d(g):
    """LN2 + FFN (fp8 DoubleRow) + output.

    FFN weights are pre-scaled by 64 on the host so fp8e4 quantization stays
    in the normal range; the 1/64 rides the gelu's scale operand (FFN1) and
    the epilogue's scalar multiply (FFN2). b2 is pre-folded into the f32
    residual copy of ln2(x) (bias be2f = beta2 + b2)."""
    nc, tc = g.nc, g.tc
    FSC = 1.0 / 64
    x2T = g.x2T
    with tc.tile_pool(name="cp", bufs=3) as cp, \
         tc.tile_pool(name="psB", bufs=4, space="PSUM") as psB, \
         tc.tile_pool(name="x3p", bufs=1) as x3p:
        g.psb = psB
        # x3T8[kp]: ln2(x) in fp8, DoubleRow-interleaved over feature pairs;
        # x3L8: the x8-scaled fp8 quantization residual (split precision)
        x3T8 = [x3p.tile([P, 2, 512], f8, tag=f"x3T8{c}", name=f"x3T8{c}")
                for c in range(CC // 2)]
        x3L8 = [x3p.tile([P, 2, 512], f8, tag=f"x3L8{c}", name=f"x3L8{c}")
                for c in range(CC // 2)]
        x3Tf = [x3p.tile([P, 512], f32, tag=f"x3Tf{c}", name=f"x3Tf{c}")
                for c in range(CC)]

        def ln2_in(rt):
            # transpose x2 row-tile into 2 batched PSUM tiles; bn_stats and
            # the normalize read PSUM directly (no Act assembly copies)
            pts = []
            st = g.stats.tile([P, 2, 6], f32, tag="bnst", name="bnst")
            for half in range(2):
                pt = g.psb.tile([P, 512], bt16, tag="psb", name="ps_t2")
                for i in range(4):
                    c = half * 4 + i
                    nc.tensor.transpose(pt[:P, i * P:(i + 1) * P],
                                        x2T[c][:, rt * P:(rt + 1) * P],
                                        g.identb[:])
                nc.vector.bn_stats(out=st[:, half, :], in_=pt[:, :])
                pts.append(pt)
            mv = g.stats.tile([P, 2], f32, tag="bnmv", name="bnmv")
            nc.vector.bn_aggr(out=mv[:], in_=st[:])
            sq = g.stats.tile([P, 1], f32, tag="bnsq", name="bnsq")
            nc.scalar.activation(out=sq[:], in_=mv[:, 1:2], func=AF.Sqrt,
                                 bias=g.eps_sb[:], scale=float(C) / (C - 1))
            rstd = g.stats.tile([P, 1], f32, tag="bnrstd", name="bnrstd")
            nc.vector.reciprocal(rstd[:], sq[:])
            x2b = cp.tile([P, C], bt16, tag="x2b", name="x2b")
            with nc.allow_low_precision(reason="normalized x in bf16"):
                for half in range(2):
                    nc.vector.tensor_scalar(
                        out=x2b[:, half * 512:(half + 1) * 512],
                        in0=pts[half][:, :], scalar1=mv[:, 0:1],
                        scalar2=rstd[:], op0=OP.subtract, op1=OP.mult)
            return x2b

        def ln2_out(x2r, rt):
            def wr2(c, pt):
                with nc.allow_low_precision(reason="ffn input quantize fp8"):
                    nc.scalar.activation(
                        out=x3T8[c // 2][:, c % 2, rt * P:(rt + 1) * P],
                        in_=pt, func=AF.Identity, bias=g.be2s[:, c:c + 1],
                        scale=g.g2s[:, c:c + 1])
                nc.scalar.activation(
                    out=x3Tf[c][:, rt * P:(rt + 1) * P], in_=pt,
                    func=AF.Identity, bias=g.sb_vec["be2f"][:, c:c + 1],
                    scale=g.g2s[:, c:c + 1])
            _layernorm_post(g, x2r, wr2)

        # software-pipelined: PE transposes of tile rt+1 overlap Act/DVE of rt
        x2rs = [ln2_in(0), ln2_in(1), ln2_in(2)]
        ln2_out(x2rs[0], 0)
        x2rs.append(ln2_in(3))
        ln2_out(x2rs[1], 1)
        ln2_out(x2rs[2], 2)
        ln2_out(x2rs[3], 3)

        # x3 fp8 residual: x3lo = fp8(8 * (x3 - b2 - dequant(x3hi)))
        for c in range(CC):
            dt_ = cp.tile([P, 512], bt16, tag="dres", name="dres")
            with nc.allow_low_precision(reason="fp8 split residual"):
                nc.vector.scalar_tensor_tensor(
                    out=dt_[:], in0=x3Tf[c][:, :],
                    scalar=g.sb_vec["b2"][:, c:c + 1],
                    in1=x3T8[c // 2][:, c % 2, :],
                    op0=OP.subtract, op1=OP.subtract)
                nc.vector.tensor_scalar(
                    out=x3L8[c // 2][:, c % 2, :], in0=dt_[:], scalar1=8.0,
                    scalar2=None, op0=OP.mult)

        # FFN in fp8 DoubleRow (256-row contraction per matmul); weights
        # stream through dedicated deep-prefetch pools so transfers overlap
        # the Act-bound LN2 stretch above.
        with tc.tile_pool(name="dp", bufs=1) as dp, \
             tc.tile_pool(name="w1p", bufs=10) as w1p, \
             tc.tile_pool(name="w2p", bufs=4) as w2p:
            h1p = [dp.tile([P, 2, 512], f8, tag=f"h1_{m}", name=f"h1_{m}")
                   for m in range(MM // 2)]
            for m in range(MM):
                w1m = w1p.tile([P, 3, 4, 2, P], f8, tag="w1m", name="w1m")
                nc.gpsimd.dma_start(w1m[:], g.w1[m])
                p1 = g.ps.tile([P, 512], f32, tag="ps", name="ps_f1")
                for kp in range(4):
                    nc.tensor.matmul(p1[:], w1m[:, 0, kp], x3T8[kp][:],
                                     perf_mode=mybir.MatmulPerfMode.DoubleRow,
                                     start=(kp == 0), stop=False)
                    nc.tensor.matmul(p1[:], w1m[:, 1, kp], x3T8[kp][:],
                                     perf_mode=mybir.MatmulPerfMode.DoubleRow,
                                     start=False, stop=False)
                    nc.tensor.matmul(p1[:], w1m[:, 2, kp], x3L8[kp][:],
                                     perf_mode=mybir.MatmulPerfMode.DoubleRow,
                                     start=False, stop=(kp == 3))
                with nc.allow_low_precision(reason="ffn hidden fp8"):
                    nc.scalar.activation(
                        out=h1p[m // 2][:, m % 2, :], in_=p1[:], func=AF.Gelu,
                        bias=g.sb_vec["b1"][:, m:m + 1], scale=FSC)
            for oc in range(CC):
                w2m = w2p.tile([P, 16, 2, P], f8, tag="w2m", name="w2m")
                nc.gpsimd.dma_start(w2m[:], g.w2[oc])
                p2 = g.ps.tile([P, 512], f32, tag="ps", name="ps_f2")
                for kp in range(16):
                    nc.tensor.matmul(p2[:], w2m[:, kp], h1p[kp][:],
                                     perf_mode=mybir.MatmulPerfMode.DoubleRow,
                                     start=(kp == 0), stop=(kp == 15))
                ot = cp.tile([P, R], f32, tag="otile", name="otile")
                nc.vector.scalar_tensor_tensor(
                    out=ot[:], in0=p2[:], scalar=FSC,
                    in1=x3Tf[oc][:, :], op0=OP.mult, op1=OP.add)
                nc.sync.dma_start(g.out[oc], ot[:])


def build_kernel():
    nc = bass.Bass("TRN2", target_bir_lowering=False, num_devices=8)
    g = Ctx()
    g.nc = nc

    # Weight layouts match the SBUF tile layout exactly so every DMA is
    # contiguous per partition (>=512B elements run at full DMA rate).
    g.xc = nc.dram_tensor("xc", [RT, C], bt16, kind="ExternalInput").ap()
    g.wq = nc.dram_tensor("wq", [CC, P, CC, P], bt16, kind="ExternalInput").ap()
    g.wk = nc.dram_tensor("wk", [CC, P, CC, P], bt16, kind="ExternalInput").ap()
    g.wv = nc.dram_tensor("wv", [P, 4, 2, C], f8, kind="ExternalInput").ap()
    g.wo = nc.dram_tensor("wo", [P, CC, C], bt16, kind="ExternalInput").ap()
    # w1 packs three fp8 copies (hi*64, residual*64, hi*8) for the 3-term
    # split-precision FFN1: x3hi@W1hi + x3hi@W1lo + x3lo@W1hi8, all scale 64.
    g.w1 = nc.dram_tensor("w1", [MM, P, 3, 4, 2, P], f8,
                          kind="ExternalInput").ap()
    g.w2 = nc.dram_tensor("w2", [CC, P, 16, 2, P], f8,
                          kind="ExternalInput").ap()
    g.masks = nc.dram_tensor("masks", [P, 4, QB], bt16,
                             kind="ExternalInput").ap()
    # All per-feature vectors packed into one tensor (single DMA); svl/sve
    # are per-core per-slot causal-relevance bits for the V-zeroing scheme.
    VEC_SPECS = [("bq", CC), ("bk", CC), ("bo", CC), ("b1", MM),
                 ("b2", CC), ("g1", CC), ("be1", CC), ("g2", CC), ("be2", CC),
                 ("be2f", CC), ("svl", 16), ("sve", 8),
                 ("sv64l", 16), ("sv64e", 8)]
    NVEC = sum(n for _, n in VEC_SPECS)
    g.vecs = nc.dram_tensor("vecs", [P, NVEC], f32, kind="ExternalInput").ap()
    g.out = nc.dram_tensor("out", [CC, P, R], f32, kind="ExternalOutput").ap()

    with tile.TileContext(nc) as tc:
        g.tc = tc
        with tc.tile_pool(name="setup", bufs=1) as setup, \
             tc.tile_pool(name="stats", bufs=4) as stats, \
             tc.tile_pool(name="wstr", bufs=4) as wstr, \
             tc.tile_pool(name="x1f_p", bufs=1) as x1f_p:
            g.stats, g.wstr = stats, wstr

            ident = setup.tile([P, P], f32, tag="ident", name="ident")
            make_identity(nc, ident[:])
            g.ident = ident
            identb = setup.tile([P, P], bt16, tag="identb", name="identb")
            with nc.allow_low_precision(reason="identity matrix cast"):
                nc.vector.tensor_copy(identb[:], ident[:])
            g.identb = identb
            g.ones64 = setup.tile([1, DH], bt16, tag="ones64", name="ones64")
            nc.vector.memset(g.ones64[:], 1.0)
            g.eps_sb = setup.tile([P, 1], f32, tag="eps", name="eps")
            nc.vector.memset(g.eps_sb[:], EPS)
            vt = setup.tile([P, NVEC], f32, tag="vec_all", name="vec_all")
            nc.sync.dma_start(vt[:], g.vecs)
            g.sb_vec = {}
            off = 0
            for nm, n in VEC_SPECS:
                g.sb_vec[nm] = vt[:, off:off + n]
                off += n
            g.g1s, g.be1s = g.sb_vec["g1"], g.sb_vec["be1"]
            g.g2s, g.be2s = g.sb_vec["g2"], g.sb_vec["be2"]

            g.x1f = [x1f_p.tile([P, 512], bt16, tag=f"x1f{c}", name=f"x1f{c}")
                     for c in range(CC)]

            with tc.tile_pool(name="x2p", bufs=1) as x2p:
                g.x2T = [x2p.tile([P, 512], bt16, tag=f"x2T{m}",
                                  name=f"x2T{m}") for m in range(CC)]
                with tc.tile_pool(name="kvp", bufs=1) as kvp:
                    g.kT = [[kvp.tile([P, 512], bt16, tag=f"kT{m}_{rb}",
                                      name=f"kT{m}_{rb}") for rb in range(4)]
                            for m in range(CC)]
                    g.vv = [kvp.tile([P, H, DH + 1], bt16, tag=f"vv{kt}",
                                     name=f"vv{kt}") for kt in range(T // P)]
                    g.vvE = [kvp.tile([P, H, DH + 1], bt16, tag=f"vvE{kt}",
                                      name=f"vvE{kt}") for kt in range(8)]
                    g.qT = [kvp.tile([P, 512], bt16, tag=f"qT{m}",
                                     name=f"qT{m}") for m in range(CC)]
                    with tc.tile_pool(name="psA", bufs=6,
                                      space="PSUM") as psA, \
                         tc.tile_pool(name="psAb", bufs=2,
                                      space="PSUM") as psAb:
                        g.ps, g.psb = psA, psAb
                        _phase_a(g)
                    _phase_b(g)
                with tc.tile_pool(name="psC", bufs=4, space="PSUM") as psC:
                    g.ps = psC
                    _phase_d(g)
    _split_sync_waits(nc)
    return nc


_NC_CACHE = None


def _get_nc():
    global _NC_CACHE
    if _NC_CACHE is None:
        _NC_CACHE = build_kernel()
    return _NC_CACHE


def _prep_shared(inputs):
    scale = DH ** -0.5
    Wq = np.asarray(inputs["Wq"], np.float32)
    Wk = np.asarray(inputs["Wk"], np.float32)
    Wv = np.asarray(inputs["Wv"], np.float32)
    Wo = np.asarray(inputs["Wo"], np.float32)
    W1 = np.asarray(inputs["W1"], np.float32)
    W2 = np.asarray(inputs["W2"], np.float32)
    bv_c = np.asarray(inputs["bv"], np.float32).reshape(C)

    def kt_layout(w):
        # [C(in), C(out)] -> [m, p, c, 128]: SBUF tile order for wkm/wqm
        cin, cout = w.shape
        return np.ascontiguousarray(
            w.reshape(cin // P, P, cout // P, P).transpose(2, 1, 0, 3)
        ).astype(bf16)

    def dr_layout(w, s=64.0):
        # [K, F] -> [m, p, kp, i, 128]: fp8 DoubleRow-interleaved, x`s` scaled
        k, fdim = w.shape
        return np.ascontiguousarray(
            (w * s).reshape(k // 256, 2, P, fdim // P, P)
            .transpose(3, 2, 0, 1, 4)).astype(fp8)

    def dr3_layout(w):
        # Three stacked fp8 DoubleRow copies: hi*64, residual*64, hi*8
        hi_mat = (w * 64.0).astype(fp8).astype(np.float32) / 64.0
        return np.ascontiguousarray(np.stack(
            [dr_layout(w, 64.0), dr_layout(w - hi_mat, 64.0),
             dr_layout(w, 8.0)], axis=2))

    def row_layout(w):
        # [C(in), F] -> [p, c, F]: SBUF tile order for wvs/wos
        cin = w.shape[0]
        return np.ascontiguousarray(
            w.reshape(cin // P, P, -1).transpose(1, 0, 2)).astype(bf16)

    def vec_layout(v):
        # [n*P] -> [P, n] (transposed into partition-major SBUF layout)
        n = v.size // P
        return np.ascontiguousarray(v.reshape(n, P).T).copy()

    shared = {
        "wq": kt_layout(Wq.transpose(1, 0, 2).reshape(C, C) * scale),
        "wk": kt_layout(Wk.transpose(1, 0, 2).reshape(C, C)),
        # fp8 DoubleRow layout [p, kp, i, F], x64 scaled (descaled via svl)
        "wv": np.ascontiguousarray(
            (Wv.transpose(1, 0, 2).reshape(C, C) * 64.0)
            .reshape(4, 2, P, C).transpose(2, 0, 1, 3)).astype(fp8),
        "wo": row_layout(Wo),
        "w1": dr3_layout(W1),
        "w2": dr_layout(W2),
        "vecs_base": np.concatenate([
            vec_layout(np.asarray(inputs["bq"], np.float32).reshape(C)
                       * scale),
            vec_layout(np.asarray(inputs["bk"], np.float32).reshape(C)),
            vec_layout(np.asarray(inputs["bo"], np.float32) + bv_c @ Wo),
            vec_layout(np.asarray(inputs["b1"], np.float32)),
            vec_layout(np.asarray(inputs["b2"], np.float32)),
            vec_layout(np.asarray(inputs["gamma1"], np.float32)),
            vec_layout(np.asarray(inputs["beta1"], np.float32)),
            vec_layout(np.asarray(inputs["gamma2"], np.float32)),
            vec_layout(np.asarray(inputs["beta2"], np.float32)),
            vec_layout(np.asarray(inputs["beta2"], np.float32)
                       + np.asarray(inputs["b2"], np.float32)),
        ], axis=1),
    }
    return shared


def _core_perm(j):
    """Slot -> key-chunk permutation: diagonal (causally partial) chunks of
    the early block at slots {0,1}, of the late block at slots {8,9}."""
    fixed = {0: 2 * j, 1: 2 * j + 1, 8: 14 - 2 * j, 9: 15 - 2 * j}
    rest = [c for c in range(16) if c not in fixed.values()]
    perm = []
    for s in range(16):
        perm.append(fixed[s] if s in fixed else rest.pop(0))
    return perm


def _core_masks(j, perm):
    """[128, 4, 256] bf16 0/1 masks for the diagonal slots: entries 0,1 mask
    slots 0,1 against the early block's columns; entries 2,3 mask slots 8,9
    against the late block's columns."""
    out = np.zeros((P, 4, QB), np.float32)
    key_p = np.arange(P)[:, None]
    col = np.arange(QB)[None, :]
    for i in range(2):
        out[:, i, :] = (perm[i] * P + key_p <= j * QB + col)
        out[:, 2 + i, :] = (perm[8 + i] * P + key_p <= (7 - j) * QB + col)
    return out.astype(bf16)


def _core_sv(j, perm):
    """Per-slot relevance bits. svl[s]: late half of slot s is causally live
    (diagonal late slots 8,9 use masks instead -> 1). sve[s]: early half of
    slot s is fully live (diagonal early slots 0,1 use masks -> 1)."""
    svl = np.zeros(16, np.float32)
    sve = np.zeros(8, np.float32)
    for s in range(16):
        if s in (8, 9) or perm[s] < 14 - 2 * j:
            svl[s] = 1.0
    for s in range(8):
        if s in (0, 1) or perm[s] < 2 * j:
            sve[s] = 1.0
    return svl, sve


def _make_in_maps(inputs):
    x = np.asarray(inputs["x"], np.float32)
    shared = _prep_shared(inputs)
    vecs_base = shared.pop("vecs_base")
    in_maps = []
    for c in range(8):
        gg, j = c // 4, c % 4
        perm = _core_perm(j)
        xb = x[gg]
        xbp = np.concatenate([xb[p * P:(p + 1) * P] for p in perm], 0)
        xq = np.concatenate([xb[j * QB:(j + 1) * QB],
                             xb[(7 - j) * QB:(8 - j) * QB]], 0)
        svl, sve = _core_sv(j, perm)
        m = dict(shared)
        m["xc"] = np.ascontiguousarray(
            np.concatenate([xbp, xq], 0)).astype(bf16)
        m["masks"] = _core_masks(j, perm)
        m["vecs"] = np.ascontiguousarray(np.concatenate([
            vecs_base,
            np.broadcast_to(svl[None, :], (P, 16)),
            np.broadcast_to(sve[None, :], (P, 8)),
            np.broadcast_to(svl[None, :] / 64, (P, 16)),
            np.broadcast_to(sve[None, :] / 64, (P, 8)),
        ], axis=1))
        in_maps.append(m)
    return in_maps


def _assemble(results):
    out = np.zeros((B, T, C), np.float32)
    for c in range(8):
        gg, j = c // 4, c % 4
        o = results[c]["out"].reshape(C, R).T  # [512, C] rows = 2 blocks
        out[gg, j * QB:(j + 1) * QB] = o[:QB]
        out[gg, (7 - j) * QB:(8 - j) * QB] = o[QB:]
    return out


def kernel(**inputs):
    in_maps = _make_in_maps(inputs)
    nc = _get_nc()
    res = run_bass_kernel_spmd(nc, in_maps, core_ids=list(range(8)))
    return _assemble(res.results)

